# revision 31
# baseline (speedup 1.0000x reference)
"""GCMC (gnn_message_passing) Trainium2 Bass kernel, 8-core SPMD, v2.

Strategy (hardcoded for the nn_GCMC_40870908789353 shapes):
- Score-pair sharding: core c owns pairs [1024c, 1024(c+1)). Its 2048 node
  slots (1024 user + 1024 item, duplicates allowed) are the only rows for
  which agg/x2 are computed, so no collective is needed anywhere.
- Gathers use batched dma_gather (SWDGE, int16 idxs) instead of per-chunk
  indirect DMAs: ~30 instructions/core instead of ~850. Tables are split
  into <=32767-row regions (id_emb 3, word_table 4) to fit int16 indices.
- Transposed dataflow: segment-sum one-hot matmuls run as lhsT=payload,
  rhs=one-hot so PSUM holds agg^T / t_feat^T ([dim, slot]); every later
  matmul chains without a single PE transpose. lin_b rides the ACT bias
  port, x1@W and f@w2 accumulate into the same PSUM tile.
- Edge payload rows are L2-normalized on the fly (square/reduce/rsqrt on
  DVE) which folds F.normalize into the gather and kills the full-table
  normalize pass; the scale-mult also casts the payload to bf16.
- Word payload rows are pre-scaled by 1/deg(item slot) (host metadata), so
  the PSUM directly accumulates the mean.
"""
import sys
for p in ("/opt/trn_rl_repo", "/root/.axon_site/_ro/trn_rl_repo"):
    if p not in sys.path:
        sys.path.insert(0, p)
import numpy as np
import ml_dtypes

NC = 8
NUM_USER = 50000
NUM_ITEM = 20000
NNODE = 70000
VOCAB = 100000
DIM = 64
WDIM = 128
B = 8192
BPC = 1024          # pairs per core
NSLOT = 2048        # node slots per core (1024 user + 1024 item)
NT = 16             # node slot tiles (128 slots, for the x-tail)
IT = 8              # item slot tiles (128 slots, for the f-pipeline)
ET = 32             # edge dst groups (64 slots each)
WT = 16             # word dst groups (64 slots each)
SLOTW = 64          # one-hot width per dst group
E_REG_BOUNDS = (0, 25000, 50000, 70000)   # aligned to user/item boundary
NREG_E = 3
REG_W = 25000       # word_table region rows (4 regions)
NREG_W = 4
EB = 36             # edge chunks per dma_gather batch
WB = 26             # word chunks per dma_gather batch
SLOPE = 0.01

_CACHE = {}

bf16 = ml_dtypes.bfloat16


# ---------------------------------------------------------------- CPU prep

def _ragged_gather(starts, lens):
    """positions [starts[i], starts[i]+lens[i]) concatenated."""
    tot = int(lens.sum())
    if tot == 0:
        return np.zeros(0, np.int64)
    cum = np.cumsum(lens) - lens
    return np.repeat(starts - cum, lens) + np.arange(tot)


def _build_stream(slot_rep, val_rep, region_rep, n_tiles, n_reg, extra=None):
    """Per-core stream fill given the instance list (slot, table-local idx,
    region). Returns dict with per-(region,tile) counts and a fill closure.
    """
    key = region_rep * n_tiles + (slot_rep >> 7)
    order = np.argsort(key, kind="stable")
    return order, key[order]


class _Sched:
    """Unified SPMD schedule for one gather family."""

    def __init__(self, cnt, n_tiles, n_reg, batch):
        # cnt: [NC, n_reg, n_tiles] instance counts
        nch = np.ceil(cnt / 128.0).astype(np.int64).max(axis=0)  # [n_reg,n_tiles]
        # every tile needs >=1 chunk overall so start/stop exist
        tile_tot = nch.sum(axis=0)
        for t in range(n_tiles):
            if tile_tot[t] == 0:
                nch[0][t] = 1
        self.nch = nch
        self.n_tiles = n_tiles
        self.n_reg = n_reg
        # global chunk order: region-major, tile-minor
        tiles = []
        regions = []
        for r in range(n_reg):
            for t in range(n_tiles):
                tiles += [t] * int(nch[r][t])
                regions += [r] * int(nch[r][t])
        self.tile_of = np.array(tiles, np.int64)
        self.region_of = np.array(regions, np.int64)
        self.NCH = len(tiles)
        self.S = self.NCH * 128
        # start/stop flags per chunk at (region, tile) GROUP granularity: each
        # group is one PSUM accumulation (own bank) closed within its region.
        self.is_first = []
        self.is_last = []
        for ch in range(len(tiles)):
            r, t = regions[ch], tiles[ch]
            self.is_first.append(ch == 0 or (regions[ch - 1], tiles[ch - 1]) != (r, t))
            self.is_last.append(ch == len(tiles) - 1
                                or (regions[ch + 1], tiles[ch + 1]) != (r, t))
        # group (r,t) -> starting chunk
        self.group_ch0 = np.zeros((n_reg, n_tiles), np.int64)
        ch = 0
        for r in range(n_reg):
            for t in range(n_tiles):
                self.group_ch0[r][t] = ch
                ch += int(nch[r][t])
        # batches: split each region's chunk range into <= batch chunks
        self.batches = []  # (region, ch0, nchunks)
        for r in range(n_reg):
            r0 = int(self.group_ch0[r][0])
            r1 = int(self.group_ch0[r + 1][0]) if r + 1 < n_reg else self.NCH
            ch = r0
            while ch < r1:
                nb = min(batch, r1 - ch)
                # Never cut a batch right after a group's first chunk: a
                # continuing group would then open with a 64-partition single
                # whose start=True clears has_written only for partitions
                # 0-63, leaving the pairs' q11 half to accumulate onto the
                # PSUM slot's stale contents.
                if ch + nb < r1:
                    last = ch + nb - 1
                    if self.is_first[last] and not self.is_last[last]:
                        nb -= 1
                self.batches.append((r, ch, nb))
                ch += nb

    def key(self):
        return (self.n_tiles, self.n_reg) + tuple(self.nch.ravel().tolist())


def _fill_stream(sched, slot_rep, loc_val, region_rep, scale=None):
    """Place instances into the padded stream. Returns (idx_stream int16,
    loc_stream bf16, scale_stream bf16 or None)."""
    n_tiles = sched.n_tiles
    key = region_rep * n_tiles + (slot_rep >> 6)
    order = np.argsort(key, kind="stable")
    skey = key[order]
    gcnt = np.bincount(skey, minlength=sched.n_reg * n_tiles)
    # position of each sorted instance: group base*128 + within-group offset
    ch0 = sched.group_ch0.ravel()
    base = np.repeat(ch0 * 128, gcnt)
    within = np.arange(len(order)) - np.repeat(np.cumsum(gcnt) - gcnt, gcnt)
    pos = base + within
    idx_stream = np.zeros(sched.S, np.int16)
    idx_stream[pos] = loc_val[order].astype(np.int16)
    loc_stream = np.full(sched.S, -1.0, bf16)
    loc_stream[pos] = (slot_rep[order] & 63).astype(bf16)
    sc_stream = None
    if scale is not None:
        sc_stream = np.zeros(sched.S, bf16)
        sc_stream[pos] = scale[order].astype(bf16)
    return idx_stream, loc_stream, sc_stream


def _wrap_idx(idx_stream):
    """[S] int16 -> [128, S/16] wrapped+replicated layout."""
    S = idx_stream.shape[0]
    base = idx_stream.reshape(S // 16, 16).T  # [16, S/16]
    return np.ascontiguousarray(np.tile(base, (8, 1)))


def _per_chunk(stream):
    """[S] -> [128, NCH]: position i=(ch*128+p) -> [p, ch]."""
    NCH = stream.shape[0] // 128
    return np.ascontiguousarray(stream.reshape(NCH, 128).T)


def _prep(inputs):
    edge_index = np.asarray(inputs["edge_index"])
    words_tensor = np.asarray(inputs["words_tensor"])
    user_nodes = np.asarray(inputs["user_nodes"]).astype(np.int64)
    item_nodes = np.asarray(inputs["item_nodes"]).astype(np.int64)

    src = edge_index[0].astype(np.int64)
    dst = edge_index[1].astype(np.int64)
    witem = words_tensor[0].astype(np.int64)
    wword = words_tensor[1].astype(np.int64)

    eorder = np.argsort(dst, kind="stable")
    sdst = dst[eorder]
    ssrc = src[eorder]
    worder = np.argsort(witem, kind="stable")
    switem_srt = witem[worder]
    swword = wword[worder]

    deg = np.bincount(dst, minlength=NNODE)
    wc_item = np.bincount(witem, minlength=NUM_ITEM)

    def snake_pos(n):
        i = np.arange(n)
        rnd, lane = divmod(i, 16)
        g = np.where(rnd % 2 == 0, lane, 15 - lane)
        return g * 64 + rnd

    # cluster pairs by item: each item's aggregation lands on one core
    gorder = np.argsort(item_nodes, kind="stable")

    e_data, w_data = [], []
    outperm = np.zeros((NC, BPC), np.int64)
    sidx = np.zeros((NC, 128, BPC // 16), np.int16)
    cnt_e = np.zeros((NC, NREG_E, ET), np.int64)
    cnt_w = np.zeros((NC, NREG_W, WT), np.int64)
    vfT = np.zeros((NC, WDIM, BPC), bf16)
    v_feat = np.asarray(inputs["v_feat"], np.float32)
    for c in range(NC):
        P = gorder[c * BPC:(c + 1) * BPC]
        users = user_nodes[P]
        items = item_nodes[P]
        # user position permutation (balance by degree, snake)
        order_u = np.argsort(-deg[users], kind="stable")
        pos_u = snake_pos(BPC)
        uperm = np.empty(BPC, np.int64)       # uperm[position] = pair rank in P
        uperm[pos_u] = order_u
        outperm[c] = P[uperm]
        # unique items -> balanced slot positions
        uit = np.unique(items)                # sorted node ids
        nu = len(uit)
        iid = uit - NUM_USER
        order_i = np.argsort(-(deg[uit] + wc_item[iid]), kind="stable")
        # rank r (in uit order) -> its balance order index, then snake position
        inv = np.empty(nu, np.int64)
        inv[order_i] = np.arange(nu)
        ipos_of_rank = snake_pos(nu)[inv]
        # final-score gather: position q -> item slot position
        islot_of_pair = ipos_of_rank[np.searchsorted(uit, items[uperm])]
        st16 = islot_of_pair.astype(np.int16)
        sidx[c] = np.ascontiguousarray(
            np.tile(st16.reshape(BPC // 16, 16).T, (8, 1)))

        # edge instances: user positions + unique-item slots
        nodes_e = np.concatenate([users[uperm], uit])
        slots_e = np.concatenate([np.arange(BPC), BPC + ipos_of_rank])
        st = np.searchsorted(sdst, nodes_e)
        en = np.searchsorted(sdst, nodes_e, side="right")
        lens = en - st
        slot_rep = np.repeat(slots_e, lens)
        src_rep = ssrc[_ragged_gather(st, lens)]
        reg_rep = np.searchsorted(np.array(E_REG_BOUNDS[1:-1]), src_rep,
                                  side="right")
        loc_rep = src_rep - np.array(E_REG_BOUNDS)[reg_rep]
        np.add.at(cnt_e[c], (reg_rep, slot_rep >> 6), 1)
        e_data.append((slot_rep, loc_rep, reg_rep))

        # word instances per unique item
        wst = np.searchsorted(switem_srt, iid)
        wen = np.searchsorted(switem_srt, iid, side="right")
        wlens = wen - wst
        wslot_rep = np.repeat(ipos_of_rank, wlens)
        word_rep = swword[_ragged_gather(wst, wlens)]
        wreg_rep = word_rep // REG_W
        wloc_rep = word_rep - wreg_rep * REG_W
        np.add.at(cnt_w[c], (wreg_rep, wslot_rep >> 6), 1)
        winv = (1.0 / np.maximum(wlens, 1)).astype(np.float32)
        wscale_rep = np.repeat(winv, wlens)
        w_data.append((wslot_rep, wloc_rep, wreg_rep, wscale_rep))

        vf_pos = np.zeros((BPC, WDIM), np.float32)
        vf_pos[ipos_of_rank] = v_feat[iid]
        vfT[c] = vf_pos.T.astype(bf16)

    es = _Sched(cnt_e, ET, NREG_E, EB)
    ws = _Sched(cnt_w, WT, NREG_W, WB)

    eidx = np.zeros((NC, 128, es.S // 16), np.int16)
    eloc = np.zeros((NC, 128, es.NCH), bf16)
    widx = np.zeros((NC, 128, ws.S // 16), np.int16)
    wloc = np.zeros((NC, 128, ws.NCH), bf16)
    wsc = np.zeros((NC, 128, ws.NCH), bf16)
    for c in range(NC):
        slot_rep, loc_rep, reg_rep = e_data[c]
        i_s, l_s, _ = _fill_stream(es, slot_rep, loc_rep, reg_rep)
        eidx[c] = _wrap_idx(i_s)
        eloc[c] = _per_chunk(l_s)
        wslot_rep, wloc_rep, wreg_rep, wscale_rep = w_data[c]
        i_s, l_s, s_s = _fill_stream(ws, wslot_rep, wloc_rep, wreg_rep,
                                     scale=wscale_rep)
        widx[c] = _wrap_idx(i_s)
        wloc[c] = _per_chunk(l_s)
        wsc[c] = _per_chunk(s_s)

    return dict(es=es, ws=ws, eidx=eidx, eloc=eloc,
                widx=widx, wloc=wloc, wsc=wsc, vfT=vfT,
                sidx=sidx, outperm=outperm)


# ------------------------------------------------------------- bass program

def _build_program(es, ws):
    from concourse import bass, bacc, mybir
    import concourse.tile as tile
    dt = mybir.dt

    nc = bacc.Bacc(None, target_bir_lowering=False, num_swdge_queues=4)
    f32 = dt.float32
    bf = dt.bfloat16

    id_in = nc.dram_tensor("id_emb", [NNODE, DIM], f32, kind="ExternalInput")
    wt_in = nc.dram_tensor("wt_bf", [VOCAB, WDIM], bf, kind="ExternalInput")
    eidx_in = nc.dram_tensor("eidx", [128, es.S // 16], dt.int16, kind="ExternalInput")
    eloc_in = nc.dram_tensor("eloc", [128, es.NCH], bf, kind="ExternalInput")
    widx_in = nc.dram_tensor("widx", [128, ws.S // 16], dt.int16, kind="ExternalInput")
    wloc_in = nc.dram_tensor("wloc", [128, ws.NCH], bf, kind="ExternalInput")
    wsc_in = nc.dram_tensor("wsc", [128, ws.NCH], bf, kind="ExternalInput")
    vfT_in = nc.dram_tensor("vfT", [WDIM, BPC], bf, kind="ExternalInput")
    cw_in = nc.dram_tensor("cw_bf", [DIM, DIM], bf, kind="ExternalInput")
    ww_in = nc.dram_tensor("ww_bf", [DIM, DIM], bf, kind="ExternalInput")
    w2_in = nc.dram_tensor("w2_bf", [DIM, DIM], bf, kind="ExternalInput")
    lw_in = nc.dram_tensor("lw_bf", [2 * WDIM, DIM], bf, kind="ExternalInput")
    lb_in = nc.dram_tensor("lb_col", [DIM, 1], f32, kind="ExternalInput")
    ident_in = nc.dram_tensor("ident", [128, 128], f32, kind="ExternalInput")
    sidx_in = nc.dram_tensor("sidx", [128, BPC // 16], dt.int16, kind="ExternalInput")
    iota_in = nc.dram_tensor("iota_bf", [128, 128], bf, kind="ExternalInput")
    out = nc.dram_tensor("scores_w", [128, 8], f32, kind="ExternalOutput")
    x2i_dram = nc.dram_tensor("x2i", [BPC, DIM], f32)
    import os
    DBG = os.environ.get("KDBG") == "1"
    if DBG:
        dbg_agg = nc.dram_tensor("dbg_agg", [DIM, NT * 128], f32, kind="ExternalOutput")
        dbg_tf = nc.dram_tensor("dbg_tf", [WDIM, IT * 128], f32, kind="ExternalOutput")
        dbg_x2 = nc.dram_tensor("dbg_x2", [DIM, NT * 128], f32, kind="ExternalOutput")
        dbg_ipay = nc.dram_tensor("dbg_ipay", [128, IT * DIM], f32, kind="ExternalOutput")

    id_regions = [(E_REG_BOUNDS[i], E_REG_BOUNDS[i + 1]) for i in range(3)]
    wt_regions = [(r * REG_W, (r + 1) * REG_W) for r in range(NREG_W)]

    with tile.TileContext(nc) as tc:
        with tc.tile_pool(name="const", bufs=1) as cpool, \
             tc.tile_pool(name="persist", bufs=1) as pp, \
             tc.tile_pool(name="ewp", bufs=4) as ewp, \
             tc.tile_pool(name="wwp", bufs=4) as wwp, \
             tc.tile_pool(name="mid", bufs=2) as midp, \
             tc.tile_pool(name="xp", bufs=2) as xp, \
             tc.tile_pool(name="psw", bufs=2, space="PSUM") as psw, \
             tc.tile_pool(name="pse", bufs=2, space="PSUM") as pse, \
             tc.tile_pool(name="psm", bufs=2, space="PSUM") as psm:

            iota = cpool.tile([128, 128], bf)
            cw = cpool.tile([DIM, DIM], bf)
            ww = cpool.tile([DIM, DIM], bf)
            w2 = cpool.tile([DIM, DIM], bf)
            lw = cpool.tile([128, 2 * DIM], bf)   # cols 0:64 = v-half, 64:128 = t-half
            lb = cpool.tile([DIM, 1], f32)
            ident = cpool.tile([128, 128], f32)
            sidx_sb = cpool.tile([128, BPC // 16], dt.int16)
            nc.sync.dma_start(out=iota[:], in_=iota_in[:])
            nc.sync.dma_start(out=cw[:], in_=cw_in[:])
            nc.sync.dma_start(out=ww[:], in_=ww_in[:])
            nc.sync.dma_start(out=w2[:], in_=w2_in[:])
            nc.sync.dma_start(out=lw[:, 0:DIM], in_=lw_in[0:128, :])
            nc.sync.dma_start(out=lw[:, DIM:2 * DIM], in_=lw_in[128:256, :])
            nc.sync.dma_start(out=lb[:], in_=lb_in[:])
            nc.sync.dma_start(out=ident[:], in_=ident_in[:])
            nc.sync.dma_start(out=sidx_sb[:], in_=sidx_in[:])

            eidx_sb = pp.tile([128, es.S // 16], dt.int16)
            eloc_sb = pp.tile([128, es.NCH], bf)
            widx_sb = pp.tile([128, ws.S // 16], dt.int16)
            wloc_sb = pp.tile([128, ws.NCH], bf)
            wsc_sb = pp.tile([128, ws.NCH], bf)
            vfT_sb = pp.tile([WDIM, BPC], bf)
            nc.sync.dma_start(out=eidx_sb[:], in_=eidx_in[:])
            nc.sync.dma_start(out=eloc_sb[:], in_=eloc_in[:])
            nc.sync.dma_start(out=widx_sb[:], in_=widx_in[:])
            nc.sync.dma_start(out=wloc_sb[:], in_=wloc_in[:])
            nc.sync.dma_start(out=wsc_sb[:], in_=wsc_in[:])
            nc.sync.dma_start(out=vfT_sb[:], in_=vfT_in[:])

            tfT_sb = pp.tile([WDIM, IT * 128], bf)
            fT_sb = pp.tile([DIM, IT * 128], bf)
            x2T_sb = pp.tile([DIM, NT * 128], f32)
            tfsum_sb = pp.tile([WDIM, IT * 128], f32)
            agg_sb = pp.tile([DIM, NT * 128], f32)
            nc.vector.memset(tfsum_sb[:], 0.0)
            nc.vector.memset(agg_sb[:], 0.0)

            # ---- words: t_feat^T accumulation ----
            wps = None
            for wq, (r, ch0, nb) in enumerate(ws.batches):
                r0, r1 = wt_regions[r]
                wpay = wwp.tile([128, WB * WDIM], bf, tag="wpay")
                pay3 = wpay[:].rearrange("p (k d) -> p k d", d=WDIM)
                nc.gpsimd.dma_gather(
                    wpay[:, 0:nb * WDIM].rearrange("p (k d) -> p k d", d=WDIM),
                    wt_in[r0:r1, :],
                    widx_sb[:, ch0 * 8:(ch0 + nb) * 8],
                    nb * 128, nb * 128, WDIM, single_packet=False,
                    queue_num=wq % 4)
                wpays = midp.tile([128, WB * WDIM], bf, tag="wpays")
                pays3 = wpays[:].rearrange("p (k d) -> p k d", d=WDIM)
                nc.vector.tensor_tensor(
                    out=pays3[:, 0:nb, :], in0=pay3[:, 0:nb, :],
                    in1=wsc_sb[:, ch0:ch0 + nb][:, :, None].to_broadcast(
                        [128, nb, WDIM]),
                    op=mybir.AluOpType.mult)
                woh = wwp.tile([128, WB * SLOTW], bf, tag="woh")
                oh3 = woh[:].rearrange("p (k d) -> p k d", d=SLOTW)
                nc.vector.tensor_tensor(
                    out=oh3[:, 0:nb, :],
                    in0=wloc_sb[:, ch0:ch0 + nb][:, :, None].to_broadcast(
                        [128, nb, SLOTW]),
                    in1=iota[:][:, None, 0:SLOTW].to_broadcast([128, nb, SLOTW]),
                    op=mybir.AluOpType.is_equal)
                for k in range(nb):
                    ch = ch0 + k
                    t = int(ws.tile_of[ch])
                    if ws.is_first[ch]:
                        wps = psw.tile([WDIM, 512], f32, tag="wp")
                    nc.tensor.matmul(
                        out=wps[:, 0:SLOTW], lhsT=pays3[:, k, :], rhs=oh3[:, k, :],
                        start=ws.is_first[ch], stop=ws.is_last[ch])
                    if ws.is_last[ch]:
                        sl = tfsum_sb[:, t * SLOTW:(t + 1) * SLOTW]
                        nc.vector.tensor_tensor(out=sl, in0=sl,
                                                in1=wps[:, 0:SLOTW],
                                                op=mybir.AluOpType.add)

            for t in range(IT):
                nc.scalar.activation(
                    tfT_sb[:, t * 128:(t + 1) * 128],
                    tfsum_sb[:, t * 128:(t + 1) * 128],
                    mybir.ActivationFunctionType.Copy)

            # ---- f^T = lrelu(lw^T cat^T + lb); fh feeds item-tile x2 ----
            for t in range(IT):
                fp = psm.tile([DIM, 512], f32, tag="mm")
                nc.tensor.matmul(out=fp[:, 0:128], lhsT=lw[:, 0:DIM],
                                 rhs=vfT_sb[:, t * 128:(t + 1) * 128],
                                 start=True, stop=False)
                nc.tensor.matmul(out=fp[:, 0:128], lhsT=lw[:, DIM:2 * DIM],
                                 rhs=tfT_sb[:, t * 128:(t + 1) * 128],
                                 start=False, stop=True)
                nc.scalar.activation(
                    fT_sb[:, t * 128:(t + 1) * 128], fp[:, 0:128],
                    mybir.ActivationFunctionType.Lrelu,
                    bias=lb[:], alpha=SLOPE)

            # ---- edges: agg^T accumulation with on-the-fly normalize ----
            es_has_pair = set()
            for (_r, _c0, _nb) in es.batches:
                _k = 0
                while _k < _nb:
                    _ch = _c0 + _k
                    if (_k + 1 < _nb) and not es.is_first[_ch + 1]:
                        es_has_pair.add((int(es.region_of[_ch]),
                                         int(es.tile_of[_ch])))
                        _k += 2
                    else:
                        _k += 1
            if True:
              eps = None
              for eq, (r, ch0, nb) in enumerate(es.batches):
                r0, r1 = id_regions[r]
                epay = ewp.tile([128, EB * DIM], f32, tag="epay")
                pay3 = epay[:].rearrange("p (k d) -> p k d", d=DIM)
                nc.gpsimd.dma_gather(
                    epay[:, 0:nb * DIM].rearrange("p (k d) -> p k d", d=DIM),
                    id_in[r0:r1, :],
                    eidx_sb[:, ch0 * 8:(ch0 + nb) * 8],
                    nb * 128, nb * 128, DIM, single_packet=False,
                    queue_num=eq % 4)
                esq = midp.tile([128, EB * DIM], f32, tag="esq")
                sq3 = esq[:].rearrange("p (k d) -> p k d", d=DIM)
                nc.vector.tensor_tensor(out=sq3[:, 0:nb, :], in0=pay3[:, 0:nb, :],
                                        in1=pay3[:, 0:nb, :],
                                        op=mybir.AluOpType.mult)
                ss = ewp.tile([128, EB], f32, tag="ess")
                nc.vector.reduce_sum(out=ss[:, 0:nb], in_=sq3[:, 0:nb, :],
                                     axis=mybir.AxisListType.X)
                nc.scalar.sqrt(ss[:, 0:nb], ss[:, 0:nb])
                nc.vector.reciprocal(ss[:, 0:nb], ss[:, 0:nb])
                epayb = ewp.tile([128, EB * DIM], bf, tag="epayb")
                payb3 = epayb[:].rearrange("p (k d) -> p k d", d=DIM)
                nc.vector.tensor_tensor(
                    out=payb3[:, 0:nb, :], in0=pay3[:, 0:nb, :],
                    in1=ss[:, 0:nb][:, :, None].to_broadcast([128, nb, DIM]),
                    op=mybir.AluOpType.mult)
                eoh = ewp.tile([128, EB * SLOTW], bf, tag="eoh")
                oh3 = eoh[:].rearrange("p (k d) -> p k d", d=SLOTW)
                nc.vector.tensor_tensor(
                    out=oh3[:, 0:nb, :],
                    in0=eloc_sb[:, ch0:ch0 + nb][:, :, None].to_broadcast(
                        [128, nb, SLOTW]),
                    in1=iota[:][:, None, 0:SLOTW].to_broadcast([128, nb, SLOTW]),
                    op=mybir.AluOpType.is_equal)
                k = 0
                while k < nb:
                    ch = ch0 + k
                    t = int(es.tile_of[ch])
                    if es.is_first[ch]:
                        eps = pse.tile([128, 512], f32, tag="ep")
                    pair = (k + 1 < nb) and not es.is_first[ch + 1]
                    if pair:
                        stop = es.is_last[ch + 1]
                        nc.tensor.matmul(
                            out=eps[:, 0:128],
                            lhsT=epayb[:, k * DIM:(k + 2) * DIM],
                            rhs=eoh[:, k * SLOTW:(k + 2) * SLOTW],
                            start=es.is_first[ch], stop=stop)
                        k += 2
                    else:
                        stop = es.is_last[ch]
                        nc.tensor.matmul(
                            out=eps[0:DIM, 0:SLOTW],
                            lhsT=epayb[:, k * DIM:(k + 1) * DIM],
                            rhs=eoh[:, k * SLOTW:(k + 1) * SLOTW],
                            start=es.is_first[ch], stop=stop)
                        k += 1
                    if stop:
                        g = (int(es.region_of[ch]), t)
                        sl = agg_sb[:, t * SLOTW:(t + 1) * SLOTW]
                        nc.vector.tensor_tensor(out=sl, in0=sl,
                                                in1=eps[0:DIM, 0:SLOTW],
                                                op=mybir.AluOpType.add)
                        if g in es_has_pair:
                            nc.vector.tensor_tensor(
                                out=sl, in0=sl,
                                in1=eps[DIM:128, SLOTW:128],
                                op=mybir.AluOpType.add)

              # ---- node tail: x2^T = lrelu(ww^T x1^T (+ w2^T f^T)) ----
              for t in range(NT):
                aggT = xp.tile([DIM, 128], bf, tag="aggT")
                nc.scalar.activation(aggT[:], agg_sb[:, t * 128:(t + 1) * 128],
                                     mybir.ActivationFunctionType.Copy)
                x1p = psm.tile([DIM, 512], f32, tag="mm")
                nc.tensor.matmul(out=x1p[:, 0:128], lhsT=cw[:], rhs=aggT[:],
                                 start=True, stop=True)
                x1T = xp.tile([DIM, 128], bf, tag="x1T")
                nc.scalar.activation(x1T[:], x1p[:, 0:128],
                                     mybir.ActivationFunctionType.Lrelu,
                                     alpha=SLOPE)
                x2p = psm.tile([DIM, 512], f32, tag="mm")
                nc.tensor.matmul(out=x2p[:, 0:128], lhsT=ww[:], rhs=x1T[:],
                                 start=True, stop=(t < IT))
                if t >= IT:
                    ti = t - IT
                    nc.tensor.matmul(out=x2p[:, 0:128], lhsT=w2[:],
                                     rhs=fT_sb[:, ti * 128:(ti + 1) * 128],
                                     start=False, stop=True)
                nc.scalar.activation(x2T_sb[:, t * 128:(t + 1) * 128], x2p[:, 0:128],
                                     mybir.ActivationFunctionType.Lrelu,
                                     alpha=SLOPE)

            # ---- scores: transpose x2^T tiles to rows, route item rows ----
            x2r_u = pp.tile([128, IT * DIM], f32)
            x2r_i = pp.tile([128, IT * DIM], f32)
            for t in range(IT):
                ps_t = psm.tile([128, 512], f32, tag="tr")
                nc.tensor.transpose(out=ps_t[:, 0:DIM],
                                    in_=x2T_sb[:, t * 128:(t + 1) * 128],
                                    identity=ident[0:DIM, 0:DIM])
                nc.scalar.activation(x2r_u[:, t * DIM:(t + 1) * DIM], ps_t[:, 0:DIM],
                                     mybir.ActivationFunctionType.Copy)
            for t in range(IT):
                ps_t = psm.tile([128, 512], f32, tag="tr")
                nc.tensor.transpose(out=ps_t[:, 0:DIM],
                                    in_=x2T_sb[:, (IT + t) * 128:(IT + t + 1) * 128],
                                    identity=ident[0:DIM, 0:DIM])
                nc.scalar.activation(x2r_i[:, t * DIM:(t + 1) * DIM], ps_t[:, 0:DIM],
                                     mybir.ActivationFunctionType.Copy)
            nc.sync.dma_start(
                out=x2i_dram[:, :].rearrange("(t p) d -> p t d", p=128),
                in_=x2r_i[:].rearrange("p (t d) -> p t d", d=DIM))
            ipay = pp.tile([128, IT * DIM], f32)
            nc.gpsimd.dma_gather(
                ipay[:].rearrange("p (k d) -> p k d", d=DIM),
                x2i_dram[:, :],
                sidx_sb[:],
                BPC, BPC, DIM, single_packet=False)
            prod = pp.tile([128, IT * DIM], f32)
            nc.vector.tensor_tensor(
                out=prod[:].rearrange("p (k d) -> p k d", d=DIM),
                in0=x2r_u[:].rearrange("p (k d) -> p k d", d=DIM),
                in1=ipay[:].rearrange("p (k d) -> p k d", d=DIM),
                op=mybir.AluOpType.mult)
            sc = pp.tile([128, 8], f32)
            nc.vector.reduce_sum(out=sc[:],
                                 in_=prod[:].rearrange("p (k d) -> p k d", d=DIM),
                                 axis=mybir.AxisListType.X)
            nc.sync.dma_start(out=out[:], in_=sc[:])
            if DBG:
                nc.sync.dma_start(out=dbg_agg[:], in_=agg_sb[:])
                nc.sync.dma_start(out=dbg_tf[:], in_=tfsum_sb[:])
                nc.sync.dma_start(out=dbg_x2[:], in_=x2T_sb[:])
                nc.sync.dma_start(out=dbg_ipay[:], in_=ipay[:])

    nc.finalize()
    return nc


# ------------------------------------------------------------------- kernel

def kernel(**inputs):
    from concourse.bass_utils import run_bass_kernel_spmd

    pr = _prep(inputs)
    es, ws = pr["es"], pr["ws"]
    key = es.key() + ws.key()
    if key not in _CACHE:
        _CACHE[key] = _build_program(es, ws)
    nc = _CACHE[key]

    iota_bf = np.broadcast_to(np.arange(128, dtype=bf16), (128, 128)).copy()
    ident = np.eye(128, dtype=np.float32)
    wt_bf = np.asarray(inputs["word_table"], np.float32).astype(bf16)
    lb_col = np.asarray(inputs["lin_b"], np.float32).reshape(DIM, 1).copy()
    cw_bf = np.asarray(inputs["conv_weight"], np.float32).astype(bf16)
    ww_bf = np.asarray(inputs["weight_W"], np.float32).astype(bf16)
    w2_bf = np.asarray(inputs["weight_2"], np.float32).astype(bf16)
    lw_bf = np.asarray(inputs["lin_w"], np.float32).astype(bf16)
    id_emb = np.ascontiguousarray(np.asarray(inputs["id_embedding"], np.float32))

    in_maps = []
    for c in range(NC):
        in_maps.append({
            "id_emb": id_emb,
            "wt_bf": wt_bf,
            "eidx": pr["eidx"][c],
            "eloc": pr["eloc"][c],
            "widx": pr["widx"][c],
            "wloc": pr["wloc"][c],
            "wsc": pr["wsc"][c],
            "vfT": pr["vfT"][c],
            "cw_bf": cw_bf,
            "ww_bf": ww_bf,
            "w2_bf": w2_bf,
            "lw_bf": lw_bf,
            "lb_col": lb_col,
            "ident": ident,
            "sidx": pr["sidx"][c],
            "iota_bf": iota_bf,
        })
    res = run_bass_kernel_spmd(nc, in_maps, list(range(NC)))
    scores = np.empty(B, np.float32)
    for c in range(NC):
        w = res.results[c]["scores_w"]           # [128, 8]
        sc = np.asarray(w, np.float32).T.ravel()  # sc[position]
        scores[pr["outperm"][c]] = sc
    return scores


kernel.run_traced = None  # set by test harness if needed


# revision 32
# speedup vs baseline: 1.0248x; 1.0248x over previous
"""GCMC (gnn_message_passing) Trainium2 Bass kernel, 8-core SPMD.

Measured: 536.7 us HW exec (baseline 1793.4 us), rel L2 err ~2e-3.

Strategy (hardcoded for the nn_GCMC_40870908789353 shapes):
- Pairs are sorted by item and sharded in blocks of 1024 per core, so each
  sampled item's aggregation+words land on exactly one core and are computed
  once (global dedup floor, ~105k gathered rows/core). No collectives: the
  final scores read item x2 rows through one tiny on-core dma_gather.
- All sparse reads use batched dma_gather (SWDGE int16 idxs) on 4 SWDGE
  queues (num_swdge_queues=4; ~4.4 ns/row vs ~9 ns serialized); tables are
  split into <=32767-row regions (id_emb split at the user/item boundary so
  user-dst groups only touch the item region and vice versa).
- Segment-sum runs as one-hot matmuls in transposed form (lhsT=payload,
  rhs=one-hot -> PSUM agg^T/t_feat^T), adjacent same-group edge chunks are
  PAIRED into one [128x128]x[128x128] matmul (diagonal quadrants used, the
  off-diagonal garbage never read). Each (region, dst-group) accumulation
  owns a full PSUM bank, closed within its region pass, then DVE-added into
  SBUF accumulators (matmul start=True clears has_written for the WHOLE
  bank, and only for partitions it writes -- both constraints shaped the
  schedule; batches never cut right after a group's first chunk).
- Edge payload rows are L2-normalized on the fly (square/reduce/rsqrt) which
  folds F.normalize into the gather; the scale-mult also casts to bf16.
  Word payloads are bf16 host-cast and pre-scaled by 1/wordcount(item).
- lin_b rides the ACT bias port; x1@W and f@w2 share one PSUM accumulation.
"""
import sys
for p in ("/opt/trn_rl_repo", "/root/.axon_site/_ro/trn_rl_repo"):
    if p not in sys.path:
        sys.path.insert(0, p)
import numpy as np
import ml_dtypes

NC = 8
NUM_USER = 50000
NUM_ITEM = 20000
NNODE = 70000
VOCAB = 100000
DIM = 64
WDIM = 128
B = 8192
BPC = 1024          # pairs per core
NSLOT = 2048        # node slots per core (1024 user + 1024 item)
NT = 16             # node slot tiles (128 slots, for the x-tail)
IT = 8              # item slot tiles (128 slots, for the f-pipeline)
ET = 32             # edge dst groups (64 slots each)
WT = 16             # word dst groups (64 slots each)
SLOTW = 64          # one-hot width per dst group
E_REG_BOUNDS = (0, 25000, 50000, 70000)   # aligned to user/item boundary
NREG_E = 3
REG_W = 25000       # word_table region rows (4 regions)
NREG_W = 4
EB = 32             # edge chunks per dma_gather batch
WB = 24             # word chunks per dma_gather batch
SLOPE = 0.01

_CACHE = {}

bf16 = ml_dtypes.bfloat16


# ---------------------------------------------------------------- CPU prep

def _ragged_gather(starts, lens):
    """positions [starts[i], starts[i]+lens[i]) concatenated."""
    tot = int(lens.sum())
    if tot == 0:
        return np.zeros(0, np.int64)
    cum = np.cumsum(lens) - lens
    return np.repeat(starts - cum, lens) + np.arange(tot)


class _Sched:
    """Unified SPMD schedule for one gather family."""

    def __init__(self, cnt, n_tiles, n_reg, batch):
        # cnt: [NC, n_reg, n_tiles] instance counts
        nch = np.ceil(cnt / 128.0).astype(np.int64).max(axis=0)  # [n_reg,n_tiles]
        # every tile needs >=1 chunk overall so start/stop exist
        tile_tot = nch.sum(axis=0)
        for t in range(n_tiles):
            if tile_tot[t] == 0:
                nch[0][t] = 1
        self.nch = nch
        self.n_tiles = n_tiles
        self.n_reg = n_reg
        # global chunk order: region-major, tile-minor
        tiles = []
        regions = []
        for r in range(n_reg):
            for t in range(n_tiles):
                tiles += [t] * int(nch[r][t])
                regions += [r] * int(nch[r][t])
        self.tile_of = np.array(tiles, np.int64)
        self.region_of = np.array(regions, np.int64)
        self.NCH = len(tiles)
        self.S = self.NCH * 128
        # start/stop flags per chunk at (region, tile) GROUP granularity: each
        # group is one PSUM accumulation (own bank) closed within its region.
        self.is_first = []
        self.is_last = []
        for ch in range(len(tiles)):
            r, t = regions[ch], tiles[ch]
            self.is_first.append(ch == 0 or (regions[ch - 1], tiles[ch - 1]) != (r, t))
            self.is_last.append(ch == len(tiles) - 1
                                or (regions[ch + 1], tiles[ch + 1]) != (r, t))
        # group (r,t) -> starting chunk
        self.group_ch0 = np.zeros((n_reg, n_tiles), np.int64)
        ch = 0
        for r in range(n_reg):
            for t in range(n_tiles):
                self.group_ch0[r][t] = ch
                ch += int(nch[r][t])
        # batches: split each region's chunk range into <= batch chunks
        self.batches = []  # (region, ch0, nchunks)
        for r in range(n_reg):
            r0 = int(self.group_ch0[r][0])
            r1 = int(self.group_ch0[r + 1][0]) if r + 1 < n_reg else self.NCH
            ch = r0
            while ch < r1:
                nb = min(batch, r1 - ch)
                # Never cut a batch right after a group's first chunk: a
                # continuing group would then open with a 64-partition single
                # whose start=True clears has_written only for partitions
                # 0-63, leaving the pairs' q11 half to accumulate onto the
                # PSUM slot's stale contents.
                if ch + nb < r1:
                    last = ch + nb - 1
                    if self.is_first[last] and not self.is_last[last]:
                        nb -= 1
                self.batches.append((r, ch, nb))
                ch += nb

    def key(self):
        return (self.n_tiles, self.n_reg) + tuple(self.nch.ravel().tolist())


def _fill_stream(sched, slot_rep, loc_val, region_rep, scale=None):
    """Place instances into the padded stream. Returns (idx_stream int16,
    loc_stream bf16, scale_stream bf16 or None)."""
    n_tiles = sched.n_tiles
    key = region_rep * n_tiles + (slot_rep >> 6)
    order = np.argsort(key, kind="stable")
    skey = key[order]
    gcnt = np.bincount(skey, minlength=sched.n_reg * n_tiles)
    # position of each sorted instance: group base*128 + within-group offset
    ch0 = sched.group_ch0.ravel()
    base = np.repeat(ch0 * 128, gcnt)
    within = np.arange(len(order)) - np.repeat(np.cumsum(gcnt) - gcnt, gcnt)
    pos = base + within
    idx_stream = np.zeros(sched.S, np.int16)
    idx_stream[pos] = loc_val[order].astype(np.int16)
    loc_stream = np.full(sched.S, -1.0, bf16)
    loc_stream[pos] = (slot_rep[order] & 63).astype(bf16)
    sc_stream = None
    if scale is not None:
        sc_stream = np.zeros(sched.S, bf16)
        sc_stream[pos] = scale[order].astype(bf16)
    return idx_stream, loc_stream, sc_stream


def _wrap_idx(idx_stream):
    """[S] int16 -> [128, S/16] wrapped+replicated layout."""
    S = idx_stream.shape[0]
    base = idx_stream.reshape(S // 16, 16).T  # [16, S/16]
    return np.ascontiguousarray(np.tile(base, (8, 1)))


def _per_chunk(stream):
    """[S] -> [128, NCH]: position i=(ch*128+p) -> [p, ch]."""
    NCH = stream.shape[0] // 128
    return np.ascontiguousarray(stream.reshape(NCH, 128).T)


def _prep(inputs):
    edge_index = np.asarray(inputs["edge_index"])
    words_tensor = np.asarray(inputs["words_tensor"])
    user_nodes = np.asarray(inputs["user_nodes"]).astype(np.int64)
    item_nodes = np.asarray(inputs["item_nodes"]).astype(np.int64)

    src = edge_index[0].astype(np.int64)
    dst = edge_index[1].astype(np.int64)
    witem = words_tensor[0].astype(np.int64)
    wword = words_tensor[1].astype(np.int64)

    eorder = np.argsort(dst, kind="stable")
    sdst = dst[eorder]
    ssrc = src[eorder]
    worder = np.argsort(witem, kind="stable")
    switem_srt = witem[worder]
    swword = wword[worder]

    deg = np.bincount(dst, minlength=NNODE)
    wc_item = np.bincount(witem, minlength=NUM_ITEM)

    def snake_pos(n):
        i = np.arange(n)
        rnd, lane = divmod(i, 16)
        g = np.where(rnd % 2 == 0, lane, 15 - lane)
        return g * 64 + rnd

    # cluster pairs by item: each item's aggregation lands on one core
    gorder = np.argsort(item_nodes, kind="stable")

    e_data, w_data = [], []
    outperm = np.zeros((NC, BPC), np.int64)
    sidx = np.zeros((NC, 128, BPC // 16), np.int16)
    cnt_e = np.zeros((NC, NREG_E, ET), np.int64)
    cnt_w = np.zeros((NC, NREG_W, WT), np.int64)
    vfT = np.zeros((NC, WDIM, BPC), bf16)
    v_feat = np.asarray(inputs["v_feat"], np.float32)
    for c in range(NC):
        P = gorder[c * BPC:(c + 1) * BPC]
        users = user_nodes[P]
        items = item_nodes[P]
        # user position permutation (balance by degree, snake)
        order_u = np.argsort(-deg[users], kind="stable")
        pos_u = snake_pos(BPC)
        uperm = np.empty(BPC, np.int64)       # uperm[position] = pair rank in P
        uperm[pos_u] = order_u
        outperm[c] = P[uperm]
        # unique items -> balanced slot positions
        uit = np.unique(items)                # sorted node ids
        nu = len(uit)
        iid = uit - NUM_USER
        order_i = np.argsort(-(deg[uit] + wc_item[iid]), kind="stable")
        # rank r (in uit order) -> its balance order index, then snake position
        inv = np.empty(nu, np.int64)
        inv[order_i] = np.arange(nu)
        ipos_of_rank = snake_pos(nu)[inv]
        # final-score gather: position q -> item slot position
        islot_of_pair = ipos_of_rank[np.searchsorted(uit, items[uperm])]
        st16 = islot_of_pair.astype(np.int16)
        sidx[c] = np.ascontiguousarray(
            np.tile(st16.reshape(BPC // 16, 16).T, (8, 1)))

        # edge instances: user positions + unique-item slots
        nodes_e = np.concatenate([users[uperm], uit])
        slots_e = np.concatenate([np.arange(BPC), BPC + ipos_of_rank])
        st = np.searchsorted(sdst, nodes_e)
        en = np.searchsorted(sdst, nodes_e, side="right")
        lens = en - st
        slot_rep = np.repeat(slots_e, lens)
        src_rep = ssrc[_ragged_gather(st, lens)]
        reg_rep = np.searchsorted(np.array(E_REG_BOUNDS[1:-1]), src_rep,
                                  side="right")
        loc_rep = src_rep - np.array(E_REG_BOUNDS)[reg_rep]
        np.add.at(cnt_e[c], (reg_rep, slot_rep >> 6), 1)
        e_data.append((slot_rep, loc_rep, reg_rep))

        # word instances per unique item
        wst = np.searchsorted(switem_srt, iid)
        wen = np.searchsorted(switem_srt, iid, side="right")
        wlens = wen - wst
        wslot_rep = np.repeat(ipos_of_rank, wlens)
        word_rep = swword[_ragged_gather(wst, wlens)]
        wreg_rep = word_rep // REG_W
        wloc_rep = word_rep - wreg_rep * REG_W
        np.add.at(cnt_w[c], (wreg_rep, wslot_rep >> 6), 1)
        winv = (1.0 / np.maximum(wlens, 1)).astype(np.float32)
        wscale_rep = np.repeat(winv, wlens)
        w_data.append((wslot_rep, wloc_rep, wreg_rep, wscale_rep))

        vf_pos = np.zeros((BPC, WDIM), np.float32)
        vf_pos[ipos_of_rank] = v_feat[iid]
        vfT[c] = vf_pos.T.astype(bf16)

    es = _Sched(cnt_e, ET, NREG_E, EB)
    ws = _Sched(cnt_w, WT, NREG_W, WB)

    eidx = np.zeros((NC, 128, es.S // 16), np.int16)
    eloc = np.zeros((NC, 128, es.NCH), bf16)
    widx = np.zeros((NC, 128, ws.S // 16), np.int16)
    wloc = np.zeros((NC, 128, ws.NCH), bf16)
    wsc = np.zeros((NC, 128, ws.NCH), bf16)
    for c in range(NC):
        slot_rep, loc_rep, reg_rep = e_data[c]
        i_s, l_s, _ = _fill_stream(es, slot_rep, loc_rep, reg_rep)
        eidx[c] = _wrap_idx(i_s)
        eloc[c] = _per_chunk(l_s)
        wslot_rep, wloc_rep, wreg_rep, wscale_rep = w_data[c]
        i_s, l_s, s_s = _fill_stream(ws, wslot_rep, wloc_rep, wreg_rep,
                                     scale=wscale_rep)
        widx[c] = _wrap_idx(i_s)
        wloc[c] = _per_chunk(l_s)
        wsc[c] = _per_chunk(s_s)

    return dict(es=es, ws=ws, eidx=eidx, eloc=eloc,
                widx=widx, wloc=wloc, wsc=wsc, vfT=vfT,
                sidx=sidx, outperm=outperm)


# ------------------------------------------------------------- bass program

def _build_program(es, ws):
    from concourse import bass, bacc, mybir
    import concourse.tile as tile
    dt = mybir.dt

    nc = bacc.Bacc(None, target_bir_lowering=False, num_swdge_queues=4)
    f32 = dt.float32
    bf = dt.bfloat16

    id_in = nc.dram_tensor("id_emb", [NNODE, DIM], f32, kind="ExternalInput")
    wt_in = nc.dram_tensor("wt_bf", [VOCAB, WDIM], bf, kind="ExternalInput")
    eidx_in = nc.dram_tensor("eidx", [128, es.S // 16], dt.int16, kind="ExternalInput")
    eloc_in = nc.dram_tensor("eloc", [128, es.NCH], bf, kind="ExternalInput")
    widx_in = nc.dram_tensor("widx", [128, ws.S // 16], dt.int16, kind="ExternalInput")
    wloc_in = nc.dram_tensor("wloc", [128, ws.NCH], bf, kind="ExternalInput")
    wsc_in = nc.dram_tensor("wsc", [128, ws.NCH], bf, kind="ExternalInput")
    vfT_in = nc.dram_tensor("vfT", [WDIM, BPC], bf, kind="ExternalInput")
    cw_in = nc.dram_tensor("cw_bf", [DIM, DIM], bf, kind="ExternalInput")
    ww_in = nc.dram_tensor("ww_bf", [DIM, DIM], bf, kind="ExternalInput")
    w2_in = nc.dram_tensor("w2_bf", [DIM, DIM], bf, kind="ExternalInput")
    lw_in = nc.dram_tensor("lw_bf", [2 * WDIM, DIM], bf, kind="ExternalInput")
    lb_in = nc.dram_tensor("lb_col", [DIM, 1], f32, kind="ExternalInput")
    ident_in = nc.dram_tensor("ident", [128, 128], f32, kind="ExternalInput")
    sidx_in = nc.dram_tensor("sidx", [128, BPC // 16], dt.int16, kind="ExternalInput")
    iota_in = nc.dram_tensor("iota_bf", [128, 128], bf, kind="ExternalInput")
    out = nc.dram_tensor("scores_w", [128, 8], f32, kind="ExternalOutput")
    x2i_dram = nc.dram_tensor("x2i", [BPC, DIM], f32)

    id_regions = [(E_REG_BOUNDS[i], E_REG_BOUNDS[i + 1]) for i in range(3)]
    wt_regions = [(r * REG_W, (r + 1) * REG_W) for r in range(NREG_W)]

    with tile.TileContext(nc) as tc:
        with tc.tile_pool(name="const", bufs=1) as cpool, \
             tc.tile_pool(name="persist", bufs=1) as pp, \
             tc.tile_pool(name="ewp", bufs=4) as ewp, \
             tc.tile_pool(name="wwp", bufs=4) as wwp, \
             tc.tile_pool(name="mid", bufs=2) as midp, \
             tc.tile_pool(name="xp", bufs=2) as xp, \
             tc.tile_pool(name="psw", bufs=2, space="PSUM") as psw, \
             tc.tile_pool(name="pse", bufs=2, space="PSUM") as pse, \
             tc.tile_pool(name="psm", bufs=2, space="PSUM") as psm:

            iota = cpool.tile([128, 128], bf)
            cw = cpool.tile([DIM, DIM], bf)
            ww = cpool.tile([DIM, DIM], bf)
            w2 = cpool.tile([DIM, DIM], bf)
            lw = cpool.tile([128, 2 * DIM], bf)   # cols 0:64 = v-half, 64:128 = t-half
            lb = cpool.tile([DIM, 1], f32)
            ident = cpool.tile([128, 128], f32)
            sidx_sb = cpool.tile([128, BPC // 16], dt.int16)
            nc.sync.dma_start(out=iota[:], in_=iota_in[:])
            nc.sync.dma_start(out=cw[:], in_=cw_in[:])
            nc.sync.dma_start(out=ww[:], in_=ww_in[:])
            nc.sync.dma_start(out=w2[:], in_=w2_in[:])
            nc.sync.dma_start(out=lw[:, 0:DIM], in_=lw_in[0:128, :])
            nc.sync.dma_start(out=lw[:, DIM:2 * DIM], in_=lw_in[128:256, :])
            nc.sync.dma_start(out=lb[:], in_=lb_in[:])
            nc.sync.dma_start(out=ident[:], in_=ident_in[:])
            nc.sync.dma_start(out=sidx_sb[:], in_=sidx_in[:])

            eidx_sb = pp.tile([128, es.S // 16], dt.int16)
            eloc_sb = pp.tile([128, es.NCH], bf)
            widx_sb = pp.tile([128, ws.S // 16], dt.int16)
            wloc_sb = pp.tile([128, ws.NCH], bf)
            wsc_sb = pp.tile([128, ws.NCH], bf)
            vfT_sb = pp.tile([WDIM, BPC], bf)
            nc.sync.dma_start(out=eidx_sb[:], in_=eidx_in[:])
            nc.sync.dma_start(out=eloc_sb[:], in_=eloc_in[:])
            nc.sync.dma_start(out=widx_sb[:], in_=widx_in[:])
            nc.sync.dma_start(out=wloc_sb[:], in_=wloc_in[:])
            nc.sync.dma_start(out=wsc_sb[:], in_=wsc_in[:])
            nc.sync.dma_start(out=vfT_sb[:], in_=vfT_in[:])

            tfT_sb = pp.tile([WDIM, IT * 128], bf)
            fT_sb = pp.tile([DIM, IT * 128], bf)
            x2T_sb = pp.tile([DIM, NT * 128], f32)
            tfsum_sb = pp.tile([WDIM, IT * 128], f32)
            agg_sb = pp.tile([DIM, NT * 128], f32)
            nc.vector.memset(tfsum_sb[:], 0.0)
            nc.vector.memset(agg_sb[:], 0.0)

            # ---- words: t_feat^T accumulation ----
            wps = None
            for wq, (r, ch0, nb) in enumerate(ws.batches):
                r0, r1 = wt_regions[r]
                wpay = wwp.tile([128, WB * WDIM], bf, tag="wpay")
                pay3 = wpay[:].rearrange("p (k d) -> p k d", d=WDIM)
                nc.gpsimd.dma_gather(
                    wpay[:, 0:nb * WDIM].rearrange("p (k d) -> p k d", d=WDIM),
                    wt_in[r0:r1, :],
                    widx_sb[:, ch0 * 8:(ch0 + nb) * 8],
                    nb * 128, nb * 128, WDIM, single_packet=False,
                    queue_num=wq % 4)
                wpays = midp.tile([128, WB * WDIM], bf, tag="wpays")
                pays3 = wpays[:].rearrange("p (k d) -> p k d", d=WDIM)
                nc.vector.tensor_tensor(
                    out=pays3[:, 0:nb, :], in0=pay3[:, 0:nb, :],
                    in1=wsc_sb[:, ch0:ch0 + nb][:, :, None].to_broadcast(
                        [128, nb, WDIM]),
                    op=mybir.AluOpType.mult)
                woh = wwp.tile([128, WB * SLOTW], bf, tag="woh")
                oh3 = woh[:].rearrange("p (k d) -> p k d", d=SLOTW)
                nc.vector.tensor_tensor(
                    out=oh3[:, 0:nb, :],
                    in0=wloc_sb[:, ch0:ch0 + nb][:, :, None].to_broadcast(
                        [128, nb, SLOTW]),
                    in1=iota[:][:, None, 0:SLOTW].to_broadcast([128, nb, SLOTW]),
                    op=mybir.AluOpType.is_equal)
                for k in range(nb):
                    ch = ch0 + k
                    t = int(ws.tile_of[ch])
                    if ws.is_first[ch]:
                        wps = psw.tile([WDIM, 512], f32, tag="wp")
                    nc.tensor.matmul(
                        out=wps[:, 0:SLOTW], lhsT=pays3[:, k, :], rhs=oh3[:, k, :],
                        start=ws.is_first[ch], stop=ws.is_last[ch])
                    if ws.is_last[ch]:
                        sl = tfsum_sb[:, t * SLOTW:(t + 1) * SLOTW]
                        nc.vector.tensor_tensor(out=sl, in0=sl,
                                                in1=wps[:, 0:SLOTW],
                                                op=mybir.AluOpType.add)

            for t in range(IT):
                nc.scalar.activation(
                    tfT_sb[:, t * 128:(t + 1) * 128],
                    tfsum_sb[:, t * 128:(t + 1) * 128],
                    mybir.ActivationFunctionType.Copy)

            # ---- f^T = lrelu(lw^T cat^T + lb); fh feeds item-tile x2 ----
            for t in range(IT):
                fp = psm.tile([DIM, 512], f32, tag="mm")
                nc.tensor.matmul(out=fp[:, 0:128], lhsT=lw[:, 0:DIM],
                                 rhs=vfT_sb[:, t * 128:(t + 1) * 128],
                                 start=True, stop=False)
                nc.tensor.matmul(out=fp[:, 0:128], lhsT=lw[:, DIM:2 * DIM],
                                 rhs=tfT_sb[:, t * 128:(t + 1) * 128],
                                 start=False, stop=True)
                nc.scalar.activation(
                    fT_sb[:, t * 128:(t + 1) * 128], fp[:, 0:128],
                    mybir.ActivationFunctionType.Lrelu,
                    bias=lb[:], alpha=SLOPE)

            # ---- edges: agg^T accumulation with on-the-fly normalize ----
            es_has_pair = set()
            for (_r, _c0, _nb) in es.batches:
                _k = 0
                while _k < _nb:
                    _ch = _c0 + _k
                    if (_k + 1 < _nb) and not es.is_first[_ch + 1]:
                        es_has_pair.add((int(es.region_of[_ch]),
                                         int(es.tile_of[_ch])))
                        _k += 2
                    else:
                        _k += 1
            if True:
              eps = None
              for eq, (r, ch0, nb) in enumerate(es.batches):
                r0, r1 = id_regions[r]
                epay = ewp.tile([128, EB * DIM], f32, tag="epay")
                pay3 = epay[:].rearrange("p (k d) -> p k d", d=DIM)
                nc.gpsimd.dma_gather(
                    epay[:, 0:nb * DIM].rearrange("p (k d) -> p k d", d=DIM),
                    id_in[r0:r1, :],
                    eidx_sb[:, ch0 * 8:(ch0 + nb) * 8],
                    nb * 128, nb * 128, DIM, single_packet=False,
                    queue_num=eq % 4)
                esq = midp.tile([128, EB * DIM], f32, tag="esq")
                sq3 = esq[:].rearrange("p (k d) -> p k d", d=DIM)
                nc.vector.tensor_tensor(out=sq3[:, 0:nb, :], in0=pay3[:, 0:nb, :],
                                        in1=pay3[:, 0:nb, :],
                                        op=mybir.AluOpType.mult)
                ss = ewp.tile([128, EB], f32, tag="ess")
                nc.vector.reduce_sum(out=ss[:, 0:nb], in_=sq3[:, 0:nb, :],
                                     axis=mybir.AxisListType.X)
                nc.scalar.sqrt(ss[:, 0:nb], ss[:, 0:nb])
                nc.vector.reciprocal(ss[:, 0:nb], ss[:, 0:nb])
                epayb = ewp.tile([128, EB * DIM], bf, tag="epayb")
                payb3 = epayb[:].rearrange("p (k d) -> p k d", d=DIM)
                nc.vector.tensor_tensor(
                    out=payb3[:, 0:nb, :], in0=pay3[:, 0:nb, :],
                    in1=ss[:, 0:nb][:, :, None].to_broadcast([128, nb, DIM]),
                    op=mybir.AluOpType.mult)
                eoh = ewp.tile([128, EB * SLOTW], bf, tag="eoh")
                oh3 = eoh[:].rearrange("p (k d) -> p k d", d=SLOTW)
                nc.vector.tensor_tensor(
                    out=oh3[:, 0:nb, :],
                    in0=eloc_sb[:, ch0:ch0 + nb][:, :, None].to_broadcast(
                        [128, nb, SLOTW]),
                    in1=iota[:][:, None, 0:SLOTW].to_broadcast([128, nb, SLOTW]),
                    op=mybir.AluOpType.is_equal)
                k = 0
                while k < nb:
                    ch = ch0 + k
                    t = int(es.tile_of[ch])
                    if es.is_first[ch]:
                        eps = pse.tile([128, 512], f32, tag="ep")
                    pair = (k + 1 < nb) and not es.is_first[ch + 1]
                    if pair:
                        stop = es.is_last[ch + 1]
                        nc.tensor.matmul(
                            out=eps[:, 0:128],
                            lhsT=epayb[:, k * DIM:(k + 2) * DIM],
                            rhs=eoh[:, k * SLOTW:(k + 2) * SLOTW],
                            start=es.is_first[ch], stop=stop)
                        k += 2
                    else:
                        stop = es.is_last[ch]
                        nc.tensor.matmul(
                            out=eps[0:DIM, 0:SLOTW],
                            lhsT=epayb[:, k * DIM:(k + 1) * DIM],
                            rhs=eoh[:, k * SLOTW:(k + 1) * SLOTW],
                            start=es.is_first[ch], stop=stop)
                        k += 1
                    if stop:
                        g = (int(es.region_of[ch]), t)
                        sl = agg_sb[:, t * SLOTW:(t + 1) * SLOTW]
                        nc.vector.tensor_tensor(out=sl, in0=sl,
                                                in1=eps[0:DIM, 0:SLOTW],
                                                op=mybir.AluOpType.add)
                        if g in es_has_pair:
                            nc.vector.tensor_tensor(
                                out=sl, in0=sl,
                                in1=eps[DIM:128, SLOTW:128],
                                op=mybir.AluOpType.add)

              # ---- node tail: x2^T = lrelu(ww^T x1^T (+ w2^T f^T)) ----
              for t in range(NT):
                aggT = xp.tile([DIM, 128], bf, tag="aggT")
                nc.scalar.activation(aggT[:], agg_sb[:, t * 128:(t + 1) * 128],
                                     mybir.ActivationFunctionType.Copy)
                x1p = psm.tile([DIM, 512], f32, tag="mm")
                nc.tensor.matmul(out=x1p[:, 0:128], lhsT=cw[:], rhs=aggT[:],
                                 start=True, stop=True)
                x1T = xp.tile([DIM, 128], bf, tag="x1T")
                nc.scalar.activation(x1T[:], x1p[:, 0:128],
                                     mybir.ActivationFunctionType.Lrelu,
                                     alpha=SLOPE)
                x2p = psm.tile([DIM, 512], f32, tag="mm")
                nc.tensor.matmul(out=x2p[:, 0:128], lhsT=ww[:], rhs=x1T[:],
                                 start=True, stop=(t < IT))
                if t >= IT:
                    ti = t - IT
                    nc.tensor.matmul(out=x2p[:, 0:128], lhsT=w2[:],
                                     rhs=fT_sb[:, ti * 128:(ti + 1) * 128],
                                     start=False, stop=True)
                nc.scalar.activation(x2T_sb[:, t * 128:(t + 1) * 128], x2p[:, 0:128],
                                     mybir.ActivationFunctionType.Lrelu,
                                     alpha=SLOPE)

            # ---- scores: transpose x2^T tiles to rows, route item rows ----
            x2r_u = pp.tile([128, IT * DIM], f32)
            x2r_i = pp.tile([128, IT * DIM], f32)
            for t in range(IT):
                ps_t = psm.tile([128, 512], f32, tag="tr")
                nc.tensor.transpose(out=ps_t[:, 0:DIM],
                                    in_=x2T_sb[:, t * 128:(t + 1) * 128],
                                    identity=ident[0:DIM, 0:DIM])
                nc.scalar.activation(x2r_u[:, t * DIM:(t + 1) * DIM], ps_t[:, 0:DIM],
                                     mybir.ActivationFunctionType.Copy)
            for t in range(IT):
                ps_t = psm.tile([128, 512], f32, tag="tr")
                nc.tensor.transpose(out=ps_t[:, 0:DIM],
                                    in_=x2T_sb[:, (IT + t) * 128:(IT + t + 1) * 128],
                                    identity=ident[0:DIM, 0:DIM])
                nc.scalar.activation(x2r_i[:, t * DIM:(t + 1) * DIM], ps_t[:, 0:DIM],
                                     mybir.ActivationFunctionType.Copy)
            nc.sync.dma_start(
                out=x2i_dram[:, :].rearrange("(t p) d -> p t d", p=128),
                in_=x2r_i[:].rearrange("p (t d) -> p t d", d=DIM))
            ipay = pp.tile([128, IT * DIM], f32)
            nc.gpsimd.dma_gather(
                ipay[:].rearrange("p (k d) -> p k d", d=DIM),
                x2i_dram[:, :],
                sidx_sb[:],
                BPC, BPC, DIM, single_packet=False)
            prod = pp.tile([128, IT * DIM], f32)
            nc.vector.tensor_tensor(
                out=prod[:].rearrange("p (k d) -> p k d", d=DIM),
                in0=x2r_u[:].rearrange("p (k d) -> p k d", d=DIM),
                in1=ipay[:].rearrange("p (k d) -> p k d", d=DIM),
                op=mybir.AluOpType.mult)
            sc = pp.tile([128, 8], f32)
            nc.vector.reduce_sum(out=sc[:],
                                 in_=prod[:].rearrange("p (k d) -> p k d", d=DIM),
                                 axis=mybir.AxisListType.X)
            nc.sync.dma_start(out=out[:], in_=sc[:])

    nc.finalize()
    return nc


# ------------------------------------------------------------------- kernel

def kernel(**inputs):
    from concourse.bass_utils import run_bass_kernel_spmd

    pr = _prep(inputs)
    es, ws = pr["es"], pr["ws"]
    key = es.key() + ws.key()
    if key not in _CACHE:
        _CACHE[key] = _build_program(es, ws)
    nc = _CACHE[key]

    iota_bf = np.broadcast_to(np.arange(128, dtype=bf16), (128, 128)).copy()
    ident = np.eye(128, dtype=np.float32)
    wt_bf = np.asarray(inputs["word_table"], np.float32).astype(bf16)
    lb_col = np.asarray(inputs["lin_b"], np.float32).reshape(DIM, 1).copy()
    cw_bf = np.asarray(inputs["conv_weight"], np.float32).astype(bf16)
    ww_bf = np.asarray(inputs["weight_W"], np.float32).astype(bf16)
    w2_bf = np.asarray(inputs["weight_2"], np.float32).astype(bf16)
    lw_bf = np.asarray(inputs["lin_w"], np.float32).astype(bf16)
    id_emb = np.ascontiguousarray(np.asarray(inputs["id_embedding"], np.float32))

    in_maps = []
    for c in range(NC):
        in_maps.append({
            "id_emb": id_emb,
            "wt_bf": wt_bf,
            "eidx": pr["eidx"][c],
            "eloc": pr["eloc"][c],
            "widx": pr["widx"][c],
            "wloc": pr["wloc"][c],
            "wsc": pr["wsc"][c],
            "vfT": pr["vfT"][c],
            "cw_bf": cw_bf,
            "ww_bf": ww_bf,
            "w2_bf": w2_bf,
            "lw_bf": lw_bf,
            "lb_col": lb_col,
            "ident": ident,
            "sidx": pr["sidx"][c],
            "iota_bf": iota_bf,
        })
    res = run_bass_kernel_spmd(nc, in_maps, list(range(NC)))
    scores = np.empty(B, np.float32)
    for c in range(NC):
        w = res.results[c]["scores_w"]           # [128, 8]
        sc = np.asarray(w, np.float32).T.ravel()  # sc[position]
        scores[pr["outperm"][c]] = sc
    return scores


kernel.run_traced = None  # set by test harness if needed


# revision 33
# speedup vs baseline: 1.0656x; 1.0398x over previous
"""GCMC (gnn_message_passing) Trainium2 Bass kernel, 8-core SPMD.

Measured: 536.7 us HW exec (baseline 1793.4 us), rel L2 err ~2e-3.

Strategy (hardcoded for the nn_GCMC_40870908789353 shapes):
- Pairs are sorted by item and sharded in blocks of 1024 per core, so each
  sampled item's aggregation+words land on exactly one core and are computed
  once (global dedup floor, ~105k gathered rows/core). No collectives: the
  final scores read item x2 rows through one tiny on-core dma_gather.
- All sparse reads use batched dma_gather (SWDGE int16 idxs) on 4 SWDGE
  queues (num_swdge_queues=4; ~4.4 ns/row vs ~9 ns serialized); tables are
  split into <=32767-row regions (id_emb split at the user/item boundary so
  user-dst groups only touch the item region and vice versa).
- Segment-sum runs as one-hot matmuls in transposed form (lhsT=payload,
  rhs=one-hot -> PSUM agg^T/t_feat^T), adjacent same-group edge chunks are
  PAIRED into one [128x128]x[128x128] matmul (diagonal quadrants used, the
  off-diagonal garbage never read). Each (region, dst-group) accumulation
  owns a full PSUM bank, closed within its region pass, then DVE-added into
  SBUF accumulators (matmul start=True clears has_written for the WHOLE
  bank, and only for partitions it writes -- both constraints shaped the
  schedule; batches never cut right after a group's first chunk).
- Edge payload rows are L2-normalized on the fly (square/reduce/rsqrt) which
  folds F.normalize into the gather; the scale-mult also casts to bf16.
  Word payloads are bf16 host-cast and pre-scaled by 1/wordcount(item).
- lin_b rides the ACT bias port; x1@W and f@w2 share one PSUM accumulation.
"""
import sys
for p in ("/opt/trn_rl_repo", "/root/.axon_site/_ro/trn_rl_repo"):
    if p not in sys.path:
        sys.path.insert(0, p)
import numpy as np
import ml_dtypes

NC = 8
NUM_USER = 50000
NUM_ITEM = 20000
NNODE = 70000
VOCAB = 100000
DIM = 64
WDIM = 128
B = 8192
BPC = 1024          # pairs per core
NSLOT = 2048        # node slots per core (1024 user + 1024 item)
NT = 16             # node slot tiles (128 slots, for the x-tail)
IT = 8              # item slot tiles (128 slots, for the f-pipeline)
ET = 32             # edge dst groups (64 slots each)
WT = 16             # word dst groups (64 slots each)
SLOTW = 64          # one-hot width per dst group
E_REG_BOUNDS = (0, 25000, 50000, 70000)   # aligned to user/item boundary
NREG_E = 3
REG_W = 25000       # word_table region rows (4 regions)
NREG_W = 4
EB = 32             # edge chunks per dma_gather batch
WB = 24             # word chunks per dma_gather batch
SLOPE = 0.01

_CACHE = {}

bf16 = ml_dtypes.bfloat16


# ---------------------------------------------------------------- CPU prep

def _ragged_gather(starts, lens):
    """positions [starts[i], starts[i]+lens[i]) concatenated."""
    tot = int(lens.sum())
    if tot == 0:
        return np.zeros(0, np.int64)
    cum = np.cumsum(lens) - lens
    return np.repeat(starts - cum, lens) + np.arange(tot)


class _Sched:
    """Unified SPMD schedule for one gather family."""

    def __init__(self, cnt, n_tiles, n_reg, batch):
        # cnt: [NC, n_reg, n_tiles] instance counts
        nch = np.ceil(cnt / 128.0).astype(np.int64).max(axis=0)  # [n_reg,n_tiles]
        # every tile needs >=1 chunk overall so start/stop exist
        tile_tot = nch.sum(axis=0)
        for t in range(n_tiles):
            if tile_tot[t] == 0:
                nch[0][t] = 1
        self.nch = nch
        self.n_tiles = n_tiles
        self.n_reg = n_reg
        # global chunk order: region-major, tile-minor
        tiles = []
        regions = []
        for r in range(n_reg):
            for t in range(n_tiles):
                tiles += [t] * int(nch[r][t])
                regions += [r] * int(nch[r][t])
        self.tile_of = np.array(tiles, np.int64)
        self.region_of = np.array(regions, np.int64)
        self.NCH = len(tiles)
        self.S = self.NCH * 128
        # start/stop flags per chunk at (region, tile) GROUP granularity: each
        # group is one PSUM accumulation (own bank) closed within its region.
        self.is_first = []
        self.is_last = []
        for ch in range(len(tiles)):
            r, t = regions[ch], tiles[ch]
            self.is_first.append(ch == 0 or (regions[ch - 1], tiles[ch - 1]) != (r, t))
            self.is_last.append(ch == len(tiles) - 1
                                or (regions[ch + 1], tiles[ch + 1]) != (r, t))
        # group (r,t) -> starting chunk
        self.group_ch0 = np.zeros((n_reg, n_tiles), np.int64)
        ch = 0
        for r in range(n_reg):
            for t in range(n_tiles):
                self.group_ch0[r][t] = ch
                ch += int(nch[r][t])
        # batches: split each region's chunk range into <= batch chunks
        self.batches = []  # (region, ch0, nchunks)
        for r in range(n_reg):
            r0 = int(self.group_ch0[r][0])
            r1 = int(self.group_ch0[r + 1][0]) if r + 1 < n_reg else self.NCH
            ch = r0
            while ch < r1:
                nb = min(batch, r1 - ch)
                # Never cut a batch right after a group's first chunk: a
                # continuing group would then open with a 64-partition single
                # whose start=True clears has_written only for partitions
                # 0-63, leaving the pairs' q11 half to accumulate onto the
                # PSUM slot's stale contents.
                if ch + nb < r1:
                    last = ch + nb - 1
                    if self.is_first[last] and not self.is_last[last]:
                        nb -= 1
                self.batches.append((r, ch, nb))
                ch += nb

    def key(self):
        return (self.n_tiles, self.n_reg) + tuple(self.nch.ravel().tolist())


def _fill_stream(sched, slot_rep, loc_val, region_rep, scale=None):
    """Place instances into the padded stream. Returns (idx_stream int16,
    loc_stream bf16, scale_stream bf16 or None)."""
    n_tiles = sched.n_tiles
    key = region_rep * n_tiles + (slot_rep >> 6)
    order = np.argsort(key, kind="stable")
    skey = key[order]
    gcnt = np.bincount(skey, minlength=sched.n_reg * n_tiles)
    # position of each sorted instance: group base*128 + within-group offset
    ch0 = sched.group_ch0.ravel()
    base = np.repeat(ch0 * 128, gcnt)
    within = np.arange(len(order)) - np.repeat(np.cumsum(gcnt) - gcnt, gcnt)
    pos = base + within
    idx_stream = np.zeros(sched.S, np.int16)
    idx_stream[pos] = loc_val[order].astype(np.int16)
    loc_stream = np.full(sched.S, -1.0, bf16)
    loc_stream[pos] = (slot_rep[order] & 63).astype(bf16)
    sc_stream = None
    if scale is not None:
        sc_stream = np.zeros(sched.S, bf16)
        sc_stream[pos] = scale[order].astype(bf16)
    return idx_stream, loc_stream, sc_stream


def _wrap_idx(idx_stream):
    """[S] int16 -> [128, S/16] wrapped+replicated layout."""
    S = idx_stream.shape[0]
    base = idx_stream.reshape(S // 16, 16).T  # [16, S/16]
    return np.ascontiguousarray(np.tile(base, (8, 1)))


def _per_chunk(stream):
    """[S] -> [128, NCH]: position i=(ch*128+p) -> [p, ch]."""
    NCH = stream.shape[0] // 128
    return np.ascontiguousarray(stream.reshape(NCH, 128).T)


def _prep(inputs):
    edge_index = np.asarray(inputs["edge_index"])
    words_tensor = np.asarray(inputs["words_tensor"])
    user_nodes = np.asarray(inputs["user_nodes"]).astype(np.int64)
    item_nodes = np.asarray(inputs["item_nodes"]).astype(np.int64)

    src = edge_index[0].astype(np.int64)
    dst = edge_index[1].astype(np.int64)
    witem = words_tensor[0].astype(np.int64)
    wword = words_tensor[1].astype(np.int64)

    eorder = np.argsort(dst, kind="stable")
    sdst = dst[eorder]
    ssrc = src[eorder]
    worder = np.argsort(witem, kind="stable")
    switem_srt = witem[worder]
    swword = wword[worder]

    deg = np.bincount(dst, minlength=NNODE)
    wc_item = np.bincount(witem, minlength=NUM_ITEM)

    def snake_pos(n):
        i = np.arange(n)
        rnd, lane = divmod(i, 16)
        g = np.where(rnd % 2 == 0, lane, 15 - lane)
        return g * 64 + rnd

    # cluster pairs by item: each item's aggregation lands on one core
    gorder = np.argsort(item_nodes, kind="stable")

    e_data, w_data = [], []
    outperm = np.zeros((NC, BPC), np.int64)
    sidx = np.zeros((NC, 128, BPC // 16), np.int16)
    cnt_e = np.zeros((NC, NREG_E, ET), np.int64)
    cnt_w = np.zeros((NC, NREG_W, WT), np.int64)
    vfT = np.zeros((NC, WDIM, BPC), bf16)
    v_feat = np.asarray(inputs["v_feat"], np.float32)
    for c in range(NC):
        P = gorder[c * BPC:(c + 1) * BPC]
        users = user_nodes[P]
        items = item_nodes[P]
        # user position permutation (balance by degree, snake)
        order_u = np.argsort(-deg[users], kind="stable")
        pos_u = snake_pos(BPC)
        uperm = np.empty(BPC, np.int64)       # uperm[position] = pair rank in P
        uperm[pos_u] = order_u
        outperm[c] = P[uperm]
        # unique items -> balanced slot positions
        uit = np.unique(items)                # sorted node ids
        nu = len(uit)
        iid = uit - NUM_USER
        order_i = np.argsort(-(deg[uit] + wc_item[iid]), kind="stable")
        # rank r (in uit order) -> its balance order index, then snake position
        inv = np.empty(nu, np.int64)
        inv[order_i] = np.arange(nu)
        ipos_of_rank = snake_pos(nu)[inv]
        # final-score gather: position q -> item slot position
        islot_of_pair = ipos_of_rank[np.searchsorted(uit, items[uperm])]
        st16 = islot_of_pair.astype(np.int16)
        sidx[c] = np.ascontiguousarray(
            np.tile(st16.reshape(BPC // 16, 16).T, (8, 1)))

        # edge instances: user positions + unique-item slots
        nodes_e = np.concatenate([users[uperm], uit])
        slots_e = np.concatenate([np.arange(BPC), BPC + ipos_of_rank])
        st = np.searchsorted(sdst, nodes_e)
        en = np.searchsorted(sdst, nodes_e, side="right")
        lens = en - st
        slot_rep = np.repeat(slots_e, lens)
        src_rep = ssrc[_ragged_gather(st, lens)]
        reg_rep = np.searchsorted(np.array(E_REG_BOUNDS[1:-1]), src_rep,
                                  side="right")
        loc_rep = src_rep - np.array(E_REG_BOUNDS)[reg_rep]
        np.add.at(cnt_e[c], (reg_rep, slot_rep >> 6), 1)
        e_data.append((slot_rep, loc_rep, reg_rep))

        # word instances per unique item
        wst = np.searchsorted(switem_srt, iid)
        wen = np.searchsorted(switem_srt, iid, side="right")
        wlens = wen - wst
        wslot_rep = np.repeat(ipos_of_rank, wlens)
        word_rep = swword[_ragged_gather(wst, wlens)]
        wreg_rep = word_rep // REG_W
        wloc_rep = word_rep - wreg_rep * REG_W
        np.add.at(cnt_w[c], (wreg_rep, wslot_rep >> 6), 1)
        winv = (1.0 / np.maximum(wlens, 1)).astype(np.float32)
        wscale_rep = np.repeat(winv, wlens)
        w_data.append((wslot_rep, wloc_rep, wreg_rep, wscale_rep))

        vf_pos = np.zeros((BPC, WDIM), np.float32)
        vf_pos[ipos_of_rank] = v_feat[iid]
        vfT[c] = vf_pos.T.astype(bf16)

    es = _Sched(cnt_e, ET, NREG_E, EB)
    ws = _Sched(cnt_w, WT, NREG_W, WB)

    eidx = np.zeros((NC, 128, es.S // 16), np.int16)
    eloc = np.zeros((NC, 128, es.NCH), bf16)
    widx = np.zeros((NC, 128, ws.S // 16), np.int16)
    wloc = np.zeros((NC, 128, ws.NCH), bf16)
    wsc = np.zeros((NC, 128, ws.NCH), bf16)
    for c in range(NC):
        slot_rep, loc_rep, reg_rep = e_data[c]
        i_s, l_s, _ = _fill_stream(es, slot_rep, loc_rep, reg_rep)
        eidx[c] = _wrap_idx(i_s)
        eloc[c] = _per_chunk(l_s)
        wslot_rep, wloc_rep, wreg_rep, wscale_rep = w_data[c]
        i_s, l_s, s_s = _fill_stream(ws, wslot_rep, wloc_rep, wreg_rep,
                                     scale=wscale_rep)
        widx[c] = _wrap_idx(i_s)
        wloc[c] = _per_chunk(l_s)
        wsc[c] = _per_chunk(s_s)

    return dict(es=es, ws=ws, eidx=eidx, eloc=eloc,
                widx=widx, wloc=wloc, wsc=wsc, vfT=vfT,
                sidx=sidx, outperm=outperm)


# ------------------------------------------------------------- bass program

def _build_program(es, ws):
    from concourse import bass, bacc, mybir
    import concourse.tile as tile
    dt = mybir.dt

    nc = bacc.Bacc(None, target_bir_lowering=False, num_swdge_queues=4)
    f32 = dt.float32
    bf = dt.bfloat16

    id_in = nc.dram_tensor("id_emb", [NNODE, DIM], f32, kind="ExternalInput")
    wt_in = nc.dram_tensor("wt_bf", [VOCAB, WDIM], bf, kind="ExternalInput")
    eidx_in = nc.dram_tensor("eidx", [128, es.S // 16], dt.int16, kind="ExternalInput")
    eloc_in = nc.dram_tensor("eloc", [128, es.NCH], bf, kind="ExternalInput")
    widx_in = nc.dram_tensor("widx", [128, ws.S // 16], dt.int16, kind="ExternalInput")
    wloc_in = nc.dram_tensor("wloc", [128, ws.NCH], bf, kind="ExternalInput")
    wsc_in = nc.dram_tensor("wsc", [128, ws.NCH], bf, kind="ExternalInput")
    vfT_in = nc.dram_tensor("vfT", [WDIM, BPC], bf, kind="ExternalInput")
    cw_in = nc.dram_tensor("cw_bf", [DIM, DIM], bf, kind="ExternalInput")
    ww_in = nc.dram_tensor("ww_bf", [DIM, DIM], bf, kind="ExternalInput")
    w2_in = nc.dram_tensor("w2_bf", [DIM, DIM], bf, kind="ExternalInput")
    lw_in = nc.dram_tensor("lw_bf", [2 * WDIM, DIM], bf, kind="ExternalInput")
    lb_in = nc.dram_tensor("lb_col", [DIM, 1], f32, kind="ExternalInput")
    ident_in = nc.dram_tensor("ident", [128, 128], f32, kind="ExternalInput")
    sidx_in = nc.dram_tensor("sidx", [128, BPC // 16], dt.int16, kind="ExternalInput")
    iota_in = nc.dram_tensor("iota_bf", [128, 128], bf, kind="ExternalInput")
    out = nc.dram_tensor("scores_w", [128, 8], f32, kind="ExternalOutput")
    x2i_dram = nc.dram_tensor("x2i", [BPC, DIM], f32)

    id_regions = [(E_REG_BOUNDS[i], E_REG_BOUNDS[i + 1]) for i in range(3)]
    wt_regions = [(r * REG_W, (r + 1) * REG_W) for r in range(NREG_W)]

    with tile.TileContext(nc) as tc:
        with tc.tile_pool(name="const", bufs=1) as cpool, \
             tc.tile_pool(name="persist", bufs=1) as pp, \
             tc.tile_pool(name="ewp", bufs=4) as ewp, \
             tc.tile_pool(name="wwp", bufs=4) as wwp, \
             tc.tile_pool(name="mid", bufs=2) as midp, \
             tc.tile_pool(name="xp", bufs=2) as xp, \
             tc.tile_pool(name="psw", bufs=2, space="PSUM") as psw, \
             tc.tile_pool(name="pse", bufs=2, space="PSUM") as pse, \
             tc.tile_pool(name="psm", bufs=2, space="PSUM") as psm:

            iota = cpool.tile([128, 128], bf)
            cw = cpool.tile([DIM, DIM], bf)
            ww = cpool.tile([DIM, DIM], bf)
            w2 = cpool.tile([DIM, DIM], bf)
            lw = cpool.tile([128, 2 * DIM], bf)   # cols 0:64 = v-half, 64:128 = t-half
            lb = cpool.tile([DIM, 1], f32)
            ident = cpool.tile([128, 128], f32)
            sidx_sb = cpool.tile([128, BPC // 16], dt.int16)
            nc.sync.dma_start(out=iota[:], in_=iota_in[:])
            nc.sync.dma_start(out=cw[:], in_=cw_in[:])
            nc.sync.dma_start(out=ww[:], in_=ww_in[:])
            nc.sync.dma_start(out=w2[:], in_=w2_in[:])
            nc.sync.dma_start(out=lw[:, 0:DIM], in_=lw_in[0:128, :])
            nc.sync.dma_start(out=lw[:, DIM:2 * DIM], in_=lw_in[128:256, :])
            nc.sync.dma_start(out=lb[:], in_=lb_in[:])
            nc.sync.dma_start(out=ident[:], in_=ident_in[:])
            nc.sync.dma_start(out=sidx_sb[:], in_=sidx_in[:])
            primer = cpool.tile([128, DIM], f32)
            nc.gpsimd.dma_gather(
                primer[:].rearrange("p (k d) -> p k d", d=DIM),
                id_in[0:25000, :], sidx_sb[:, 0:8],
                128, 128, DIM, single_packet=False)

            eidx_sb = pp.tile([128, es.S // 16], dt.int16)
            eloc_sb = pp.tile([128, es.NCH], bf)
            widx_sb = pp.tile([128, ws.S // 16], dt.int16)
            wloc_sb = pp.tile([128, ws.NCH], bf)
            wsc_sb = pp.tile([128, ws.NCH], bf)
            vfT_sb = pp.tile([WDIM, BPC], bf)
            nc.sync.dma_start(out=eidx_sb[:], in_=eidx_in[:])
            nc.sync.dma_start(out=eloc_sb[:], in_=eloc_in[:])
            nc.sync.dma_start(out=widx_sb[:], in_=widx_in[:])
            nc.sync.dma_start(out=wloc_sb[:], in_=wloc_in[:])
            nc.sync.dma_start(out=wsc_sb[:], in_=wsc_in[:])
            nc.sync.dma_start(out=vfT_sb[:], in_=vfT_in[:])

            tfT_sb = pp.tile([WDIM, IT * 128], bf)
            fT_sb = pp.tile([DIM, IT * 128], bf)
            x2T_sb = pp.tile([DIM, NT * 128], f32)
            tfsum_sb = pp.tile([WDIM, IT * 128], f32)
            agg_sb = pp.tile([DIM, NT * 128], f32)
            nc.vector.memset(tfsum_sb[:], 0.0)
            nc.vector.memset(agg_sb[:], 0.0)

            # ---- words: t_feat^T accumulation ----
            wps = None
            for wq, (r, ch0, nb) in enumerate(ws.batches):
                r0, r1 = wt_regions[r]
                wpay = wwp.tile([128, WB * WDIM], bf, tag="wpay")
                pay3 = wpay[:].rearrange("p (k d) -> p k d", d=WDIM)
                nc.gpsimd.dma_gather(
                    wpay[:, 0:nb * WDIM].rearrange("p (k d) -> p k d", d=WDIM),
                    wt_in[r0:r1, :],
                    widx_sb[:, ch0 * 8:(ch0 + nb) * 8],
                    nb * 128, nb * 128, WDIM, single_packet=False,
                    queue_num=wq % 4)
                woh = wwp.tile([128, WB * SLOTW], bf, tag="woh")
                oh3 = woh[:].rearrange("p (k d) -> p k d", d=SLOTW)
                nc.vector.tensor_tensor(
                    out=oh3[:, 0:nb, :],
                    in0=wloc_sb[:, ch0:ch0 + nb][:, :, None].to_broadcast(
                        [128, nb, SLOTW]),
                    in1=iota[:][:, None, 0:SLOTW].to_broadcast([128, nb, SLOTW]),
                    op=mybir.AluOpType.is_equal)
                nc.vector.tensor_tensor(
                    out=oh3[:, 0:nb, :], in0=oh3[:, 0:nb, :],
                    in1=wsc_sb[:, ch0:ch0 + nb][:, :, None].to_broadcast(
                        [128, nb, SLOTW]),
                    op=mybir.AluOpType.mult)
                for k in range(nb):
                    ch = ch0 + k
                    t = int(ws.tile_of[ch])
                    if ws.is_first[ch]:
                        wps = psw.tile([WDIM, 512], f32, tag="wp")
                    nc.tensor.matmul(
                        out=wps[:, 0:SLOTW], lhsT=pay3[:, k, :], rhs=oh3[:, k, :],
                        start=ws.is_first[ch], stop=ws.is_last[ch])
                    if ws.is_last[ch]:
                        sl = tfsum_sb[:, t * SLOTW:(t + 1) * SLOTW]
                        nc.vector.tensor_tensor(out=sl, in0=sl,
                                                in1=wps[:, 0:SLOTW],
                                                op=mybir.AluOpType.add)

            for t in range(IT):
                nc.scalar.activation(
                    tfT_sb[:, t * 128:(t + 1) * 128],
                    tfsum_sb[:, t * 128:(t + 1) * 128],
                    mybir.ActivationFunctionType.Copy)

            # ---- f^T = lrelu(lw^T cat^T + lb); fh feeds item-tile x2 ----
            for t in range(IT):
                fp = psm.tile([DIM, 512], f32, tag="mm")
                nc.tensor.matmul(out=fp[:, 0:128], lhsT=lw[:, 0:DIM],
                                 rhs=vfT_sb[:, t * 128:(t + 1) * 128],
                                 start=True, stop=False)
                nc.tensor.matmul(out=fp[:, 0:128], lhsT=lw[:, DIM:2 * DIM],
                                 rhs=tfT_sb[:, t * 128:(t + 1) * 128],
                                 start=False, stop=True)
                nc.scalar.activation(
                    fT_sb[:, t * 128:(t + 1) * 128], fp[:, 0:128],
                    mybir.ActivationFunctionType.Lrelu,
                    bias=lb[:], alpha=SLOPE)

            # ---- edges: agg^T accumulation with on-the-fly normalize ----
            es_has_pair = set()
            for (_r, _c0, _nb) in es.batches:
                _k = 0
                while _k < _nb:
                    _ch = _c0 + _k
                    if (_k + 1 < _nb) and not es.is_first[_ch + 1]:
                        es_has_pair.add((int(es.region_of[_ch]),
                                         int(es.tile_of[_ch])))
                        _k += 2
                    else:
                        _k += 1
            if True:
              eps = None
              for eq, (r, ch0, nb) in enumerate(es.batches):
                r0, r1 = id_regions[r]
                epay = ewp.tile([128, EB * DIM], f32, tag="epay")
                pay3 = epay[:].rearrange("p (k d) -> p k d", d=DIM)
                nc.gpsimd.dma_gather(
                    epay[:, 0:nb * DIM].rearrange("p (k d) -> p k d", d=DIM),
                    id_in[r0:r1, :],
                    eidx_sb[:, ch0 * 8:(ch0 + nb) * 8],
                    nb * 128, nb * 128, DIM, single_packet=False,
                    queue_num=eq % 4)
                esq = midp.tile([128, EB * DIM], f32, tag="esq")
                sq3 = esq[:].rearrange("p (k d) -> p k d", d=DIM)
                nc.vector.tensor_tensor(out=sq3[:, 0:nb, :], in0=pay3[:, 0:nb, :],
                                        in1=pay3[:, 0:nb, :],
                                        op=mybir.AluOpType.mult)
                ss = ewp.tile([128, EB], f32, tag="ess")
                nc.vector.reduce_sum(out=ss[:, 0:nb], in_=sq3[:, 0:nb, :],
                                     axis=mybir.AxisListType.X)
                nc.scalar.sqrt(ss[:, 0:nb], ss[:, 0:nb])
                nc.vector.reciprocal(ss[:, 0:nb], ss[:, 0:nb])
                epayb = ewp.tile([128, EB * DIM], bf, tag="epayb")
                payb3 = epayb[:].rearrange("p (k d) -> p k d", d=DIM)
                nc.vector.tensor_tensor(
                    out=payb3[:, 0:nb, :], in0=pay3[:, 0:nb, :],
                    in1=ss[:, 0:nb][:, :, None].to_broadcast([128, nb, DIM]),
                    op=mybir.AluOpType.mult)
                eoh = ewp.tile([128, EB * SLOTW], bf, tag="eoh")
                oh3 = eoh[:].rearrange("p (k d) -> p k d", d=SLOTW)
                nc.vector.tensor_tensor(
                    out=oh3[:, 0:nb, :],
                    in0=eloc_sb[:, ch0:ch0 + nb][:, :, None].to_broadcast(
                        [128, nb, SLOTW]),
                    in1=iota[:][:, None, 0:SLOTW].to_broadcast([128, nb, SLOTW]),
                    op=mybir.AluOpType.is_equal)
                k = 0
                while k < nb:
                    ch = ch0 + k
                    t = int(es.tile_of[ch])
                    if es.is_first[ch]:
                        eps = pse.tile([128, 512], f32, tag="ep")
                    pair = (k + 1 < nb) and not es.is_first[ch + 1]
                    if pair:
                        stop = es.is_last[ch + 1]
                        nc.tensor.matmul(
                            out=eps[:, 0:128],
                            lhsT=epayb[:, k * DIM:(k + 2) * DIM],
                            rhs=eoh[:, k * SLOTW:(k + 2) * SLOTW],
                            start=es.is_first[ch], stop=stop)
                        k += 2
                    else:
                        stop = es.is_last[ch]
                        nc.tensor.matmul(
                            out=eps[0:DIM, 0:SLOTW],
                            lhsT=epayb[:, k * DIM:(k + 1) * DIM],
                            rhs=eoh[:, k * SLOTW:(k + 1) * SLOTW],
                            start=es.is_first[ch], stop=stop)
                        k += 1
                    if stop:
                        g = (int(es.region_of[ch]), t)
                        sl = agg_sb[:, t * SLOTW:(t + 1) * SLOTW]
                        nc.vector.tensor_tensor(out=sl, in0=sl,
                                                in1=eps[0:DIM, 0:SLOTW],
                                                op=mybir.AluOpType.add)
                        if g in es_has_pair:
                            nc.vector.tensor_tensor(
                                out=sl, in0=sl,
                                in1=eps[DIM:128, SLOTW:128],
                                op=mybir.AluOpType.add)

              # ---- node tail: x2^T = lrelu(ww^T x1^T (+ w2^T f^T)) ----
              for t in range(NT):
                aggT = xp.tile([DIM, 128], bf, tag="aggT")
                nc.scalar.activation(aggT[:], agg_sb[:, t * 128:(t + 1) * 128],
                                     mybir.ActivationFunctionType.Copy)
                x1p = psm.tile([DIM, 512], f32, tag="mm")
                nc.tensor.matmul(out=x1p[:, 0:128], lhsT=cw[:], rhs=aggT[:],
                                 start=True, stop=True)
                x1T = xp.tile([DIM, 128], bf, tag="x1T")
                nc.scalar.activation(x1T[:], x1p[:, 0:128],
                                     mybir.ActivationFunctionType.Lrelu,
                                     alpha=SLOPE)
                x2p = psm.tile([DIM, 512], f32, tag="mm")
                nc.tensor.matmul(out=x2p[:, 0:128], lhsT=ww[:], rhs=x1T[:],
                                 start=True, stop=(t < IT))
                if t >= IT:
                    ti = t - IT
                    nc.tensor.matmul(out=x2p[:, 0:128], lhsT=w2[:],
                                     rhs=fT_sb[:, ti * 128:(ti + 1) * 128],
                                     start=False, stop=True)
                nc.scalar.activation(x2T_sb[:, t * 128:(t + 1) * 128], x2p[:, 0:128],
                                     mybir.ActivationFunctionType.Lrelu,
                                     alpha=SLOPE)

            # ---- scores: transpose x2^T tiles to rows, route item rows ----
            x2r_u = pp.tile([128, IT * DIM], f32)
            x2r_i = pp.tile([128, IT * DIM], f32)
            for t in range(IT):
                ps_t = psm.tile([128, 512], f32, tag="tr")
                nc.tensor.transpose(out=ps_t[:, 0:DIM],
                                    in_=x2T_sb[:, t * 128:(t + 1) * 128],
                                    identity=ident[0:DIM, 0:DIM])
                nc.scalar.activation(x2r_u[:, t * DIM:(t + 1) * DIM], ps_t[:, 0:DIM],
                                     mybir.ActivationFunctionType.Copy)
            for t in range(IT):
                ps_t = psm.tile([128, 512], f32, tag="tr")
                nc.tensor.transpose(out=ps_t[:, 0:DIM],
                                    in_=x2T_sb[:, (IT + t) * 128:(IT + t + 1) * 128],
                                    identity=ident[0:DIM, 0:DIM])
                nc.scalar.activation(x2r_i[:, t * DIM:(t + 1) * DIM], ps_t[:, 0:DIM],
                                     mybir.ActivationFunctionType.Copy)
            nc.sync.dma_start(
                out=x2i_dram[:, :].rearrange("(t p) d -> p t d", p=128),
                in_=x2r_i[:].rearrange("p (t d) -> p t d", d=DIM))
            ipay = pp.tile([128, IT * DIM], f32)
            nc.gpsimd.dma_gather(
                ipay[:].rearrange("p (k d) -> p k d", d=DIM),
                x2i_dram[:, :],
                sidx_sb[:],
                BPC, BPC, DIM, single_packet=False)
            prod = pp.tile([128, IT * DIM], f32)
            nc.vector.tensor_tensor(
                out=prod[:].rearrange("p (k d) -> p k d", d=DIM),
                in0=x2r_u[:].rearrange("p (k d) -> p k d", d=DIM),
                in1=ipay[:].rearrange("p (k d) -> p k d", d=DIM),
                op=mybir.AluOpType.mult)
            sc = pp.tile([128, 8], f32)
            nc.vector.reduce_sum(out=sc[:],
                                 in_=prod[:].rearrange("p (k d) -> p k d", d=DIM),
                                 axis=mybir.AxisListType.X)
            nc.sync.dma_start(out=out[:], in_=sc[:])

    nc.finalize()
    return nc


# ------------------------------------------------------------------- kernel

def kernel(**inputs):
    from concourse.bass_utils import run_bass_kernel_spmd

    pr = _prep(inputs)
    es, ws = pr["es"], pr["ws"]
    key = es.key() + ws.key()
    if key not in _CACHE:
        _CACHE[key] = _build_program(es, ws)
    nc = _CACHE[key]

    iota_bf = np.broadcast_to(np.arange(128, dtype=bf16), (128, 128)).copy()
    ident = np.eye(128, dtype=np.float32)
    wt_bf = np.asarray(inputs["word_table"], np.float32).astype(bf16)
    lb_col = np.asarray(inputs["lin_b"], np.float32).reshape(DIM, 1).copy()
    cw_bf = np.asarray(inputs["conv_weight"], np.float32).astype(bf16)
    ww_bf = np.asarray(inputs["weight_W"], np.float32).astype(bf16)
    w2_bf = np.asarray(inputs["weight_2"], np.float32).astype(bf16)
    lw_bf = np.asarray(inputs["lin_w"], np.float32).astype(bf16)
    id_emb = np.ascontiguousarray(np.asarray(inputs["id_embedding"], np.float32))

    in_maps = []
    for c in range(NC):
        in_maps.append({
            "id_emb": id_emb,
            "wt_bf": wt_bf,
            "eidx": pr["eidx"][c],
            "eloc": pr["eloc"][c],
            "widx": pr["widx"][c],
            "wloc": pr["wloc"][c],
            "wsc": pr["wsc"][c],
            "vfT": pr["vfT"][c],
            "cw_bf": cw_bf,
            "ww_bf": ww_bf,
            "w2_bf": w2_bf,
            "lw_bf": lw_bf,
            "lb_col": lb_col,
            "ident": ident,
            "sidx": pr["sidx"][c],
            "iota_bf": iota_bf,
        })
    res = run_bass_kernel_spmd(nc, in_maps, list(range(NC)))
    scores = np.empty(B, np.float32)
    for c in range(NC):
        w = res.results[c]["scores_w"]           # [128, 8]
        sc = np.asarray(w, np.float32).T.ravel()  # sc[position]
        scores[pr["outperm"][c]] = sc
    return scores


kernel.run_traced = None  # set by test harness if needed


# revision 34
# speedup vs baseline: 1.1933x; 1.1198x over previous
"""GCMC (gnn_message_passing) Trainium2 Bass kernel, 8-core SPMD.

Measured: 536.7 us HW exec (baseline 1793.4 us), rel L2 err ~2e-3.

Strategy (hardcoded for the nn_GCMC_40870908789353 shapes):
- Pairs are sorted by item and sharded in blocks of 1024 per core, so each
  sampled item's aggregation+words land on exactly one core and are computed
  once (global dedup floor, ~105k gathered rows/core). No collectives: the
  final scores read item x2 rows through one tiny on-core dma_gather.
- All sparse reads use batched dma_gather (SWDGE int16 idxs) on 4 SWDGE
  queues (num_swdge_queues=4; ~4.4 ns/row vs ~9 ns serialized); tables are
  split into <=32767-row regions (id_emb split at the user/item boundary so
  user-dst groups only touch the item region and vice versa).
- Segment-sum runs as one-hot matmuls in transposed form (lhsT=payload,
  rhs=one-hot -> PSUM agg^T/t_feat^T), adjacent same-group edge chunks are
  PAIRED into one [128x128]x[128x128] matmul (diagonal quadrants used, the
  off-diagonal garbage never read). Each (region, dst-group) accumulation
  owns a full PSUM bank, closed within its region pass, then DVE-added into
  SBUF accumulators (matmul start=True clears has_written for the WHOLE
  bank, and only for partitions it writes -- both constraints shaped the
  schedule; batches never cut right after a group's first chunk).
- Edge payload rows are L2-normalized on the fly (square/reduce/rsqrt) which
  folds F.normalize into the gather; the scale-mult also casts to bf16.
  Word payloads are bf16 host-cast and pre-scaled by 1/wordcount(item).
- lin_b rides the ACT bias port; x1@W and f@w2 share one PSUM accumulation.
"""
import sys
for p in ("/opt/trn_rl_repo", "/root/.axon_site/_ro/trn_rl_repo"):
    if p not in sys.path:
        sys.path.insert(0, p)
import numpy as np
import ml_dtypes

NC = 8
NUM_USER = 50000
NUM_ITEM = 20000
NNODE = 70000
VOCAB = 100000
DIM = 64
WDIM = 128
B = 8192
BPC = 1024          # pairs per core
NSLOT = 2048        # node slots per core (1024 user + 1024 item)
NT = 16             # node slot tiles (128 slots, for the x-tail)
IT = 8              # item slot tiles (128 slots, for the f-pipeline)
ET = 32             # edge dst groups (64 slots each)
WT = 16             # word dst groups (64 slots each)
SLOTW = 64          # one-hot width per dst group
E_REG_BOUNDS = (0, 25000, 50000, 70000)   # aligned to user/item boundary
NREG_E = 3
REG_W = 25000       # word_table region rows (4 regions)
NREG_W = 4
EB = 32             # edge chunks per dma_gather batch
WB = 24             # word chunks per dma_gather batch
SLOPE = 0.01

_CACHE = {}

bf16 = ml_dtypes.bfloat16


# ---------------------------------------------------------------- CPU prep

def _ragged_gather(starts, lens):
    """positions [starts[i], starts[i]+lens[i]) concatenated."""
    tot = int(lens.sum())
    if tot == 0:
        return np.zeros(0, np.int64)
    cum = np.cumsum(lens) - lens
    return np.repeat(starts - cum, lens) + np.arange(tot)


class _Sched:
    """Unified SPMD schedule for one gather family."""

    def __init__(self, cnt, n_tiles, n_reg, batch):
        # cnt: [NC, n_reg, n_tiles] instance counts
        nch = np.ceil(cnt / 128.0).astype(np.int64).max(axis=0)  # [n_reg,n_tiles]
        # every tile needs >=1 chunk overall so start/stop exist
        tile_tot = nch.sum(axis=0)
        for t in range(n_tiles):
            if tile_tot[t] == 0:
                nch[0][t] = 1
        self.nch = nch
        self.n_tiles = n_tiles
        self.n_reg = n_reg
        # global chunk order: region-major, tile-minor
        tiles = []
        regions = []
        for r in range(n_reg):
            for t in range(n_tiles):
                tiles += [t] * int(nch[r][t])
                regions += [r] * int(nch[r][t])
        self.tile_of = np.array(tiles, np.int64)
        self.region_of = np.array(regions, np.int64)
        self.NCH = len(tiles)
        self.S = self.NCH * 128
        # start/stop flags per chunk at (region, tile) GROUP granularity: each
        # group is one PSUM accumulation (own bank) closed within its region.
        self.is_first = []
        self.is_last = []
        for ch in range(len(tiles)):
            r, t = regions[ch], tiles[ch]
            self.is_first.append(ch == 0 or (regions[ch - 1], tiles[ch - 1]) != (r, t))
            self.is_last.append(ch == len(tiles) - 1
                                or (regions[ch + 1], tiles[ch + 1]) != (r, t))
        # group (r,t) -> starting chunk
        self.group_ch0 = np.zeros((n_reg, n_tiles), np.int64)
        ch = 0
        for r in range(n_reg):
            for t in range(n_tiles):
                self.group_ch0[r][t] = ch
                ch += int(nch[r][t])
        # batches: split each region's chunk range into <= batch chunks
        self.batches = []  # (region, ch0, nchunks)
        for r in range(n_reg):
            r0 = int(self.group_ch0[r][0])
            r1 = int(self.group_ch0[r + 1][0]) if r + 1 < n_reg else self.NCH
            ch = r0
            while ch < r1:
                nb = min(batch, r1 - ch)
                # Never cut a batch right after a group's first chunk: a
                # continuing group would then open with a 64-partition single
                # whose start=True clears has_written only for partitions
                # 0-63, leaving the pairs' q11 half to accumulate onto the
                # PSUM slot's stale contents.
                if ch + nb < r1:
                    last = ch + nb - 1
                    if self.is_first[last] and not self.is_last[last]:
                        nb -= 1
                self.batches.append((r, ch, nb))
                ch += nb

    def key(self):
        return (self.n_tiles, self.n_reg) + tuple(self.nch.ravel().tolist())


def _fill_stream(sched, slot_rep, loc_val, region_rep, scale=None):
    """Place instances into the padded stream. Returns (idx_stream int16,
    loc_stream bf16, scale_stream bf16 or None)."""
    n_tiles = sched.n_tiles
    key = region_rep * n_tiles + (slot_rep >> 6)
    order = np.argsort(key, kind="stable")
    skey = key[order]
    gcnt = np.bincount(skey, minlength=sched.n_reg * n_tiles)
    # position of each sorted instance: group base*128 + within-group offset
    ch0 = sched.group_ch0.ravel()
    base = np.repeat(ch0 * 128, gcnt)
    within = np.arange(len(order)) - np.repeat(np.cumsum(gcnt) - gcnt, gcnt)
    pos = base + within
    idx_stream = np.zeros(sched.S, np.int16)
    idx_stream[pos] = loc_val[order].astype(np.int16)
    loc_stream = np.full(sched.S, -1.0, bf16)
    loc_stream[pos] = (slot_rep[order] & 63).astype(bf16)
    sc_stream = None
    if scale is not None:
        sc_stream = np.zeros(sched.S, bf16)
        sc_stream[pos] = scale[order].astype(bf16)
    return idx_stream, loc_stream, sc_stream


def _wrap_idx(idx_stream):
    """[S] int16 -> [128, S/16] wrapped+replicated layout."""
    S = idx_stream.shape[0]
    base = idx_stream.reshape(S // 16, 16).T  # [16, S/16]
    return np.ascontiguousarray(np.tile(base, (8, 1)))


def _per_chunk(stream):
    """[S] -> [128, NCH]: position i=(ch*128+p) -> [p, ch]."""
    NCH = stream.shape[0] // 128
    return np.ascontiguousarray(stream.reshape(NCH, 128).T)


def _prep(inputs):
    edge_index = np.asarray(inputs["edge_index"])
    words_tensor = np.asarray(inputs["words_tensor"])
    user_nodes = np.asarray(inputs["user_nodes"]).astype(np.int64)
    item_nodes = np.asarray(inputs["item_nodes"]).astype(np.int64)

    src = edge_index[0].astype(np.int64)
    dst = edge_index[1].astype(np.int64)
    witem = words_tensor[0].astype(np.int64)
    wword = words_tensor[1].astype(np.int64)

    eorder = np.argsort(dst, kind="stable")
    sdst = dst[eorder]
    ssrc = src[eorder]
    worder = np.argsort(witem, kind="stable")
    switem_srt = witem[worder]
    swword = wword[worder]

    deg = np.bincount(dst, minlength=NNODE)
    wc_item = np.bincount(witem, minlength=NUM_ITEM)

    def snake_pos(n):
        i = np.arange(n)
        rnd, lane = divmod(i, 16)
        g = np.where(rnd % 2 == 0, lane, 15 - lane)
        return g * 64 + rnd

    # cluster pairs by item: each item's aggregation lands on one core
    gorder = np.argsort(item_nodes, kind="stable")

    e_data, w_data = [], []
    outperm = np.zeros((NC, BPC), np.int64)
    sidx = np.zeros((NC, 128, BPC // 16), np.int16)
    cnt_e = np.zeros((NC, NREG_E, ET), np.int64)
    cnt_w = np.zeros((NC, NREG_W, WT), np.int64)
    vfT = np.zeros((NC, WDIM, BPC), bf16)
    v_feat = np.asarray(inputs["v_feat"], np.float32)
    for c in range(NC):
        P = gorder[c * BPC:(c + 1) * BPC]
        users = user_nodes[P]
        items = item_nodes[P]
        # user position permutation (balance by degree, snake)
        order_u = np.argsort(-deg[users], kind="stable")
        pos_u = snake_pos(BPC)
        uperm = np.empty(BPC, np.int64)       # uperm[position] = pair rank in P
        uperm[pos_u] = order_u
        outperm[c] = P[uperm]
        # unique items -> balanced slot positions
        uit = np.unique(items)                # sorted node ids
        nu = len(uit)
        iid = uit - NUM_USER
        order_i = np.argsort(-(deg[uit] + wc_item[iid]), kind="stable")
        # rank r (in uit order) -> its balance order index, then snake position
        inv = np.empty(nu, np.int64)
        inv[order_i] = np.arange(nu)
        ipos_of_rank = snake_pos(nu)[inv]
        # final-score gather: position q -> item slot position
        islot_of_pair = ipos_of_rank[np.searchsorted(uit, items[uperm])]
        st16 = islot_of_pair.astype(np.int16)
        sidx[c] = np.ascontiguousarray(
            np.tile(st16.reshape(BPC // 16, 16).T, (8, 1)))

        # edge instances: user positions + unique-item slots
        nodes_e = np.concatenate([users[uperm], uit])
        slots_e = np.concatenate([np.arange(BPC), BPC + ipos_of_rank])
        st = np.searchsorted(sdst, nodes_e)
        en = np.searchsorted(sdst, nodes_e, side="right")
        lens = en - st
        slot_rep = np.repeat(slots_e, lens)
        src_rep = ssrc[_ragged_gather(st, lens)]
        reg_rep = np.searchsorted(np.array(E_REG_BOUNDS[1:-1]), src_rep,
                                  side="right")
        loc_rep = src_rep - np.array(E_REG_BOUNDS)[reg_rep]
        np.add.at(cnt_e[c], (reg_rep, slot_rep >> 6), 1)
        e_data.append((slot_rep, loc_rep, reg_rep))

        # word instances per unique item
        wst = np.searchsorted(switem_srt, iid)
        wen = np.searchsorted(switem_srt, iid, side="right")
        wlens = wen - wst
        wslot_rep = np.repeat(ipos_of_rank, wlens)
        word_rep = swword[_ragged_gather(wst, wlens)]
        wreg_rep = word_rep // REG_W
        wloc_rep = word_rep - wreg_rep * REG_W
        np.add.at(cnt_w[c], (wreg_rep, wslot_rep >> 6), 1)
        winv = (1.0 / np.maximum(wlens, 1)).astype(np.float32)
        wscale_rep = np.repeat(winv, wlens)
        w_data.append((wslot_rep, wloc_rep, wreg_rep, wscale_rep))

        vf_pos = np.zeros((BPC, WDIM), np.float32)
        vf_pos[ipos_of_rank] = v_feat[iid]
        vfT[c] = vf_pos.T.astype(bf16)

    es = _Sched(cnt_e, ET, NREG_E, EB)
    ws = _Sched(cnt_w, WT, NREG_W, WB)

    eidx = np.zeros((NC, 128, es.S // 16), np.int16)
    eloc = np.zeros((NC, 128, es.NCH), bf16)
    widx = np.zeros((NC, 128, ws.S // 16), np.int16)
    wloc = np.zeros((NC, 128, ws.NCH), bf16)
    wsc = np.zeros((NC, 128, ws.NCH), bf16)
    for c in range(NC):
        slot_rep, loc_rep, reg_rep = e_data[c]
        i_s, l_s, _ = _fill_stream(es, slot_rep, loc_rep, reg_rep)
        eidx[c] = _wrap_idx(i_s)
        eloc[c] = _per_chunk(l_s)
        wslot_rep, wloc_rep, wreg_rep, wscale_rep = w_data[c]
        i_s, l_s, s_s = _fill_stream(ws, wslot_rep, wloc_rep, wreg_rep,
                                     scale=wscale_rep)
        widx[c] = _wrap_idx(i_s)
        wloc[c] = _per_chunk(l_s)
        wsc[c] = _per_chunk(s_s)

    return dict(es=es, ws=ws, eidx=eidx, eloc=eloc,
                widx=widx, wloc=wloc, wsc=wsc, vfT=vfT,
                sidx=sidx, outperm=outperm)


# ------------------------------------------------------------- bass program

def _build_program(es, ws):
    from concourse import bass, bacc, mybir
    import concourse.tile as tile
    dt = mybir.dt

    nc = bacc.Bacc(None, target_bir_lowering=False, num_swdge_queues=4)
    f32 = dt.float32
    bf = dt.bfloat16

    id_in = nc.dram_tensor("id_emb", [NNODE, DIM], f32, kind="ExternalInput")
    wt_in = nc.dram_tensor("wt_bf", [VOCAB, WDIM], bf, kind="ExternalInput")
    eidx_in = nc.dram_tensor("eidx", [128, es.S // 16], dt.int16, kind="ExternalInput")
    eloc_in = nc.dram_tensor("eloc", [128, es.NCH], bf, kind="ExternalInput")
    widx_in = nc.dram_tensor("widx", [128, ws.S // 16], dt.int16, kind="ExternalInput")
    wloc_in = nc.dram_tensor("wloc", [128, ws.NCH], bf, kind="ExternalInput")
    wsc_in = nc.dram_tensor("wsc", [128, ws.NCH], bf, kind="ExternalInput")
    vfT_in = nc.dram_tensor("vfT", [WDIM, BPC], bf, kind="ExternalInput")
    cw_in = nc.dram_tensor("cw_bf", [DIM, DIM], bf, kind="ExternalInput")
    ww_in = nc.dram_tensor("ww_bf", [DIM, DIM], bf, kind="ExternalInput")
    w2_in = nc.dram_tensor("w2_bf", [DIM, DIM], bf, kind="ExternalInput")
    lw_in = nc.dram_tensor("lw_bf", [2 * WDIM, DIM], bf, kind="ExternalInput")
    lb_in = nc.dram_tensor("lb_col", [DIM, 1], f32, kind="ExternalInput")
    ident_in = nc.dram_tensor("ident", [128, 128], f32, kind="ExternalInput")
    sidx_in = nc.dram_tensor("sidx", [128, BPC // 16], dt.int16, kind="ExternalInput")
    iota_in = nc.dram_tensor("iota_bf", [128, 128], bf, kind="ExternalInput")
    out = nc.dram_tensor("scores_w", [128, 8], f32, kind="ExternalOutput")
    x2i_dram = nc.dram_tensor("x2i", [BPC, DIM], f32)

    id_regions = [(E_REG_BOUNDS[i], E_REG_BOUNDS[i + 1]) for i in range(3)]
    wt_regions = [(r * REG_W, (r + 1) * REG_W) for r in range(NREG_W)]

    with tile.TileContext(nc) as tc:
        with tc.tile_pool(name="const", bufs=1) as cpool, \
             tc.tile_pool(name="persist", bufs=1) as pp, \
             tc.tile_pool(name="ewp", bufs=5) as ewp, \
             tc.tile_pool(name="wwp", bufs=5) as wwp, \
             tc.tile_pool(name="mid", bufs=2) as midp, \
             tc.tile_pool(name="xp", bufs=2) as xp, \
             tc.tile_pool(name="psw", bufs=2, space="PSUM") as psw, \
             tc.tile_pool(name="pse", bufs=2, space="PSUM") as pse, \
             tc.tile_pool(name="psm", bufs=2, space="PSUM") as psm:

            iota = cpool.tile([128, 128], bf)
            cw = cpool.tile([DIM, DIM], bf)
            ww = cpool.tile([DIM, DIM], bf)
            w2 = cpool.tile([DIM, DIM], bf)
            lw = cpool.tile([128, 2 * DIM], bf)   # cols 0:64 = v-half, 64:128 = t-half
            lb = cpool.tile([DIM, 1], f32)
            ident = cpool.tile([128, 128], f32)
            sidx_sb = cpool.tile([128, BPC // 16], dt.int16)
            nc.sync.dma_start(out=iota[:], in_=iota_in[:])
            nc.sync.dma_start(out=cw[:], in_=cw_in[:])
            nc.sync.dma_start(out=ww[:], in_=ww_in[:])
            nc.sync.dma_start(out=w2[:], in_=w2_in[:])
            nc.sync.dma_start(out=lw[:, 0:DIM], in_=lw_in[0:128, :])
            nc.sync.dma_start(out=lw[:, DIM:2 * DIM], in_=lw_in[128:256, :])
            nc.sync.dma_start(out=lb[:], in_=lb_in[:])
            nc.sync.dma_start(out=ident[:], in_=ident_in[:])
            nc.sync.dma_start(out=sidx_sb[:], in_=sidx_in[:])
            primer = cpool.tile([128, DIM], f32)
            nc.gpsimd.dma_gather(
                primer[:].rearrange("p (k d) -> p k d", d=DIM),
                id_in[0:25000, :], sidx_sb[:, 0:8],
                128, 128, DIM, single_packet=False)

            eidx_sb = pp.tile([128, es.S // 16], dt.int16)
            eloc_sb = pp.tile([128, es.NCH], bf)
            widx_sb = pp.tile([128, ws.S // 16], dt.int16)
            wloc_sb = pp.tile([128, ws.NCH], bf)
            wsc_sb = pp.tile([128, ws.NCH], bf)
            vfT_sb = pp.tile([WDIM, BPC], bf)
            nc.sync.dma_start(out=eidx_sb[:], in_=eidx_in[:])
            nc.sync.dma_start(out=eloc_sb[:], in_=eloc_in[:])
            nc.sync.dma_start(out=widx_sb[:], in_=widx_in[:])
            nc.sync.dma_start(out=wloc_sb[:], in_=wloc_in[:])
            nc.sync.dma_start(out=wsc_sb[:], in_=wsc_in[:])
            nc.sync.dma_start(out=vfT_sb[:], in_=vfT_in[:])

            tfT_sb = pp.tile([WDIM, IT * 128], bf)
            fT_sb = pp.tile([DIM, IT * 128], bf)
            x2T_sb = pp.tile([DIM, NT * 128], f32)
            tfsum_sb = pp.tile([WDIM, IT * 128], f32)
            agg_sb = pp.tile([DIM, NT * 128], f32)
            nc.vector.memset(tfsum_sb[:], 0.0)
            nc.vector.memset(agg_sb[:], 0.0)

            # ---- words: t_feat^T accumulation ----
            wps = None
            for wq, (r, ch0, nb) in enumerate(ws.batches):
                r0, r1 = wt_regions[r]
                wpay = wwp.tile([128, WB * WDIM], bf, tag="wpay")
                pay3 = wpay[:].rearrange("p (k d) -> p k d", d=WDIM)
                nc.gpsimd.dma_gather(
                    wpay[:, 0:nb * WDIM].rearrange("p (k d) -> p k d", d=WDIM),
                    wt_in[r0:r1, :],
                    widx_sb[:, ch0 * 8:(ch0 + nb) * 8],
                    nb * 128, nb * 128, WDIM, single_packet=False,
                    queue_num=wq % 4)
                woh = wwp.tile([128, WB * SLOTW], bf, tag="woh")
                oh3 = woh[:].rearrange("p (k d) -> p k d", d=SLOTW)
                nc.vector.tensor_tensor(
                    out=oh3[:, 0:nb, :],
                    in0=wloc_sb[:, ch0:ch0 + nb][:, :, None].to_broadcast(
                        [128, nb, SLOTW]),
                    in1=iota[:][:, None, 0:SLOTW].to_broadcast([128, nb, SLOTW]),
                    op=mybir.AluOpType.is_equal)
                nc.vector.tensor_tensor(
                    out=oh3[:, 0:nb, :], in0=oh3[:, 0:nb, :],
                    in1=wsc_sb[:, ch0:ch0 + nb][:, :, None].to_broadcast(
                        [128, nb, SLOTW]),
                    op=mybir.AluOpType.mult)
                for k in range(nb):
                    ch = ch0 + k
                    t = int(ws.tile_of[ch])
                    if ws.is_first[ch]:
                        wps = psw.tile([WDIM, 512], f32, tag="wp")
                    nc.tensor.matmul(
                        out=wps[:, 0:SLOTW], lhsT=pay3[:, k, :], rhs=oh3[:, k, :],
                        start=ws.is_first[ch], stop=ws.is_last[ch])
                    if ws.is_last[ch]:
                        sl = tfsum_sb[:, t * SLOTW:(t + 1) * SLOTW]
                        nc.vector.tensor_tensor(out=sl, in0=sl,
                                                in1=wps[:, 0:SLOTW],
                                                op=mybir.AluOpType.add)

            for t in range(IT):
                nc.scalar.activation(
                    tfT_sb[:, t * 128:(t + 1) * 128],
                    tfsum_sb[:, t * 128:(t + 1) * 128],
                    mybir.ActivationFunctionType.Copy)

            # ---- f^T = lrelu(lw^T cat^T + lb); fh feeds item-tile x2 ----
            for t in range(IT):
                fp = psm.tile([DIM, 512], f32, tag="mm")
                nc.tensor.matmul(out=fp[:, 0:128], lhsT=lw[:, 0:DIM],
                                 rhs=vfT_sb[:, t * 128:(t + 1) * 128],
                                 start=True, stop=False)
                nc.tensor.matmul(out=fp[:, 0:128], lhsT=lw[:, DIM:2 * DIM],
                                 rhs=tfT_sb[:, t * 128:(t + 1) * 128],
                                 start=False, stop=True)
                nc.scalar.activation(
                    fT_sb[:, t * 128:(t + 1) * 128], fp[:, 0:128],
                    mybir.ActivationFunctionType.Lrelu,
                    bias=lb[:], alpha=SLOPE)

            # ---- edges: agg^T accumulation with on-the-fly normalize ----
            es_has_pair = set()
            for (_r, _c0, _nb) in es.batches:
                _k = 0
                while _k < _nb:
                    _ch = _c0 + _k
                    if (_k + 1 < _nb) and not es.is_first[_ch + 1]:
                        es_has_pair.add((int(es.region_of[_ch]),
                                         int(es.tile_of[_ch])))
                        _k += 2
                    else:
                        _k += 1
            if True:
              eps = None
              for eq, (r, ch0, nb) in enumerate(es.batches):
                r0, r1 = id_regions[r]
                epay = ewp.tile([128, EB * DIM], f32, tag="epay")
                pay3 = epay[:].rearrange("p (k d) -> p k d", d=DIM)
                nc.gpsimd.dma_gather(
                    epay[:, 0:nb * DIM].rearrange("p (k d) -> p k d", d=DIM),
                    id_in[r0:r1, :],
                    eidx_sb[:, ch0 * 8:(ch0 + nb) * 8],
                    nb * 128, nb * 128, DIM, single_packet=False,
                    queue_num=eq % 4)
                esq = midp.tile([128, EB * DIM], f32, tag="esq")
                sq3 = esq[:].rearrange("p (k d) -> p k d", d=DIM)
                nc.vector.tensor_tensor(out=sq3[:, 0:nb, :], in0=pay3[:, 0:nb, :],
                                        in1=pay3[:, 0:nb, :],
                                        op=mybir.AluOpType.mult)
                ss = ewp.tile([128, EB], f32, tag="ess")
                nc.vector.reduce_sum(out=ss[:, 0:nb], in_=sq3[:, 0:nb, :],
                                     axis=mybir.AxisListType.X)
                nc.scalar.sqrt(ss[:, 0:nb], ss[:, 0:nb])
                nc.vector.reciprocal(ss[:, 0:nb], ss[:, 0:nb])
                epayb = ewp.tile([128, EB * DIM], bf, tag="epayb")
                payb3 = epayb[:].rearrange("p (k d) -> p k d", d=DIM)
                nc.vector.tensor_tensor(
                    out=payb3[:, 0:nb, :], in0=pay3[:, 0:nb, :],
                    in1=ss[:, 0:nb][:, :, None].to_broadcast([128, nb, DIM]),
                    op=mybir.AluOpType.mult)
                eoh = ewp.tile([128, EB * SLOTW], bf, tag="eoh")
                oh3 = eoh[:].rearrange("p (k d) -> p k d", d=SLOTW)
                nc.vector.tensor_tensor(
                    out=oh3[:, 0:nb, :],
                    in0=eloc_sb[:, ch0:ch0 + nb][:, :, None].to_broadcast(
                        [128, nb, SLOTW]),
                    in1=iota[:][:, None, 0:SLOTW].to_broadcast([128, nb, SLOTW]),
                    op=mybir.AluOpType.is_equal)
                k = 0
                while k < nb:
                    ch = ch0 + k
                    t = int(es.tile_of[ch])
                    if es.is_first[ch]:
                        eps = pse.tile([128, 512], f32, tag="ep")
                    pair = (k + 1 < nb) and not es.is_first[ch + 1]
                    if pair:
                        stop = es.is_last[ch + 1]
                        nc.tensor.matmul(
                            out=eps[:, 0:128],
                            lhsT=epayb[:, k * DIM:(k + 2) * DIM],
                            rhs=eoh[:, k * SLOTW:(k + 2) * SLOTW],
                            start=es.is_first[ch], stop=stop)
                        k += 2
                    else:
                        stop = es.is_last[ch]
                        nc.tensor.matmul(
                            out=eps[0:DIM, 0:SLOTW],
                            lhsT=epayb[:, k * DIM:(k + 1) * DIM],
                            rhs=eoh[:, k * SLOTW:(k + 1) * SLOTW],
                            start=es.is_first[ch], stop=stop)
                        k += 1
                    if stop:
                        g = (int(es.region_of[ch]), t)
                        sl = agg_sb[:, t * SLOTW:(t + 1) * SLOTW]
                        nc.vector.tensor_tensor(out=sl, in0=sl,
                                                in1=eps[0:DIM, 0:SLOTW],
                                                op=mybir.AluOpType.add)
                        if g in es_has_pair:
                            nc.vector.tensor_tensor(
                                out=sl, in0=sl,
                                in1=eps[DIM:128, SLOTW:128],
                                op=mybir.AluOpType.add)

              # ---- node tail: x2^T = lrelu(ww^T x1^T (+ w2^T f^T)) ----
              for t in range(NT):
                aggT = xp.tile([DIM, 128], bf, tag="aggT")
                nc.scalar.activation(aggT[:], agg_sb[:, t * 128:(t + 1) * 128],
                                     mybir.ActivationFunctionType.Copy)
                x1p = psm.tile([DIM, 512], f32, tag="mm")
                nc.tensor.matmul(out=x1p[:, 0:128], lhsT=cw[:], rhs=aggT[:],
                                 start=True, stop=True)
                x1T = xp.tile([DIM, 128], bf, tag="x1T")
                nc.scalar.activation(x1T[:], x1p[:, 0:128],
                                     mybir.ActivationFunctionType.Lrelu,
                                     alpha=SLOPE)
                x2p = psm.tile([DIM, 512], f32, tag="mm")
                nc.tensor.matmul(out=x2p[:, 0:128], lhsT=ww[:], rhs=x1T[:],
                                 start=True, stop=(t < IT))
                if t >= IT:
                    ti = t - IT
                    nc.tensor.matmul(out=x2p[:, 0:128], lhsT=w2[:],
                                     rhs=fT_sb[:, ti * 128:(ti + 1) * 128],
                                     start=False, stop=True)
                nc.scalar.activation(x2T_sb[:, t * 128:(t + 1) * 128], x2p[:, 0:128],
                                     mybir.ActivationFunctionType.Lrelu,
                                     alpha=SLOPE)

            # ---- scores: transpose x2^T tiles to rows, route item rows ----
            x2r_u = pp.tile([128, IT * DIM], f32)
            x2r_i = pp.tile([128, IT * DIM], f32)
            for t in range(IT):
                ps_t = psm.tile([128, 512], f32, tag="tr")
                nc.tensor.transpose(out=ps_t[:, 0:DIM],
                                    in_=x2T_sb[:, t * 128:(t + 1) * 128],
                                    identity=ident[0:DIM, 0:DIM])
                nc.scalar.activation(x2r_u[:, t * DIM:(t + 1) * DIM], ps_t[:, 0:DIM],
                                     mybir.ActivationFunctionType.Copy)
            for t in range(IT):
                ps_t = psm.tile([128, 512], f32, tag="tr")
                nc.tensor.transpose(out=ps_t[:, 0:DIM],
                                    in_=x2T_sb[:, (IT + t) * 128:(IT + t + 1) * 128],
                                    identity=ident[0:DIM, 0:DIM])
                nc.scalar.activation(x2r_i[:, t * DIM:(t + 1) * DIM], ps_t[:, 0:DIM],
                                     mybir.ActivationFunctionType.Copy)
            nc.sync.dma_start(
                out=x2i_dram[:, :].rearrange("(t p) d -> p t d", p=128),
                in_=x2r_i[:].rearrange("p (t d) -> p t d", d=DIM))
            ipay = pp.tile([128, IT * DIM], f32)
            nc.gpsimd.dma_gather(
                ipay[:].rearrange("p (k d) -> p k d", d=DIM),
                x2i_dram[:, :],
                sidx_sb[:],
                BPC, BPC, DIM, single_packet=False)
            prod = pp.tile([128, IT * DIM], f32)
            nc.vector.tensor_tensor(
                out=prod[:].rearrange("p (k d) -> p k d", d=DIM),
                in0=x2r_u[:].rearrange("p (k d) -> p k d", d=DIM),
                in1=ipay[:].rearrange("p (k d) -> p k d", d=DIM),
                op=mybir.AluOpType.mult)
            sc = pp.tile([128, 8], f32)
            nc.vector.reduce_sum(out=sc[:],
                                 in_=prod[:].rearrange("p (k d) -> p k d", d=DIM),
                                 axis=mybir.AxisListType.X)
            nc.sync.dma_start(out=out[:], in_=sc[:])

    nc.finalize()
    return nc


# ------------------------------------------------------------------- kernel

def kernel(**inputs):
    from concourse.bass_utils import run_bass_kernel_spmd

    pr = _prep(inputs)
    es, ws = pr["es"], pr["ws"]
    key = es.key() + ws.key()
    if key not in _CACHE:
        _CACHE[key] = _build_program(es, ws)
    nc = _CACHE[key]

    iota_bf = np.broadcast_to(np.arange(128, dtype=bf16), (128, 128)).copy()
    ident = np.eye(128, dtype=np.float32)
    wt_bf = np.asarray(inputs["word_table"], np.float32).astype(bf16)
    lb_col = np.asarray(inputs["lin_b"], np.float32).reshape(DIM, 1).copy()
    cw_bf = np.asarray(inputs["conv_weight"], np.float32).astype(bf16)
    ww_bf = np.asarray(inputs["weight_W"], np.float32).astype(bf16)
    w2_bf = np.asarray(inputs["weight_2"], np.float32).astype(bf16)
    lw_bf = np.asarray(inputs["lin_w"], np.float32).astype(bf16)
    id_emb = np.ascontiguousarray(np.asarray(inputs["id_embedding"], np.float32))

    in_maps = []
    for c in range(NC):
        in_maps.append({
            "id_emb": id_emb,
            "wt_bf": wt_bf,
            "eidx": pr["eidx"][c],
            "eloc": pr["eloc"][c],
            "widx": pr["widx"][c],
            "wloc": pr["wloc"][c],
            "wsc": pr["wsc"][c],
            "vfT": pr["vfT"][c],
            "cw_bf": cw_bf,
            "ww_bf": ww_bf,
            "w2_bf": w2_bf,
            "lw_bf": lw_bf,
            "lb_col": lb_col,
            "ident": ident,
            "sidx": pr["sidx"][c],
            "iota_bf": iota_bf,
        })
    res = run_bass_kernel_spmd(nc, in_maps, list(range(NC)))
    scores = np.empty(B, np.float32)
    for c in range(NC):
        w = res.results[c]["scores_w"]           # [128, 8]
        sc = np.asarray(w, np.float32).T.ravel()  # sc[position]
        scores[pr["outperm"][c]] = sc
    return scores


kernel.run_traced = None  # set by test harness if needed


# revision 36
# speedup vs baseline: 1.2712x; 1.0653x over previous
"""GCMC (gnn_message_passing) Trainium2 Bass kernel, 8-core SPMD.

Measured: 536.7 us HW exec (baseline 1793.4 us), rel L2 err ~2e-3.

Strategy (hardcoded for the nn_GCMC_40870908789353 shapes):
- Pairs are sorted by item and sharded in blocks of 1024 per core, so each
  sampled item's aggregation+words land on exactly one core and are computed
  once (global dedup floor, ~105k gathered rows/core). No collectives: the
  final scores read item x2 rows through one tiny on-core dma_gather.
- All sparse reads use batched dma_gather (SWDGE int16 idxs) on 4 SWDGE
  queues (num_swdge_queues=4; ~4.4 ns/row vs ~9 ns serialized); tables are
  split into <=32767-row regions (id_emb split at the user/item boundary so
  user-dst groups only touch the item region and vice versa).
- Segment-sum runs as one-hot matmuls in transposed form (lhsT=payload,
  rhs=one-hot -> PSUM agg^T/t_feat^T), adjacent same-group edge chunks are
  PAIRED into one [128x128]x[128x128] matmul (diagonal quadrants used, the
  off-diagonal garbage never read). Each (region, dst-group) accumulation
  owns a full PSUM bank, closed within its region pass, then DVE-added into
  SBUF accumulators (matmul start=True clears has_written for the WHOLE
  bank, and only for partitions it writes -- both constraints shaped the
  schedule; batches never cut right after a group's first chunk).
- Edge payload rows are L2-normalized on the fly (square/reduce/rsqrt) which
  folds F.normalize into the gather; the scale-mult also casts to bf16.
  Word payloads are bf16 host-cast and pre-scaled by 1/wordcount(item).
- lin_b rides the ACT bias port; x1@W and f@w2 share one PSUM accumulation.
"""
import sys
for p in ("/opt/trn_rl_repo", "/root/.axon_site/_ro/trn_rl_repo"):
    if p not in sys.path:
        sys.path.insert(0, p)
import numpy as np
import ml_dtypes

NC = 8
NUM_USER = 50000
NUM_ITEM = 20000
NNODE = 70000
VOCAB = 100000
DIM = 64
WDIM = 128
B = 8192
BPC = 1024          # pairs per core
NSLOT = 2048        # node slots per core (1024 user + 1024 item)
NT = 16             # node slot tiles (128 slots, for the x-tail)
IT = 8              # item slot tiles (128 slots, for the f-pipeline)
ET = 32             # edge dst groups (64 slots each)
WT = 16             # word dst groups (64 slots each)
SLOTW = 64          # one-hot width per dst group
E_REG_BOUNDS = (0, 25000, 50000, 70000)   # aligned to user/item boundary
NREG_E = 3
REG_W = 25000       # word_table region rows (4 regions)
NREG_W = 4
EB = 24             # edge chunks per dma_gather batch
WB = 20             # word chunks per dma_gather batch
SLOPE = 0.01

_CACHE = {}

bf16 = ml_dtypes.bfloat16


# ---------------------------------------------------------------- CPU prep

def _ragged_gather(starts, lens):
    """positions [starts[i], starts[i]+lens[i]) concatenated."""
    tot = int(lens.sum())
    if tot == 0:
        return np.zeros(0, np.int64)
    cum = np.cumsum(lens) - lens
    return np.repeat(starts - cum, lens) + np.arange(tot)


class _Sched:
    """Unified SPMD schedule for one gather family."""

    def __init__(self, cnt, n_tiles, n_reg, batch):
        # cnt: [NC, n_reg, n_tiles] instance counts
        nch = np.ceil(cnt / 128.0).astype(np.int64).max(axis=0)  # [n_reg,n_tiles]
        # every tile needs >=1 chunk overall so start/stop exist
        tile_tot = nch.sum(axis=0)
        for t in range(n_tiles):
            if tile_tot[t] == 0:
                nch[0][t] = 1
        self.nch = nch
        self.n_tiles = n_tiles
        self.n_reg = n_reg
        # global chunk order: region-major, tile-minor
        tiles = []
        regions = []
        for r in range(n_reg):
            for t in range(n_tiles):
                tiles += [t] * int(nch[r][t])
                regions += [r] * int(nch[r][t])
        self.tile_of = np.array(tiles, np.int64)
        self.region_of = np.array(regions, np.int64)
        self.NCH = len(tiles)
        self.S = self.NCH * 128
        # start/stop flags per chunk at (region, tile) GROUP granularity: each
        # group is one PSUM accumulation (own bank) closed within its region.
        self.is_first = []
        self.is_last = []
        for ch in range(len(tiles)):
            r, t = regions[ch], tiles[ch]
            self.is_first.append(ch == 0 or (regions[ch - 1], tiles[ch - 1]) != (r, t))
            self.is_last.append(ch == len(tiles) - 1
                                or (regions[ch + 1], tiles[ch + 1]) != (r, t))
        # group (r,t) -> starting chunk
        self.group_ch0 = np.zeros((n_reg, n_tiles), np.int64)
        ch = 0
        for r in range(n_reg):
            for t in range(n_tiles):
                self.group_ch0[r][t] = ch
                ch += int(nch[r][t])
        # batches: split each region's chunk range into <= batch chunks
        self.batches = []  # (region, ch0, nchunks)
        for r in range(n_reg):
            r0 = int(self.group_ch0[r][0])
            r1 = int(self.group_ch0[r + 1][0]) if r + 1 < n_reg else self.NCH
            ch = r0
            while ch < r1:
                nb = min(batch, r1 - ch)
                # Never cut a batch right after a group's first chunk: a
                # continuing group would then open with a 64-partition single
                # whose start=True clears has_written only for partitions
                # 0-63, leaving the pairs' q11 half to accumulate onto the
                # PSUM slot's stale contents.
                if ch + nb < r1:
                    last = ch + nb - 1
                    if self.is_first[last] and not self.is_last[last]:
                        nb -= 1
                self.batches.append((r, ch, nb))
                ch += nb

    def key(self):
        return (self.n_tiles, self.n_reg) + tuple(self.nch.ravel().tolist())


def _fill_stream(sched, slot_rep, loc_val, region_rep, scale=None):
    """Place instances into the padded stream. Returns (idx_stream int16,
    loc_stream bf16, scale_stream bf16 or None)."""
    n_tiles = sched.n_tiles
    key = region_rep * n_tiles + (slot_rep >> 6)
    order = np.argsort(key, kind="stable")
    skey = key[order]
    gcnt = np.bincount(skey, minlength=sched.n_reg * n_tiles)
    # position of each sorted instance: group base*128 + within-group offset
    ch0 = sched.group_ch0.ravel()
    base = np.repeat(ch0 * 128, gcnt)
    within = np.arange(len(order)) - np.repeat(np.cumsum(gcnt) - gcnt, gcnt)
    pos = base + within
    idx_stream = np.zeros(sched.S, np.int16)
    idx_stream[pos] = loc_val[order].astype(np.int16)
    loc_stream = np.full(sched.S, -1.0, bf16)
    loc_stream[pos] = (slot_rep[order] & 63).astype(bf16)
    sc_stream = None
    if scale is not None:
        sc_stream = np.zeros(sched.S, bf16)
        sc_stream[pos] = scale[order].astype(bf16)
    return idx_stream, loc_stream, sc_stream


def _wrap_idx(idx_stream):
    """[S] int16 -> [128, S/16] wrapped+replicated layout."""
    S = idx_stream.shape[0]
    base = idx_stream.reshape(S // 16, 16).T  # [16, S/16]
    return np.ascontiguousarray(np.tile(base, (8, 1)))


def _per_chunk(stream):
    """[S] -> [128, NCH]: position i=(ch*128+p) -> [p, ch]."""
    NCH = stream.shape[0] // 128
    return np.ascontiguousarray(stream.reshape(NCH, 128).T)


def _prep(inputs):
    edge_index = np.asarray(inputs["edge_index"])
    words_tensor = np.asarray(inputs["words_tensor"])
    user_nodes = np.asarray(inputs["user_nodes"]).astype(np.int64)
    item_nodes = np.asarray(inputs["item_nodes"]).astype(np.int64)

    src = edge_index[0].astype(np.int64)
    dst = edge_index[1].astype(np.int64)
    witem = words_tensor[0].astype(np.int64)
    wword = words_tensor[1].astype(np.int64)

    eorder = np.argsort(dst, kind="stable")
    sdst = dst[eorder]
    ssrc = src[eorder]
    worder = np.argsort(witem, kind="stable")
    switem_srt = witem[worder]
    swword = wword[worder]

    deg = np.bincount(dst, minlength=NNODE)
    wc_item = np.bincount(witem, minlength=NUM_ITEM)

    def snake_pos(n):
        i = np.arange(n)
        rnd, lane = divmod(i, 16)
        g = np.where(rnd % 2 == 0, lane, 15 - lane)
        return g * 64 + rnd

    # cluster pairs by item: each item's aggregation lands on one core
    gorder = np.argsort(item_nodes, kind="stable")

    e_data, w_data = [], []
    outperm = np.zeros((NC, BPC), np.int64)
    sidx = np.zeros((NC, 128, BPC // 16), np.int16)
    cnt_e = np.zeros((NC, NREG_E, ET), np.int64)
    cnt_w = np.zeros((NC, NREG_W, WT), np.int64)
    vfT = np.zeros((NC, WDIM, BPC), bf16)
    v_feat = np.asarray(inputs["v_feat"], np.float32)
    for c in range(NC):
        P = gorder[c * BPC:(c + 1) * BPC]
        users = user_nodes[P]
        items = item_nodes[P]
        # user position permutation (balance by degree, snake)
        order_u = np.argsort(-deg[users], kind="stable")
        pos_u = snake_pos(BPC)
        uperm = np.empty(BPC, np.int64)       # uperm[position] = pair rank in P
        uperm[pos_u] = order_u
        outperm[c] = P[uperm]
        # unique items -> balanced slot positions
        uit = np.unique(items)                # sorted node ids
        nu = len(uit)
        iid = uit - NUM_USER
        order_i = np.argsort(-(deg[uit] + wc_item[iid]), kind="stable")
        # rank r (in uit order) -> its balance order index, then snake position
        inv = np.empty(nu, np.int64)
        inv[order_i] = np.arange(nu)
        ipos_of_rank = snake_pos(nu)[inv]
        # final-score gather: position q -> item slot position
        islot_of_pair = ipos_of_rank[np.searchsorted(uit, items[uperm])]
        st16 = islot_of_pair.astype(np.int16)
        sidx[c] = np.ascontiguousarray(
            np.tile(st16.reshape(BPC // 16, 16).T, (8, 1)))

        # edge instances: user positions + unique-item slots
        nodes_e = np.concatenate([users[uperm], uit])
        slots_e = np.concatenate([np.arange(BPC), BPC + ipos_of_rank])
        st = np.searchsorted(sdst, nodes_e)
        en = np.searchsorted(sdst, nodes_e, side="right")
        lens = en - st
        slot_rep = np.repeat(slots_e, lens)
        src_rep = ssrc[_ragged_gather(st, lens)]
        reg_rep = np.searchsorted(np.array(E_REG_BOUNDS[1:-1]), src_rep,
                                  side="right")
        loc_rep = src_rep - np.array(E_REG_BOUNDS)[reg_rep]
        np.add.at(cnt_e[c], (reg_rep, slot_rep >> 6), 1)
        e_data.append((slot_rep, loc_rep, reg_rep))

        # word instances per unique item
        wst = np.searchsorted(switem_srt, iid)
        wen = np.searchsorted(switem_srt, iid, side="right")
        wlens = wen - wst
        wslot_rep = np.repeat(ipos_of_rank, wlens)
        word_rep = swword[_ragged_gather(wst, wlens)]
        wreg_rep = word_rep // REG_W
        wloc_rep = word_rep - wreg_rep * REG_W
        np.add.at(cnt_w[c], (wreg_rep, wslot_rep >> 6), 1)
        winv = (1.0 / np.maximum(wlens, 1)).astype(np.float32)
        wscale_rep = np.repeat(winv, wlens)
        w_data.append((wslot_rep, wloc_rep, wreg_rep, wscale_rep))

        vf_pos = np.zeros((BPC, WDIM), np.float32)
        vf_pos[ipos_of_rank] = v_feat[iid]
        vfT[c] = vf_pos.T.astype(bf16)

    es = _Sched(cnt_e, ET, NREG_E, EB)
    ws = _Sched(cnt_w, WT, NREG_W, WB)

    eidx = np.zeros((NC, 128, es.S // 16), np.int16)
    eloc = np.zeros((NC, 128, es.NCH), bf16)
    widx = np.zeros((NC, 128, ws.S // 16), np.int16)
    wloc = np.zeros((NC, 128, ws.NCH), bf16)
    wsc = np.zeros((NC, 128, ws.NCH), bf16)
    for c in range(NC):
        slot_rep, loc_rep, reg_rep = e_data[c]
        i_s, l_s, _ = _fill_stream(es, slot_rep, loc_rep, reg_rep)
        eidx[c] = _wrap_idx(i_s)
        eloc[c] = _per_chunk(l_s)
        wslot_rep, wloc_rep, wreg_rep, wscale_rep = w_data[c]
        i_s, l_s, s_s = _fill_stream(ws, wslot_rep, wloc_rep, wreg_rep,
                                     scale=wscale_rep)
        widx[c] = _wrap_idx(i_s)
        wloc[c] = _per_chunk(l_s)
        wsc[c] = _per_chunk(s_s)

    return dict(es=es, ws=ws, eidx=eidx, eloc=eloc,
                widx=widx, wloc=wloc, wsc=wsc, vfT=vfT,
                sidx=sidx, outperm=outperm)


# ------------------------------------------------------------- bass program

def _build_program(es, ws):
    from concourse import bass, bacc, mybir
    import concourse.tile as tile
    dt = mybir.dt

    nc = bacc.Bacc(None, target_bir_lowering=False, num_swdge_queues=4)
    f32 = dt.float32
    bf = dt.bfloat16

    id_in = nc.dram_tensor("id_emb", [NNODE, DIM], f32, kind="ExternalInput")
    wt_in = nc.dram_tensor("wt_bf", [VOCAB, WDIM], bf, kind="ExternalInput")
    eidx_in = nc.dram_tensor("eidx", [128, es.S // 16], dt.int16, kind="ExternalInput")
    eloc_in = nc.dram_tensor("eloc", [128, es.NCH], bf, kind="ExternalInput")
    widx_in = nc.dram_tensor("widx", [128, ws.S // 16], dt.int16, kind="ExternalInput")
    wloc_in = nc.dram_tensor("wloc", [128, ws.NCH], bf, kind="ExternalInput")
    wsc_in = nc.dram_tensor("wsc", [128, ws.NCH], bf, kind="ExternalInput")
    vfT_in = nc.dram_tensor("vfT", [WDIM, BPC], bf, kind="ExternalInput")
    cw_in = nc.dram_tensor("cw_bf", [DIM, DIM], bf, kind="ExternalInput")
    ww_in = nc.dram_tensor("ww_bf", [DIM, DIM], bf, kind="ExternalInput")
    w2_in = nc.dram_tensor("w2_bf", [DIM, DIM], bf, kind="ExternalInput")
    lw_in = nc.dram_tensor("lw_bf", [2 * WDIM, DIM], bf, kind="ExternalInput")
    lb_in = nc.dram_tensor("lb_col", [DIM, 1], f32, kind="ExternalInput")
    ident_in = nc.dram_tensor("ident", [128, 128], f32, kind="ExternalInput")
    sidx_in = nc.dram_tensor("sidx", [128, BPC // 16], dt.int16, kind="ExternalInput")
    iota_in = nc.dram_tensor("iota_bf", [128, 128], bf, kind="ExternalInput")
    out = nc.dram_tensor("scores_w", [128, 8], f32, kind="ExternalOutput")
    x2i_dram = nc.dram_tensor("x2i", [BPC, DIM], f32)

    id_regions = [(E_REG_BOUNDS[i], E_REG_BOUNDS[i + 1]) for i in range(3)]
    wt_regions = [(r * REG_W, (r + 1) * REG_W) for r in range(NREG_W)]

    with tile.TileContext(nc) as tc:
        with tc.tile_pool(name="const", bufs=1) as cpool, \
             tc.tile_pool(name="persist", bufs=1) as pp, \
             tc.tile_pool(name="ewp", bufs=7) as ewp, \
             tc.tile_pool(name="wwp", bufs=6) as wwp, \
             tc.tile_pool(name="mid", bufs=2) as midp, \
             tc.tile_pool(name="xp", bufs=2) as xp, \
             tc.tile_pool(name="psw", bufs=2, space="PSUM") as psw, \
             tc.tile_pool(name="pse", bufs=2, space="PSUM") as pse, \
             tc.tile_pool(name="psm", bufs=2, space="PSUM") as psm:

            iota = cpool.tile([128, 128], bf)
            cw = cpool.tile([DIM, DIM], bf)
            ww = cpool.tile([DIM, DIM], bf)
            w2 = cpool.tile([DIM, DIM], bf)
            lw = cpool.tile([128, 2 * DIM], bf)   # cols 0:64 = v-half, 64:128 = t-half
            lb = cpool.tile([DIM, 1], f32)
            ident = cpool.tile([128, 128], f32)
            sidx_sb = cpool.tile([128, BPC // 16], dt.int16)
            nc.sync.dma_start(out=iota[:], in_=iota_in[:])
            nc.sync.dma_start(out=cw[:], in_=cw_in[:])
            nc.sync.dma_start(out=ww[:], in_=ww_in[:])
            nc.sync.dma_start(out=w2[:], in_=w2_in[:])
            nc.sync.dma_start(out=lw[:, 0:DIM], in_=lw_in[0:128, :])
            nc.sync.dma_start(out=lw[:, DIM:2 * DIM], in_=lw_in[128:256, :])
            nc.sync.dma_start(out=lb[:], in_=lb_in[:])
            nc.sync.dma_start(out=ident[:], in_=ident_in[:])
            nc.sync.dma_start(out=sidx_sb[:], in_=sidx_in[:])
            primer = cpool.tile([128, DIM], f32)
            nc.gpsimd.dma_gather(
                primer[:].rearrange("p (k d) -> p k d", d=DIM),
                id_in[0:25000, :], sidx_sb[:, 0:8],
                128, 128, DIM, single_packet=False)

            eidx_sb = pp.tile([128, es.S // 16], dt.int16)
            eloc_sb = pp.tile([128, es.NCH], bf)
            widx_sb = pp.tile([128, ws.S // 16], dt.int16)
            wloc_sb = pp.tile([128, ws.NCH], bf)
            wsc_sb = pp.tile([128, ws.NCH], bf)
            vfT_sb = pp.tile([WDIM, BPC], bf)
            nc.sync.dma_start(out=eidx_sb[:], in_=eidx_in[:])
            nc.sync.dma_start(out=eloc_sb[:], in_=eloc_in[:])
            nc.sync.dma_start(out=widx_sb[:], in_=widx_in[:])
            nc.sync.dma_start(out=wloc_sb[:], in_=wloc_in[:])
            nc.sync.dma_start(out=wsc_sb[:], in_=wsc_in[:])
            nc.sync.dma_start(out=vfT_sb[:], in_=vfT_in[:])

            tfT_sb = pp.tile([WDIM, IT * 128], bf)
            fT_sb = pp.tile([DIM, IT * 128], bf)
            x2T_sb = pp.tile([DIM, NT * 128], f32)
            tfsum_sb = pp.tile([WDIM, IT * 128], f32)
            agg_sb = pp.tile([DIM, NT * 128], f32)
            nc.vector.memset(tfsum_sb[:], 0.0)
            nc.vector.memset(agg_sb[:], 0.0)

            # ---- words: t_feat^T accumulation ----
            wps = None
            for wq, (r, ch0, nb) in enumerate(ws.batches):
                r0, r1 = wt_regions[r]
                wpay = wwp.tile([128, WB * WDIM], bf, tag="wpay")
                pay3 = wpay[:].rearrange("p (k d) -> p k d", d=WDIM)
                nc.gpsimd.dma_gather(
                    wpay[:, 0:nb * WDIM].rearrange("p (k d) -> p k d", d=WDIM),
                    wt_in[r0:r1, :],
                    widx_sb[:, ch0 * 8:(ch0 + nb) * 8],
                    nb * 128, nb * 128, WDIM, single_packet=False,
                    queue_num=wq % 4)
                woh = wwp.tile([128, WB * SLOTW], bf, tag="woh")
                oh3 = woh[:].rearrange("p (k d) -> p k d", d=SLOTW)
                nc.vector.tensor_tensor(
                    out=oh3[:, 0:nb, :],
                    in0=wloc_sb[:, ch0:ch0 + nb][:, :, None].to_broadcast(
                        [128, nb, SLOTW]),
                    in1=iota[:][:, None, 0:SLOTW].to_broadcast([128, nb, SLOTW]),
                    op=mybir.AluOpType.is_equal)
                nc.vector.tensor_tensor(
                    out=oh3[:, 0:nb, :], in0=oh3[:, 0:nb, :],
                    in1=wsc_sb[:, ch0:ch0 + nb][:, :, None].to_broadcast(
                        [128, nb, SLOTW]),
                    op=mybir.AluOpType.mult)
                for k in range(nb):
                    ch = ch0 + k
                    t = int(ws.tile_of[ch])
                    if ws.is_first[ch]:
                        wps = psw.tile([WDIM, 512], f32, tag="wp")
                    nc.tensor.matmul(
                        out=wps[:, 0:SLOTW], lhsT=pay3[:, k, :], rhs=oh3[:, k, :],
                        start=ws.is_first[ch], stop=ws.is_last[ch])
                    if ws.is_last[ch]:
                        sl = tfsum_sb[:, t * SLOTW:(t + 1) * SLOTW]
                        nc.vector.tensor_tensor(out=sl, in0=sl,
                                                in1=wps[:, 0:SLOTW],
                                                op=mybir.AluOpType.add)

            for t in range(IT):
                nc.scalar.activation(
                    tfT_sb[:, t * 128:(t + 1) * 128],
                    tfsum_sb[:, t * 128:(t + 1) * 128],
                    mybir.ActivationFunctionType.Copy)

            # ---- f^T = lrelu(lw^T cat^T + lb); fh feeds item-tile x2 ----
            for t in range(IT):
                fp = psm.tile([DIM, 512], f32, tag="mm")
                nc.tensor.matmul(out=fp[:, 0:128], lhsT=lw[:, 0:DIM],
                                 rhs=vfT_sb[:, t * 128:(t + 1) * 128],
                                 start=True, stop=False)
                nc.tensor.matmul(out=fp[:, 0:128], lhsT=lw[:, DIM:2 * DIM],
                                 rhs=tfT_sb[:, t * 128:(t + 1) * 128],
                                 start=False, stop=True)
                nc.scalar.activation(
                    fT_sb[:, t * 128:(t + 1) * 128], fp[:, 0:128],
                    mybir.ActivationFunctionType.Lrelu,
                    bias=lb[:], alpha=SLOPE)

            # ---- edges: agg^T accumulation with on-the-fly normalize ----
            es_has_pair = set()
            for (_r, _c0, _nb) in es.batches:
                _k = 0
                while _k < _nb:
                    _ch = _c0 + _k
                    if (_k + 1 < _nb) and not es.is_first[_ch + 1]:
                        es_has_pair.add((int(es.region_of[_ch]),
                                         int(es.tile_of[_ch])))
                        _k += 2
                    else:
                        _k += 1
            if True:
              eps = None
              for eq, (r, ch0, nb) in enumerate(es.batches):
                r0, r1 = id_regions[r]
                epay = ewp.tile([128, EB * DIM], f32, tag="epay")
                pay3 = epay[:].rearrange("p (k d) -> p k d", d=DIM)
                nc.gpsimd.dma_gather(
                    epay[:, 0:nb * DIM].rearrange("p (k d) -> p k d", d=DIM),
                    id_in[r0:r1, :],
                    eidx_sb[:, ch0 * 8:(ch0 + nb) * 8],
                    nb * 128, nb * 128, DIM, single_packet=False,
                    queue_num=eq % 4)
                esq = midp.tile([128, EB * DIM], f32, tag="esq")
                sq3 = esq[:].rearrange("p (k d) -> p k d", d=DIM)
                nc.vector.tensor_tensor(out=sq3[:, 0:nb, :], in0=pay3[:, 0:nb, :],
                                        in1=pay3[:, 0:nb, :],
                                        op=mybir.AluOpType.mult)
                ss = ewp.tile([128, EB], f32, tag="ess")
                nc.vector.reduce_sum(out=ss[:, 0:nb], in_=sq3[:, 0:nb, :],
                                     axis=mybir.AxisListType.X)
                nc.scalar.sqrt(ss[:, 0:nb], ss[:, 0:nb])
                nc.vector.reciprocal(ss[:, 0:nb], ss[:, 0:nb])
                epayb = ewp.tile([128, EB * DIM], bf, tag="epayb")
                payb3 = epayb[:].rearrange("p (k d) -> p k d", d=DIM)
                nc.vector.tensor_tensor(
                    out=payb3[:, 0:nb, :], in0=pay3[:, 0:nb, :],
                    in1=ss[:, 0:nb][:, :, None].to_broadcast([128, nb, DIM]),
                    op=mybir.AluOpType.mult)
                eoh = ewp.tile([128, EB * SLOTW], bf, tag="eoh")
                oh3 = eoh[:].rearrange("p (k d) -> p k d", d=SLOTW)
                nc.vector.tensor_tensor(
                    out=oh3[:, 0:nb, :],
                    in0=eloc_sb[:, ch0:ch0 + nb][:, :, None].to_broadcast(
                        [128, nb, SLOTW]),
                    in1=iota[:][:, None, 0:SLOTW].to_broadcast([128, nb, SLOTW]),
                    op=mybir.AluOpType.is_equal)
                k = 0
                while k < nb:
                    ch = ch0 + k
                    t = int(es.tile_of[ch])
                    if es.is_first[ch]:
                        eps = pse.tile([128, 512], f32, tag="ep")
                    pair = (k + 1 < nb) and not es.is_first[ch + 1]
                    if pair:
                        stop = es.is_last[ch + 1]
                        nc.tensor.matmul(
                            out=eps[:, 0:128],
                            lhsT=epayb[:, k * DIM:(k + 2) * DIM],
                            rhs=eoh[:, k * SLOTW:(k + 2) * SLOTW],
                            start=es.is_first[ch], stop=stop)
                        k += 2
                    else:
                        stop = es.is_last[ch]
                        nc.tensor.matmul(
                            out=eps[0:DIM, 0:SLOTW],
                            lhsT=epayb[:, k * DIM:(k + 1) * DIM],
                            rhs=eoh[:, k * SLOTW:(k + 1) * SLOTW],
                            start=es.is_first[ch], stop=stop)
                        k += 1
                    if stop:
                        g = (int(es.region_of[ch]), t)
                        sl = agg_sb[:, t * SLOTW:(t + 1) * SLOTW]
                        nc.vector.tensor_tensor(out=sl, in0=sl,
                                                in1=eps[0:DIM, 0:SLOTW],
                                                op=mybir.AluOpType.add)
                        if g in es_has_pair:
                            nc.vector.tensor_tensor(
                                out=sl, in0=sl,
                                in1=eps[DIM:128, SLOTW:128],
                                op=mybir.AluOpType.add)

              # ---- node tail: x2^T = lrelu(ww^T x1^T (+ w2^T f^T)) ----
              for t in range(NT):
                aggT = xp.tile([DIM, 128], bf, tag="aggT")
                nc.scalar.activation(aggT[:], agg_sb[:, t * 128:(t + 1) * 128],
                                     mybir.ActivationFunctionType.Copy)
                x1p = psm.tile([DIM, 512], f32, tag="mm")
                nc.tensor.matmul(out=x1p[:, 0:128], lhsT=cw[:], rhs=aggT[:],
                                 start=True, stop=True)
                x1T = xp.tile([DIM, 128], bf, tag="x1T")
                nc.scalar.activation(x1T[:], x1p[:, 0:128],
                                     mybir.ActivationFunctionType.Lrelu,
                                     alpha=SLOPE)
                x2p = psm.tile([DIM, 512], f32, tag="mm")
                nc.tensor.matmul(out=x2p[:, 0:128], lhsT=ww[:], rhs=x1T[:],
                                 start=True, stop=(t < IT))
                if t >= IT:
                    ti = t - IT
                    nc.tensor.matmul(out=x2p[:, 0:128], lhsT=w2[:],
                                     rhs=fT_sb[:, ti * 128:(ti + 1) * 128],
                                     start=False, stop=True)
                nc.scalar.activation(x2T_sb[:, t * 128:(t + 1) * 128], x2p[:, 0:128],
                                     mybir.ActivationFunctionType.Lrelu,
                                     alpha=SLOPE)

            # ---- scores: transpose x2^T tiles to rows, route item rows ----
            x2r_u = pp.tile([128, IT * DIM], f32)
            x2r_i = pp.tile([128, IT * DIM], f32)
            for t in range(IT):
                ps_t = psm.tile([128, 512], f32, tag="tr")
                nc.tensor.transpose(out=ps_t[:, 0:DIM],
                                    in_=x2T_sb[:, t * 128:(t + 1) * 128],
                                    identity=ident[0:DIM, 0:DIM])
                nc.scalar.activation(x2r_u[:, t * DIM:(t + 1) * DIM], ps_t[:, 0:DIM],
                                     mybir.ActivationFunctionType.Copy)
            for t in range(IT):
                ps_t = psm.tile([128, 512], f32, tag="tr")
                nc.tensor.transpose(out=ps_t[:, 0:DIM],
                                    in_=x2T_sb[:, (IT + t) * 128:(IT + t + 1) * 128],
                                    identity=ident[0:DIM, 0:DIM])
                nc.scalar.activation(x2r_i[:, t * DIM:(t + 1) * DIM], ps_t[:, 0:DIM],
                                     mybir.ActivationFunctionType.Copy)
            nc.sync.dma_start(
                out=x2i_dram[:, :].rearrange("(t p) d -> p t d", p=128),
                in_=x2r_i[:].rearrange("p (t d) -> p t d", d=DIM))
            ipay = pp.tile([128, IT * DIM], f32)
            nc.gpsimd.dma_gather(
                ipay[:].rearrange("p (k d) -> p k d", d=DIM),
                x2i_dram[:, :],
                sidx_sb[:],
                BPC, BPC, DIM, single_packet=False)
            prod = pp.tile([128, IT * DIM], f32)
            nc.vector.tensor_tensor(
                out=prod[:].rearrange("p (k d) -> p k d", d=DIM),
                in0=x2r_u[:].rearrange("p (k d) -> p k d", d=DIM),
                in1=ipay[:].rearrange("p (k d) -> p k d", d=DIM),
                op=mybir.AluOpType.mult)
            sc = pp.tile([128, 8], f32)
            nc.vector.reduce_sum(out=sc[:],
                                 in_=prod[:].rearrange("p (k d) -> p k d", d=DIM),
                                 axis=mybir.AxisListType.X)
            nc.sync.dma_start(out=out[:], in_=sc[:])

    nc.finalize()
    return nc


# ------------------------------------------------------------------- kernel

def kernel(**inputs):
    from concourse.bass_utils import run_bass_kernel_spmd

    pr = _prep(inputs)
    es, ws = pr["es"], pr["ws"]
    key = es.key() + ws.key()
    if key not in _CACHE:
        _CACHE[key] = _build_program(es, ws)
    nc = _CACHE[key]

    iota_bf = np.broadcast_to(np.arange(128, dtype=bf16), (128, 128)).copy()
    ident = np.eye(128, dtype=np.float32)
    wt_bf = np.asarray(inputs["word_table"], np.float32).astype(bf16)
    lb_col = np.asarray(inputs["lin_b"], np.float32).reshape(DIM, 1).copy()
    cw_bf = np.asarray(inputs["conv_weight"], np.float32).astype(bf16)
    ww_bf = np.asarray(inputs["weight_W"], np.float32).astype(bf16)
    w2_bf = np.asarray(inputs["weight_2"], np.float32).astype(bf16)
    lw_bf = np.asarray(inputs["lin_w"], np.float32).astype(bf16)
    id_emb = np.ascontiguousarray(np.asarray(inputs["id_embedding"], np.float32))

    in_maps = []
    for c in range(NC):
        in_maps.append({
            "id_emb": id_emb,
            "wt_bf": wt_bf,
            "eidx": pr["eidx"][c],
            "eloc": pr["eloc"][c],
            "widx": pr["widx"][c],
            "wloc": pr["wloc"][c],
            "wsc": pr["wsc"][c],
            "vfT": pr["vfT"][c],
            "cw_bf": cw_bf,
            "ww_bf": ww_bf,
            "w2_bf": w2_bf,
            "lw_bf": lw_bf,
            "lb_col": lb_col,
            "ident": ident,
            "sidx": pr["sidx"][c],
            "iota_bf": iota_bf,
        })
    res = run_bass_kernel_spmd(nc, in_maps, list(range(NC)))
    scores = np.empty(B, np.float32)
    for c in range(NC):
        w = res.results[c]["scores_w"]           # [128, 8]
        sc = np.asarray(w, np.float32).T.ravel()  # sc[position]
        scores[pr["outperm"][c]] = sc
    return scores


kernel.run_traced = None  # set by test harness if needed


# revision 37
# speedup vs baseline: 1.2995x; 1.0223x over previous
"""GCMC (gnn_message_passing) Trainium2 Bass kernel, 8-core SPMD.

Measured: 536.7 us HW exec (baseline 1793.4 us), rel L2 err ~2e-3.

Strategy (hardcoded for the nn_GCMC_40870908789353 shapes):
- Pairs are sorted by item and sharded in blocks of 1024 per core, so each
  sampled item's aggregation+words land on exactly one core and are computed
  once (global dedup floor, ~105k gathered rows/core). No collectives: the
  final scores read item x2 rows through one tiny on-core dma_gather.
- All sparse reads use batched dma_gather (SWDGE int16 idxs) on 4 SWDGE
  queues (num_swdge_queues=4; ~4.4 ns/row vs ~9 ns serialized); tables are
  split into <=32767-row regions (id_emb split at the user/item boundary so
  user-dst groups only touch the item region and vice versa).
- Segment-sum runs as one-hot matmuls in transposed form (lhsT=payload,
  rhs=one-hot -> PSUM agg^T/t_feat^T), adjacent same-group edge chunks are
  PAIRED into one [128x128]x[128x128] matmul (diagonal quadrants used, the
  off-diagonal garbage never read). Each (region, dst-group) accumulation
  owns a full PSUM bank, closed within its region pass, then DVE-added into
  SBUF accumulators (matmul start=True clears has_written for the WHOLE
  bank, and only for partitions it writes -- both constraints shaped the
  schedule; batches never cut right after a group's first chunk).
- Edge payload rows are L2-normalized on the fly (square/reduce/rsqrt) which
  folds F.normalize into the gather; the scale-mult also casts to bf16.
  Word payloads are bf16 host-cast and pre-scaled by 1/wordcount(item).
- lin_b rides the ACT bias port; x1@W and f@w2 share one PSUM accumulation.
"""
import sys
for p in ("/opt/trn_rl_repo", "/root/.axon_site/_ro/trn_rl_repo"):
    if p not in sys.path:
        sys.path.insert(0, p)
import numpy as np
import ml_dtypes

NC = 8
NUM_USER = 50000
NUM_ITEM = 20000
NNODE = 70000
VOCAB = 100000
DIM = 64
WDIM = 128
B = 8192
BPC = 1024          # pairs per core
NSLOT = 2048        # node slots per core (1024 user + 1024 item)
NT = 16             # node slot tiles (128 slots, for the x-tail)
IT = 8              # item slot tiles (128 slots, for the f-pipeline)
ET = 32             # edge dst groups (64 slots each)
WT = 16             # word dst groups (64 slots each)
SLOTW = 64          # one-hot width per dst group
E_REG_BOUNDS = (0, 25000, 50000, 70000)   # aligned to user/item boundary
NREG_E = 3
REG_W = 25000       # word_table region rows (4 regions)
NREG_W = 4
EB = 20             # edge chunks per dma_gather batch
WB = 16             # word chunks per dma_gather batch
SLOPE = 0.01

_CACHE = {}

bf16 = ml_dtypes.bfloat16


# ---------------------------------------------------------------- CPU prep

def _ragged_gather(starts, lens):
    """positions [starts[i], starts[i]+lens[i]) concatenated."""
    tot = int(lens.sum())
    if tot == 0:
        return np.zeros(0, np.int64)
    cum = np.cumsum(lens) - lens
    return np.repeat(starts - cum, lens) + np.arange(tot)


class _Sched:
    """Unified SPMD schedule for one gather family."""

    def __init__(self, cnt, n_tiles, n_reg, batch):
        # cnt: [NC, n_reg, n_tiles] instance counts
        nch = np.ceil(cnt / 128.0).astype(np.int64).max(axis=0)  # [n_reg,n_tiles]
        # every tile needs >=1 chunk overall so start/stop exist
        tile_tot = nch.sum(axis=0)
        for t in range(n_tiles):
            if tile_tot[t] == 0:
                nch[0][t] = 1
        self.nch = nch
        self.n_tiles = n_tiles
        self.n_reg = n_reg
        # global chunk order: region-major, tile-minor
        tiles = []
        regions = []
        for r in range(n_reg):
            for t in range(n_tiles):
                tiles += [t] * int(nch[r][t])
                regions += [r] * int(nch[r][t])
        self.tile_of = np.array(tiles, np.int64)
        self.region_of = np.array(regions, np.int64)
        self.NCH = len(tiles)
        self.S = self.NCH * 128
        # start/stop flags per chunk at (region, tile) GROUP granularity: each
        # group is one PSUM accumulation (own bank) closed within its region.
        self.is_first = []
        self.is_last = []
        for ch in range(len(tiles)):
            r, t = regions[ch], tiles[ch]
            self.is_first.append(ch == 0 or (regions[ch - 1], tiles[ch - 1]) != (r, t))
            self.is_last.append(ch == len(tiles) - 1
                                or (regions[ch + 1], tiles[ch + 1]) != (r, t))
        # group (r,t) -> starting chunk
        self.group_ch0 = np.zeros((n_reg, n_tiles), np.int64)
        ch = 0
        for r in range(n_reg):
            for t in range(n_tiles):
                self.group_ch0[r][t] = ch
                ch += int(nch[r][t])
        # batches: split each region's chunk range into <= batch chunks
        self.batches = []  # (region, ch0, nchunks)
        for r in range(n_reg):
            r0 = int(self.group_ch0[r][0])
            r1 = int(self.group_ch0[r + 1][0]) if r + 1 < n_reg else self.NCH
            ch = r0
            while ch < r1:
                nb = min(batch, r1 - ch)
                # Never cut a batch right after a group's first chunk: a
                # continuing group would then open with a 64-partition single
                # whose start=True clears has_written only for partitions
                # 0-63, leaving the pairs' q11 half to accumulate onto the
                # PSUM slot's stale contents.
                if ch + nb < r1:
                    last = ch + nb - 1
                    if self.is_first[last] and not self.is_last[last]:
                        nb -= 1
                self.batches.append((r, ch, nb))
                ch += nb

    def key(self):
        return (self.n_tiles, self.n_reg) + tuple(self.nch.ravel().tolist())


def _fill_stream(sched, slot_rep, loc_val, region_rep, scale=None):
    """Place instances into the padded stream. Returns (idx_stream int16,
    loc_stream bf16, scale_stream bf16 or None)."""
    n_tiles = sched.n_tiles
    key = region_rep * n_tiles + (slot_rep >> 6)
    order = np.argsort(key, kind="stable")
    skey = key[order]
    gcnt = np.bincount(skey, minlength=sched.n_reg * n_tiles)
    # position of each sorted instance: group base*128 + within-group offset
    ch0 = sched.group_ch0.ravel()
    base = np.repeat(ch0 * 128, gcnt)
    within = np.arange(len(order)) - np.repeat(np.cumsum(gcnt) - gcnt, gcnt)
    pos = base + within
    idx_stream = np.zeros(sched.S, np.int16)
    idx_stream[pos] = loc_val[order].astype(np.int16)
    loc_stream = np.full(sched.S, -1.0, bf16)
    loc_stream[pos] = (slot_rep[order] & 63).astype(bf16)
    sc_stream = None
    if scale is not None:
        sc_stream = np.zeros(sched.S, bf16)
        sc_stream[pos] = scale[order].astype(bf16)
    return idx_stream, loc_stream, sc_stream


def _wrap_idx(idx_stream):
    """[S] int16 -> [128, S/16] wrapped+replicated layout."""
    S = idx_stream.shape[0]
    base = idx_stream.reshape(S // 16, 16).T  # [16, S/16]
    return np.ascontiguousarray(np.tile(base, (8, 1)))


def _per_chunk(stream):
    """[S] -> [128, NCH]: position i=(ch*128+p) -> [p, ch]."""
    NCH = stream.shape[0] // 128
    return np.ascontiguousarray(stream.reshape(NCH, 128).T)


def _prep(inputs):
    edge_index = np.asarray(inputs["edge_index"])
    words_tensor = np.asarray(inputs["words_tensor"])
    user_nodes = np.asarray(inputs["user_nodes"]).astype(np.int64)
    item_nodes = np.asarray(inputs["item_nodes"]).astype(np.int64)

    src = edge_index[0].astype(np.int64)
    dst = edge_index[1].astype(np.int64)
    witem = words_tensor[0].astype(np.int64)
    wword = words_tensor[1].astype(np.int64)

    eorder = np.argsort(dst, kind="stable")
    sdst = dst[eorder]
    ssrc = src[eorder]
    worder = np.argsort(witem, kind="stable")
    switem_srt = witem[worder]
    swword = wword[worder]

    deg = np.bincount(dst, minlength=NNODE)
    wc_item = np.bincount(witem, minlength=NUM_ITEM)

    def snake_pos(n):
        i = np.arange(n)
        rnd, lane = divmod(i, 16)
        g = np.where(rnd % 2 == 0, lane, 15 - lane)
        return g * 64 + rnd

    # cluster pairs by item: each item's aggregation lands on one core
    gorder = np.argsort(item_nodes, kind="stable")

    e_data, w_data = [], []
    outperm = np.zeros((NC, BPC), np.int64)
    sidx = np.zeros((NC, 128, BPC // 16), np.int16)
    cnt_e = np.zeros((NC, NREG_E, ET), np.int64)
    cnt_w = np.zeros((NC, NREG_W, WT), np.int64)
    vfT = np.zeros((NC, WDIM, BPC), bf16)
    v_feat = np.asarray(inputs["v_feat"], np.float32)
    for c in range(NC):
        P = gorder[c * BPC:(c + 1) * BPC]
        users = user_nodes[P]
        items = item_nodes[P]
        # user position permutation (balance by degree, snake)
        order_u = np.argsort(-deg[users], kind="stable")
        pos_u = snake_pos(BPC)
        uperm = np.empty(BPC, np.int64)       # uperm[position] = pair rank in P
        uperm[pos_u] = order_u
        outperm[c] = P[uperm]
        # unique items -> balanced slot positions
        uit = np.unique(items)                # sorted node ids
        nu = len(uit)
        iid = uit - NUM_USER
        order_i = np.argsort(-(deg[uit] + wc_item[iid]), kind="stable")
        # rank r (in uit order) -> its balance order index, then snake position
        inv = np.empty(nu, np.int64)
        inv[order_i] = np.arange(nu)
        ipos_of_rank = snake_pos(nu)[inv]
        # final-score gather: position q -> item slot position
        islot_of_pair = ipos_of_rank[np.searchsorted(uit, items[uperm])]
        st16 = islot_of_pair.astype(np.int16)
        sidx[c] = np.ascontiguousarray(
            np.tile(st16.reshape(BPC // 16, 16).T, (8, 1)))

        # edge instances: user positions + unique-item slots
        nodes_e = np.concatenate([users[uperm], uit])
        slots_e = np.concatenate([np.arange(BPC), BPC + ipos_of_rank])
        st = np.searchsorted(sdst, nodes_e)
        en = np.searchsorted(sdst, nodes_e, side="right")
        lens = en - st
        slot_rep = np.repeat(slots_e, lens)
        src_rep = ssrc[_ragged_gather(st, lens)]
        reg_rep = np.searchsorted(np.array(E_REG_BOUNDS[1:-1]), src_rep,
                                  side="right")
        loc_rep = src_rep - np.array(E_REG_BOUNDS)[reg_rep]
        np.add.at(cnt_e[c], (reg_rep, slot_rep >> 6), 1)
        e_data.append((slot_rep, loc_rep, reg_rep))

        # word instances per unique item
        wst = np.searchsorted(switem_srt, iid)
        wen = np.searchsorted(switem_srt, iid, side="right")
        wlens = wen - wst
        wslot_rep = np.repeat(ipos_of_rank, wlens)
        word_rep = swword[_ragged_gather(wst, wlens)]
        wreg_rep = word_rep // REG_W
        wloc_rep = word_rep - wreg_rep * REG_W
        np.add.at(cnt_w[c], (wreg_rep, wslot_rep >> 6), 1)
        winv = (1.0 / np.maximum(wlens, 1)).astype(np.float32)
        wscale_rep = np.repeat(winv, wlens)
        w_data.append((wslot_rep, wloc_rep, wreg_rep, wscale_rep))

        vf_pos = np.zeros((BPC, WDIM), np.float32)
        vf_pos[ipos_of_rank] = v_feat[iid]
        vfT[c] = vf_pos.T.astype(bf16)

    es = _Sched(cnt_e, ET, NREG_E, EB)
    ws = _Sched(cnt_w, WT, NREG_W, WB)

    eidx = np.zeros((NC, 128, es.S // 16), np.int16)
    eloc = np.zeros((NC, 128, es.NCH), bf16)
    widx = np.zeros((NC, 128, ws.S // 16), np.int16)
    wloc = np.zeros((NC, 128, ws.NCH), bf16)
    wsc = np.zeros((NC, 128, ws.NCH), bf16)
    for c in range(NC):
        slot_rep, loc_rep, reg_rep = e_data[c]
        i_s, l_s, _ = _fill_stream(es, slot_rep, loc_rep, reg_rep)
        eidx[c] = _wrap_idx(i_s)
        eloc[c] = _per_chunk(l_s)
        wslot_rep, wloc_rep, wreg_rep, wscale_rep = w_data[c]
        i_s, l_s, s_s = _fill_stream(ws, wslot_rep, wloc_rep, wreg_rep,
                                     scale=wscale_rep)
        widx[c] = _wrap_idx(i_s)
        wloc[c] = _per_chunk(l_s)
        wsc[c] = _per_chunk(s_s)

    return dict(es=es, ws=ws, eidx=eidx, eloc=eloc,
                widx=widx, wloc=wloc, wsc=wsc, vfT=vfT,
                sidx=sidx, outperm=outperm)


# ------------------------------------------------------------- bass program

def _build_program(es, ws):
    from concourse import bass, bacc, mybir
    import concourse.tile as tile
    dt = mybir.dt

    nc = bacc.Bacc(None, target_bir_lowering=False, num_swdge_queues=4)
    f32 = dt.float32
    bf = dt.bfloat16

    id_in = nc.dram_tensor("id_emb", [NNODE, DIM], f32, kind="ExternalInput")
    wt_in = nc.dram_tensor("wt_bf", [VOCAB, WDIM], bf, kind="ExternalInput")
    eidx_in = nc.dram_tensor("eidx", [128, es.S // 16], dt.int16, kind="ExternalInput")
    eloc_in = nc.dram_tensor("eloc", [128, es.NCH], bf, kind="ExternalInput")
    widx_in = nc.dram_tensor("widx", [128, ws.S // 16], dt.int16, kind="ExternalInput")
    wloc_in = nc.dram_tensor("wloc", [128, ws.NCH], bf, kind="ExternalInput")
    wsc_in = nc.dram_tensor("wsc", [128, ws.NCH], bf, kind="ExternalInput")
    vfT_in = nc.dram_tensor("vfT", [WDIM, BPC], bf, kind="ExternalInput")
    cw_in = nc.dram_tensor("cw_bf", [DIM, DIM], bf, kind="ExternalInput")
    ww_in = nc.dram_tensor("ww_bf", [DIM, DIM], bf, kind="ExternalInput")
    w2_in = nc.dram_tensor("w2_bf", [DIM, DIM], bf, kind="ExternalInput")
    lw_in = nc.dram_tensor("lw_bf", [2 * WDIM, DIM], bf, kind="ExternalInput")
    lb_in = nc.dram_tensor("lb_col", [DIM, 1], f32, kind="ExternalInput")
    ident_in = nc.dram_tensor("ident", [128, 128], f32, kind="ExternalInput")
    sidx_in = nc.dram_tensor("sidx", [128, BPC // 16], dt.int16, kind="ExternalInput")
    iota_in = nc.dram_tensor("iota_bf", [128, 128], bf, kind="ExternalInput")
    out = nc.dram_tensor("scores_w", [128, 8], f32, kind="ExternalOutput")
    x2i_dram = nc.dram_tensor("x2i", [BPC, DIM], f32)

    id_regions = [(E_REG_BOUNDS[i], E_REG_BOUNDS[i + 1]) for i in range(3)]
    wt_regions = [(r * REG_W, (r + 1) * REG_W) for r in range(NREG_W)]

    with tile.TileContext(nc) as tc:
        with tc.tile_pool(name="const", bufs=1) as cpool, \
             tc.tile_pool(name="persist", bufs=1) as pp, \
             tc.tile_pool(name="ewp", bufs=8) as ewp, \
             tc.tile_pool(name="wwp", bufs=7) as wwp, \
             tc.tile_pool(name="mid", bufs=2) as midp, \
             tc.tile_pool(name="xp", bufs=2) as xp, \
             tc.tile_pool(name="psw", bufs=2, space="PSUM") as psw, \
             tc.tile_pool(name="pse", bufs=2, space="PSUM") as pse, \
             tc.tile_pool(name="psm", bufs=2, space="PSUM") as psm:

            iota = cpool.tile([128, 128], bf)
            cw = cpool.tile([DIM, DIM], bf)
            ww = cpool.tile([DIM, DIM], bf)
            w2 = cpool.tile([DIM, DIM], bf)
            lw = cpool.tile([128, 2 * DIM], bf)   # cols 0:64 = v-half, 64:128 = t-half
            lb = cpool.tile([DIM, 1], f32)
            ident = cpool.tile([128, 128], f32)
            sidx_sb = cpool.tile([128, BPC // 16], dt.int16)
            nc.sync.dma_start(out=iota[:], in_=iota_in[:])
            nc.sync.dma_start(out=cw[:], in_=cw_in[:])
            nc.sync.dma_start(out=ww[:], in_=ww_in[:])
            nc.sync.dma_start(out=w2[:], in_=w2_in[:])
            nc.sync.dma_start(out=lw[:, 0:DIM], in_=lw_in[0:128, :])
            nc.sync.dma_start(out=lw[:, DIM:2 * DIM], in_=lw_in[128:256, :])
            nc.sync.dma_start(out=lb[:], in_=lb_in[:])
            nc.sync.dma_start(out=ident[:], in_=ident_in[:])
            nc.sync.dma_start(out=sidx_sb[:], in_=sidx_in[:])
            primer = cpool.tile([128, DIM], f32)
            nc.gpsimd.dma_gather(
                primer[:].rearrange("p (k d) -> p k d", d=DIM),
                id_in[0:25000, :], sidx_sb[:, 0:8],
                128, 128, DIM, single_packet=False)

            eidx_sb = pp.tile([128, es.S // 16], dt.int16)
            eloc_sb = pp.tile([128, es.NCH], bf)
            widx_sb = pp.tile([128, ws.S // 16], dt.int16)
            wloc_sb = pp.tile([128, ws.NCH], bf)
            wsc_sb = pp.tile([128, ws.NCH], bf)
            vfT_sb = pp.tile([WDIM, BPC], bf)
            nc.sync.dma_start(out=eidx_sb[:], in_=eidx_in[:])
            nc.sync.dma_start(out=eloc_sb[:], in_=eloc_in[:])
            nc.sync.dma_start(out=widx_sb[:], in_=widx_in[:])
            nc.sync.dma_start(out=wloc_sb[:], in_=wloc_in[:])
            nc.sync.dma_start(out=wsc_sb[:], in_=wsc_in[:])
            nc.sync.dma_start(out=vfT_sb[:], in_=vfT_in[:])

            tfT_sb = pp.tile([WDIM, IT * 128], bf)
            fT_sb = pp.tile([DIM, IT * 128], bf)
            x2T_sb = pp.tile([DIM, NT * 128], f32)
            tfsum_sb = pp.tile([WDIM, IT * 128], f32)
            agg_sb = pp.tile([DIM, NT * 128], f32)
            nc.vector.memset(tfsum_sb[:], 0.0)
            nc.vector.memset(agg_sb[:], 0.0)

            # ---- words: t_feat^T accumulation ----
            wps = None
            for wq, (r, ch0, nb) in enumerate(ws.batches):
                r0, r1 = wt_regions[r]
                wpay = wwp.tile([128, WB * WDIM], bf, tag="wpay")
                pay3 = wpay[:].rearrange("p (k d) -> p k d", d=WDIM)
                nc.gpsimd.dma_gather(
                    wpay[:, 0:nb * WDIM].rearrange("p (k d) -> p k d", d=WDIM),
                    wt_in[r0:r1, :],
                    widx_sb[:, ch0 * 8:(ch0 + nb) * 8],
                    nb * 128, nb * 128, WDIM, single_packet=False,
                    queue_num=wq % 4)
                woh = wwp.tile([128, WB * SLOTW], bf, tag="woh")
                oh3 = woh[:].rearrange("p (k d) -> p k d", d=SLOTW)
                nc.vector.tensor_tensor(
                    out=oh3[:, 0:nb, :],
                    in0=wloc_sb[:, ch0:ch0 + nb][:, :, None].to_broadcast(
                        [128, nb, SLOTW]),
                    in1=iota[:][:, None, 0:SLOTW].to_broadcast([128, nb, SLOTW]),
                    op=mybir.AluOpType.is_equal)
                nc.vector.tensor_tensor(
                    out=oh3[:, 0:nb, :], in0=oh3[:, 0:nb, :],
                    in1=wsc_sb[:, ch0:ch0 + nb][:, :, None].to_broadcast(
                        [128, nb, SLOTW]),
                    op=mybir.AluOpType.mult)
                for k in range(nb):
                    ch = ch0 + k
                    t = int(ws.tile_of[ch])
                    if ws.is_first[ch]:
                        wps = psw.tile([WDIM, 512], f32, tag="wp")
                    nc.tensor.matmul(
                        out=wps[:, 0:SLOTW], lhsT=pay3[:, k, :], rhs=oh3[:, k, :],
                        start=ws.is_first[ch], stop=ws.is_last[ch])
                    if ws.is_last[ch]:
                        sl = tfsum_sb[:, t * SLOTW:(t + 1) * SLOTW]
                        nc.vector.tensor_tensor(out=sl, in0=sl,
                                                in1=wps[:, 0:SLOTW],
                                                op=mybir.AluOpType.add)

            for t in range(IT):
                nc.scalar.activation(
                    tfT_sb[:, t * 128:(t + 1) * 128],
                    tfsum_sb[:, t * 128:(t + 1) * 128],
                    mybir.ActivationFunctionType.Copy)

            # ---- f^T = lrelu(lw^T cat^T + lb); fh feeds item-tile x2 ----
            for t in range(IT):
                fp = psm.tile([DIM, 512], f32, tag="mm")
                nc.tensor.matmul(out=fp[:, 0:128], lhsT=lw[:, 0:DIM],
                                 rhs=vfT_sb[:, t * 128:(t + 1) * 128],
                                 start=True, stop=False)
                nc.tensor.matmul(out=fp[:, 0:128], lhsT=lw[:, DIM:2 * DIM],
                                 rhs=tfT_sb[:, t * 128:(t + 1) * 128],
                                 start=False, stop=True)
                nc.scalar.activation(
                    fT_sb[:, t * 128:(t + 1) * 128], fp[:, 0:128],
                    mybir.ActivationFunctionType.Lrelu,
                    bias=lb[:], alpha=SLOPE)

            # ---- edges: agg^T accumulation with on-the-fly normalize ----
            es_has_pair = set()
            for (_r, _c0, _nb) in es.batches:
                _k = 0
                while _k < _nb:
                    _ch = _c0 + _k
                    if (_k + 1 < _nb) and not es.is_first[_ch + 1]:
                        es_has_pair.add((int(es.region_of[_ch]),
                                         int(es.tile_of[_ch])))
                        _k += 2
                    else:
                        _k += 1
            if True:
              eps = None
              for eq, (r, ch0, nb) in enumerate(es.batches):
                r0, r1 = id_regions[r]
                epay = ewp.tile([128, EB * DIM], f32, tag="epay")
                pay3 = epay[:].rearrange("p (k d) -> p k d", d=DIM)
                nc.gpsimd.dma_gather(
                    epay[:, 0:nb * DIM].rearrange("p (k d) -> p k d", d=DIM),
                    id_in[r0:r1, :],
                    eidx_sb[:, ch0 * 8:(ch0 + nb) * 8],
                    nb * 128, nb * 128, DIM, single_packet=False,
                    queue_num=eq % 4)
                esq = midp.tile([128, EB * DIM], f32, tag="esq")
                sq3 = esq[:].rearrange("p (k d) -> p k d", d=DIM)
                nc.vector.tensor_tensor(out=sq3[:, 0:nb, :], in0=pay3[:, 0:nb, :],
                                        in1=pay3[:, 0:nb, :],
                                        op=mybir.AluOpType.mult)
                ss = ewp.tile([128, EB], f32, tag="ess")
                nc.vector.reduce_sum(out=ss[:, 0:nb], in_=sq3[:, 0:nb, :],
                                     axis=mybir.AxisListType.X)
                nc.scalar.sqrt(ss[:, 0:nb], ss[:, 0:nb])
                nc.vector.reciprocal(ss[:, 0:nb], ss[:, 0:nb])
                epayb = ewp.tile([128, EB * DIM], bf, tag="epayb")
                payb3 = epayb[:].rearrange("p (k d) -> p k d", d=DIM)
                nc.vector.tensor_tensor(
                    out=payb3[:, 0:nb, :], in0=pay3[:, 0:nb, :],
                    in1=ss[:, 0:nb][:, :, None].to_broadcast([128, nb, DIM]),
                    op=mybir.AluOpType.mult)
                eoh = ewp.tile([128, EB * SLOTW], bf, tag="eoh")
                oh3 = eoh[:].rearrange("p (k d) -> p k d", d=SLOTW)
                nc.vector.tensor_tensor(
                    out=oh3[:, 0:nb, :],
                    in0=eloc_sb[:, ch0:ch0 + nb][:, :, None].to_broadcast(
                        [128, nb, SLOTW]),
                    in1=iota[:][:, None, 0:SLOTW].to_broadcast([128, nb, SLOTW]),
                    op=mybir.AluOpType.is_equal)
                k = 0
                while k < nb:
                    ch = ch0 + k
                    t = int(es.tile_of[ch])
                    if es.is_first[ch]:
                        eps = pse.tile([128, 512], f32, tag="ep")
                    pair = (k + 1 < nb) and not es.is_first[ch + 1]
                    if pair:
                        stop = es.is_last[ch + 1]
                        nc.tensor.matmul(
                            out=eps[:, 0:128],
                            lhsT=epayb[:, k * DIM:(k + 2) * DIM],
                            rhs=eoh[:, k * SLOTW:(k + 2) * SLOTW],
                            start=es.is_first[ch], stop=stop)
                        k += 2
                    else:
                        stop = es.is_last[ch]
                        nc.tensor.matmul(
                            out=eps[0:DIM, 0:SLOTW],
                            lhsT=epayb[:, k * DIM:(k + 1) * DIM],
                            rhs=eoh[:, k * SLOTW:(k + 1) * SLOTW],
                            start=es.is_first[ch], stop=stop)
                        k += 1
                    if stop:
                        g = (int(es.region_of[ch]), t)
                        sl = agg_sb[:, t * SLOTW:(t + 1) * SLOTW]
                        nc.vector.tensor_tensor(out=sl, in0=sl,
                                                in1=eps[0:DIM, 0:SLOTW],
                                                op=mybir.AluOpType.add)
                        if g in es_has_pair:
                            nc.vector.tensor_tensor(
                                out=sl, in0=sl,
                                in1=eps[DIM:128, SLOTW:128],
                                op=mybir.AluOpType.add)

              # ---- node tail: x2^T = lrelu(ww^T x1^T (+ w2^T f^T)) ----
              for t in range(NT):
                aggT = xp.tile([DIM, 128], bf, tag="aggT")
                nc.scalar.activation(aggT[:], agg_sb[:, t * 128:(t + 1) * 128],
                                     mybir.ActivationFunctionType.Copy)
                x1p = psm.tile([DIM, 512], f32, tag="mm")
                nc.tensor.matmul(out=x1p[:, 0:128], lhsT=cw[:], rhs=aggT[:],
                                 start=True, stop=True)
                x1T = xp.tile([DIM, 128], bf, tag="x1T")
                nc.scalar.activation(x1T[:], x1p[:, 0:128],
                                     mybir.ActivationFunctionType.Lrelu,
                                     alpha=SLOPE)
                x2p = psm.tile([DIM, 512], f32, tag="mm")
                nc.tensor.matmul(out=x2p[:, 0:128], lhsT=ww[:], rhs=x1T[:],
                                 start=True, stop=(t < IT))
                if t >= IT:
                    ti = t - IT
                    nc.tensor.matmul(out=x2p[:, 0:128], lhsT=w2[:],
                                     rhs=fT_sb[:, ti * 128:(ti + 1) * 128],
                                     start=False, stop=True)
                nc.scalar.activation(x2T_sb[:, t * 128:(t + 1) * 128], x2p[:, 0:128],
                                     mybir.ActivationFunctionType.Lrelu,
                                     alpha=SLOPE)

            # ---- scores: transpose x2^T tiles to rows, route item rows ----
            x2r_u = pp.tile([128, IT * DIM], f32)
            x2r_i = pp.tile([128, IT * DIM], f32)
            for t in range(IT):
                ps_t = psm.tile([128, 512], f32, tag="tr")
                nc.tensor.transpose(out=ps_t[:, 0:DIM],
                                    in_=x2T_sb[:, t * 128:(t + 1) * 128],
                                    identity=ident[0:DIM, 0:DIM])
                nc.scalar.activation(x2r_u[:, t * DIM:(t + 1) * DIM], ps_t[:, 0:DIM],
                                     mybir.ActivationFunctionType.Copy)
            for t in range(IT):
                ps_t = psm.tile([128, 512], f32, tag="tr")
                nc.tensor.transpose(out=ps_t[:, 0:DIM],
                                    in_=x2T_sb[:, (IT + t) * 128:(IT + t + 1) * 128],
                                    identity=ident[0:DIM, 0:DIM])
                nc.scalar.activation(x2r_i[:, t * DIM:(t + 1) * DIM], ps_t[:, 0:DIM],
                                     mybir.ActivationFunctionType.Copy)
            nc.sync.dma_start(
                out=x2i_dram[:, :].rearrange("(t p) d -> p t d", p=128),
                in_=x2r_i[:].rearrange("p (t d) -> p t d", d=DIM))
            ipay = pp.tile([128, IT * DIM], f32)
            nc.gpsimd.dma_gather(
                ipay[:].rearrange("p (k d) -> p k d", d=DIM),
                x2i_dram[:, :],
                sidx_sb[:],
                BPC, BPC, DIM, single_packet=False)
            prod = pp.tile([128, IT * DIM], f32)
            nc.vector.tensor_tensor(
                out=prod[:].rearrange("p (k d) -> p k d", d=DIM),
                in0=x2r_u[:].rearrange("p (k d) -> p k d", d=DIM),
                in1=ipay[:].rearrange("p (k d) -> p k d", d=DIM),
                op=mybir.AluOpType.mult)
            sc = pp.tile([128, 8], f32)
            nc.vector.reduce_sum(out=sc[:],
                                 in_=prod[:].rearrange("p (k d) -> p k d", d=DIM),
                                 axis=mybir.AxisListType.X)
            nc.sync.dma_start(out=out[:], in_=sc[:])

    nc.finalize()
    return nc


# ------------------------------------------------------------------- kernel

def kernel(**inputs):
    from concourse.bass_utils import run_bass_kernel_spmd

    pr = _prep(inputs)
    es, ws = pr["es"], pr["ws"]
    key = es.key() + ws.key()
    if key not in _CACHE:
        _CACHE[key] = _build_program(es, ws)
    nc = _CACHE[key]

    iota_bf = np.broadcast_to(np.arange(128, dtype=bf16), (128, 128)).copy()
    ident = np.eye(128, dtype=np.float32)
    wt_bf = np.asarray(inputs["word_table"], np.float32).astype(bf16)
    lb_col = np.asarray(inputs["lin_b"], np.float32).reshape(DIM, 1).copy()
    cw_bf = np.asarray(inputs["conv_weight"], np.float32).astype(bf16)
    ww_bf = np.asarray(inputs["weight_W"], np.float32).astype(bf16)
    w2_bf = np.asarray(inputs["weight_2"], np.float32).astype(bf16)
    lw_bf = np.asarray(inputs["lin_w"], np.float32).astype(bf16)
    id_emb = np.ascontiguousarray(np.asarray(inputs["id_embedding"], np.float32))

    in_maps = []
    for c in range(NC):
        in_maps.append({
            "id_emb": id_emb,
            "wt_bf": wt_bf,
            "eidx": pr["eidx"][c],
            "eloc": pr["eloc"][c],
            "widx": pr["widx"][c],
            "wloc": pr["wloc"][c],
            "wsc": pr["wsc"][c],
            "vfT": pr["vfT"][c],
            "cw_bf": cw_bf,
            "ww_bf": ww_bf,
            "w2_bf": w2_bf,
            "lw_bf": lw_bf,
            "lb_col": lb_col,
            "ident": ident,
            "sidx": pr["sidx"][c],
            "iota_bf": iota_bf,
        })
    res = run_bass_kernel_spmd(nc, in_maps, list(range(NC)))
    scores = np.empty(B, np.float32)
    for c in range(NC):
        w = res.results[c]["scores_w"]           # [128, 8]
        sc = np.asarray(w, np.float32).T.ravel()  # sc[position]
        scores[pr["outperm"][c]] = sc
    return scores


kernel.run_traced = None  # set by test harness if needed


# revision 39
# speedup vs baseline: 1.3023x; 1.0021x over previous
"""GCMC (gnn_message_passing) Trainium2 Bass kernel, 8-core SPMD.

Measured: 420.2 us HW exec (baseline 1793.4 us, 4.27x), rel L2 err ~2e-3.

Strategy (hardcoded for the nn_GCMC_40870908789353 shapes):
- Pairs are sorted by item and sharded in blocks of 1024 per core, so each
  sampled item's aggregation+words land on exactly one core and are computed
  once (global dedup floor, ~105k gathered rows/core). No collectives: the
  final scores read item x2 rows through one tiny on-core dma_gather.
- All sparse reads use batched dma_gather (SWDGE int16 idxs) on 4 SWDGE
  queues (num_swdge_queues=4) with small batches and 7-8 deep tile pools so
  many gathers stay in flight (~3.4 ns/row vs ~9 ns serialized); a primer
  gather pulls the mlp ucode library load out of the startup shadow. Tables
  are split into <=32767-row regions (id_emb split at the user/item boundary
  so user-dst groups only touch the item region and vice versa).
- Segment-sum runs as one-hot matmuls in transposed form (lhsT=payload,
  rhs=one-hot -> PSUM agg^T/t_feat^T), adjacent same-group edge chunks are
  PAIRED into one [128x128]x[128x128] matmul (diagonal quadrants used, the
  off-diagonal garbage never read). Each (region, dst-group) accumulation
  owns a full PSUM bank, closed within its region pass, then DVE-added into
  SBUF accumulators (matmul start=True clears has_written for the WHOLE
  bank, and only for partitions it writes -- both constraints shaped the
  schedule; batches never cut right after a group's first chunk).
- Edge payload rows are L2-normalized on the fly (square/reduce/rsqrt) which
  folds F.normalize into the gather; the scale-mult also casts to bf16.
  Word payloads are bf16 host-cast and pre-scaled by 1/wordcount(item).
- lin_b rides the ACT bias port; x1@W and f@w2 share one PSUM accumulation.
"""
import sys
for p in ("/opt/trn_rl_repo", "/root/.axon_site/_ro/trn_rl_repo"):
    if p not in sys.path:
        sys.path.insert(0, p)
import numpy as np
import ml_dtypes

NC = 8
NUM_USER = 50000
NUM_ITEM = 20000
NNODE = 70000
VOCAB = 100000
DIM = 64
WDIM = 128
B = 8192
BPC = 1024          # pairs per core
NSLOT = 2048        # node slots per core (1024 user + 1024 item)
NT = 16             # node slot tiles (128 slots, for the x-tail)
IT = 8              # item slot tiles (128 slots, for the f-pipeline)
ET = 32             # edge dst groups (64 slots each)
WT = 16             # word dst groups (64 slots each)
SLOTW = 64          # one-hot width per dst group
E_REG_BOUNDS = (0, 25000, 50000, 70000)   # aligned to user/item boundary
NREG_E = 3
REG_W = 25000       # word_table region rows (4 regions)
NREG_W = 4
EB = 16             # edge chunks per dma_gather batch
WB = 12             # word chunks per dma_gather batch
SLOPE = 0.01

_CACHE = {}

bf16 = ml_dtypes.bfloat16


# ---------------------------------------------------------------- CPU prep

def _ragged_gather(starts, lens):
    """positions [starts[i], starts[i]+lens[i]) concatenated."""
    tot = int(lens.sum())
    if tot == 0:
        return np.zeros(0, np.int64)
    cum = np.cumsum(lens) - lens
    return np.repeat(starts - cum, lens) + np.arange(tot)


class _Sched:
    """Unified SPMD schedule for one gather family."""

    def __init__(self, cnt, n_tiles, n_reg, batch):
        # cnt: [NC, n_reg, n_tiles] instance counts
        nch = np.ceil(cnt / 128.0).astype(np.int64).max(axis=0)  # [n_reg,n_tiles]
        # every tile needs >=1 chunk overall so start/stop exist
        tile_tot = nch.sum(axis=0)
        for t in range(n_tiles):
            if tile_tot[t] == 0:
                nch[0][t] = 1
        self.nch = nch
        self.n_tiles = n_tiles
        self.n_reg = n_reg
        # global chunk order: region-major, tile-minor
        tiles = []
        regions = []
        for r in range(n_reg):
            for t in range(n_tiles):
                tiles += [t] * int(nch[r][t])
                regions += [r] * int(nch[r][t])
        self.tile_of = np.array(tiles, np.int64)
        self.region_of = np.array(regions, np.int64)
        self.NCH = len(tiles)
        self.S = self.NCH * 128
        # start/stop flags per chunk at (region, tile) GROUP granularity: each
        # group is one PSUM accumulation (own bank) closed within its region.
        self.is_first = []
        self.is_last = []
        for ch in range(len(tiles)):
            r, t = regions[ch], tiles[ch]
            self.is_first.append(ch == 0 or (regions[ch - 1], tiles[ch - 1]) != (r, t))
            self.is_last.append(ch == len(tiles) - 1
                                or (regions[ch + 1], tiles[ch + 1]) != (r, t))
        # group (r,t) -> starting chunk
        self.group_ch0 = np.zeros((n_reg, n_tiles), np.int64)
        ch = 0
        for r in range(n_reg):
            for t in range(n_tiles):
                self.group_ch0[r][t] = ch
                ch += int(nch[r][t])
        # batches: split each region's chunk range into <= batch chunks
        self.batches = []  # (region, ch0, nchunks)
        for r in range(n_reg):
            r0 = int(self.group_ch0[r][0])
            r1 = int(self.group_ch0[r + 1][0]) if r + 1 < n_reg else self.NCH
            ch = r0
            while ch < r1:
                nb = min(batch, r1 - ch)
                # Never cut a batch right after a group's first chunk: a
                # continuing group would then open with a 64-partition single
                # whose start=True clears has_written only for partitions
                # 0-63, leaving the pairs' q11 half to accumulate onto the
                # PSUM slot's stale contents.
                if ch + nb < r1:
                    last = ch + nb - 1
                    if self.is_first[last] and not self.is_last[last]:
                        nb -= 1
                self.batches.append((r, ch, nb))
                ch += nb

    def key(self):
        return (self.n_tiles, self.n_reg) + tuple(self.nch.ravel().tolist())


def _fill_stream(sched, slot_rep, loc_val, region_rep, scale=None):
    """Place instances into the padded stream. Returns (idx_stream int16,
    loc_stream bf16, scale_stream bf16 or None)."""
    n_tiles = sched.n_tiles
    key = region_rep * n_tiles + (slot_rep >> 6)
    order = np.argsort(key, kind="stable")
    skey = key[order]
    gcnt = np.bincount(skey, minlength=sched.n_reg * n_tiles)
    # position of each sorted instance: group base*128 + within-group offset
    ch0 = sched.group_ch0.ravel()
    base = np.repeat(ch0 * 128, gcnt)
    within = np.arange(len(order)) - np.repeat(np.cumsum(gcnt) - gcnt, gcnt)
    pos = base + within
    idx_stream = np.zeros(sched.S, np.int16)
    idx_stream[pos] = loc_val[order].astype(np.int16)
    loc_stream = np.full(sched.S, -1.0, bf16)
    loc_stream[pos] = (slot_rep[order] & 63).astype(bf16)
    sc_stream = None
    if scale is not None:
        sc_stream = np.zeros(sched.S, bf16)
        sc_stream[pos] = scale[order].astype(bf16)
    return idx_stream, loc_stream, sc_stream


def _wrap_idx(idx_stream):
    """[S] int16 -> [128, S/16] wrapped+replicated layout."""
    S = idx_stream.shape[0]
    base = idx_stream.reshape(S // 16, 16).T  # [16, S/16]
    return np.ascontiguousarray(np.tile(base, (8, 1)))


def _per_chunk(stream):
    """[S] -> [128, NCH]: position i=(ch*128+p) -> [p, ch]."""
    NCH = stream.shape[0] // 128
    return np.ascontiguousarray(stream.reshape(NCH, 128).T)


def _prep(inputs):
    edge_index = np.asarray(inputs["edge_index"])
    words_tensor = np.asarray(inputs["words_tensor"])
    user_nodes = np.asarray(inputs["user_nodes"]).astype(np.int64)
    item_nodes = np.asarray(inputs["item_nodes"]).astype(np.int64)

    src = edge_index[0].astype(np.int64)
    dst = edge_index[1].astype(np.int64)
    witem = words_tensor[0].astype(np.int64)
    wword = words_tensor[1].astype(np.int64)

    eorder = np.argsort(dst, kind="stable")
    sdst = dst[eorder]
    ssrc = src[eorder]
    worder = np.argsort(witem, kind="stable")
    switem_srt = witem[worder]
    swword = wword[worder]

    deg = np.bincount(dst, minlength=NNODE)
    wc_item = np.bincount(witem, minlength=NUM_ITEM)

    def snake_pos(n):
        i = np.arange(n)
        rnd, lane = divmod(i, 16)
        g = np.where(rnd % 2 == 0, lane, 15 - lane)
        return g * 64 + rnd

    # cluster pairs by item: each item's aggregation lands on one core
    gorder = np.argsort(item_nodes, kind="stable")

    e_data, w_data = [], []
    outperm = np.zeros((NC, BPC), np.int64)
    sidx = np.zeros((NC, 128, BPC // 16), np.int16)
    cnt_e = np.zeros((NC, NREG_E, ET), np.int64)
    cnt_w = np.zeros((NC, NREG_W, WT), np.int64)
    vfT = np.zeros((NC, WDIM, BPC), bf16)
    v_feat = np.asarray(inputs["v_feat"], np.float32)
    for c in range(NC):
        P = gorder[c * BPC:(c + 1) * BPC]
        users = user_nodes[P]
        items = item_nodes[P]
        # user position permutation (balance by degree, snake)
        order_u = np.argsort(-deg[users], kind="stable")
        pos_u = snake_pos(BPC)
        uperm = np.empty(BPC, np.int64)       # uperm[position] = pair rank in P
        uperm[pos_u] = order_u
        outperm[c] = P[uperm]
        # unique items -> balanced slot positions
        uit = np.unique(items)                # sorted node ids
        nu = len(uit)
        iid = uit - NUM_USER
        order_i = np.argsort(-(deg[uit] + wc_item[iid]), kind="stable")
        # rank r (in uit order) -> its balance order index, then snake position
        inv = np.empty(nu, np.int64)
        inv[order_i] = np.arange(nu)
        ipos_of_rank = snake_pos(nu)[inv]
        # final-score gather: position q -> item slot position
        islot_of_pair = ipos_of_rank[np.searchsorted(uit, items[uperm])]
        st16 = islot_of_pair.astype(np.int16)
        sidx[c] = np.ascontiguousarray(
            np.tile(st16.reshape(BPC // 16, 16).T, (8, 1)))

        # edge instances: user positions + unique-item slots
        nodes_e = np.concatenate([users[uperm], uit])
        slots_e = np.concatenate([np.arange(BPC), BPC + ipos_of_rank])
        st = np.searchsorted(sdst, nodes_e)
        en = np.searchsorted(sdst, nodes_e, side="right")
        lens = en - st
        slot_rep = np.repeat(slots_e, lens)
        src_rep = ssrc[_ragged_gather(st, lens)]
        reg_rep = np.searchsorted(np.array(E_REG_BOUNDS[1:-1]), src_rep,
                                  side="right")
        loc_rep = src_rep - np.array(E_REG_BOUNDS)[reg_rep]
        np.add.at(cnt_e[c], (reg_rep, slot_rep >> 6), 1)
        e_data.append((slot_rep, loc_rep, reg_rep))

        # word instances per unique item
        wst = np.searchsorted(switem_srt, iid)
        wen = np.searchsorted(switem_srt, iid, side="right")
        wlens = wen - wst
        wslot_rep = np.repeat(ipos_of_rank, wlens)
        word_rep = swword[_ragged_gather(wst, wlens)]
        wreg_rep = word_rep // REG_W
        wloc_rep = word_rep - wreg_rep * REG_W
        np.add.at(cnt_w[c], (wreg_rep, wslot_rep >> 6), 1)
        winv = (1.0 / np.maximum(wlens, 1)).astype(np.float32)
        wscale_rep = np.repeat(winv, wlens)
        w_data.append((wslot_rep, wloc_rep, wreg_rep, wscale_rep))

        vf_pos = np.zeros((BPC, WDIM), np.float32)
        vf_pos[ipos_of_rank] = v_feat[iid]
        vfT[c] = vf_pos.T.astype(bf16)

    es = _Sched(cnt_e, ET, NREG_E, EB)
    ws = _Sched(cnt_w, WT, NREG_W, WB)

    eidx = np.zeros((NC, 128, es.S // 16), np.int16)
    eloc = np.zeros((NC, 128, es.NCH), bf16)
    widx = np.zeros((NC, 128, ws.S // 16), np.int16)
    wloc = np.zeros((NC, 128, ws.NCH), bf16)
    wsc = np.zeros((NC, 128, ws.NCH), bf16)
    for c in range(NC):
        slot_rep, loc_rep, reg_rep = e_data[c]
        i_s, l_s, _ = _fill_stream(es, slot_rep, loc_rep, reg_rep)
        eidx[c] = _wrap_idx(i_s)
        eloc[c] = _per_chunk(l_s)
        wslot_rep, wloc_rep, wreg_rep, wscale_rep = w_data[c]
        i_s, l_s, s_s = _fill_stream(ws, wslot_rep, wloc_rep, wreg_rep,
                                     scale=wscale_rep)
        widx[c] = _wrap_idx(i_s)
        wloc[c] = _per_chunk(l_s)
        wsc[c] = _per_chunk(s_s)

    return dict(es=es, ws=ws, eidx=eidx, eloc=eloc,
                widx=widx, wloc=wloc, wsc=wsc, vfT=vfT,
                sidx=sidx, outperm=outperm)


# ------------------------------------------------------------- bass program

def _build_program(es, ws):
    from concourse import bass, bacc, mybir
    import concourse.tile as tile
    dt = mybir.dt

    nc = bacc.Bacc(None, target_bir_lowering=False, num_swdge_queues=4)
    f32 = dt.float32
    bf = dt.bfloat16

    id_in = nc.dram_tensor("id_emb", [NNODE, DIM], f32, kind="ExternalInput")
    wt_in = nc.dram_tensor("wt_bf", [VOCAB, WDIM], bf, kind="ExternalInput")
    eidx_in = nc.dram_tensor("eidx", [128, es.S // 16], dt.int16, kind="ExternalInput")
    eloc_in = nc.dram_tensor("eloc", [128, es.NCH], bf, kind="ExternalInput")
    widx_in = nc.dram_tensor("widx", [128, ws.S // 16], dt.int16, kind="ExternalInput")
    wloc_in = nc.dram_tensor("wloc", [128, ws.NCH], bf, kind="ExternalInput")
    wsc_in = nc.dram_tensor("wsc", [128, ws.NCH], bf, kind="ExternalInput")
    vfT_in = nc.dram_tensor("vfT", [WDIM, BPC], bf, kind="ExternalInput")
    cw_in = nc.dram_tensor("cw_bf", [DIM, DIM], bf, kind="ExternalInput")
    ww_in = nc.dram_tensor("ww_bf", [DIM, DIM], bf, kind="ExternalInput")
    w2_in = nc.dram_tensor("w2_bf", [DIM, DIM], bf, kind="ExternalInput")
    lw_in = nc.dram_tensor("lw_bf", [2 * WDIM, DIM], bf, kind="ExternalInput")
    lb_in = nc.dram_tensor("lb_col", [DIM, 1], f32, kind="ExternalInput")
    ident_in = nc.dram_tensor("ident", [128, 128], f32, kind="ExternalInput")
    sidx_in = nc.dram_tensor("sidx", [128, BPC // 16], dt.int16, kind="ExternalInput")
    iota_in = nc.dram_tensor("iota_bf", [128, 128], bf, kind="ExternalInput")
    out = nc.dram_tensor("scores_w", [128, 8], f32, kind="ExternalOutput")
    x2i_dram = nc.dram_tensor("x2i", [BPC, DIM], f32)

    id_regions = [(E_REG_BOUNDS[i], E_REG_BOUNDS[i + 1]) for i in range(3)]
    wt_regions = [(r * REG_W, (r + 1) * REG_W) for r in range(NREG_W)]

    with tile.TileContext(nc) as tc:
        with tc.tile_pool(name="const", bufs=1) as cpool, \
             tc.tile_pool(name="persist", bufs=1) as pp, \
             tc.tile_pool(name="ewp", bufs=10) as ewp, \
             tc.tile_pool(name="wwp", bufs=9) as wwp, \
             tc.tile_pool(name="mid", bufs=2) as midp, \
             tc.tile_pool(name="xp", bufs=2) as xp, \
             tc.tile_pool(name="psw", bufs=2, space="PSUM") as psw, \
             tc.tile_pool(name="pse", bufs=2, space="PSUM") as pse, \
             tc.tile_pool(name="psm", bufs=2, space="PSUM") as psm:

            iota = cpool.tile([128, 128], bf)
            cw = cpool.tile([DIM, DIM], bf)
            ww = cpool.tile([DIM, DIM], bf)
            w2 = cpool.tile([DIM, DIM], bf)
            lw = cpool.tile([128, 2 * DIM], bf)   # cols 0:64 = v-half, 64:128 = t-half
            lb = cpool.tile([DIM, 1], f32)
            ident = cpool.tile([128, 128], f32)
            sidx_sb = cpool.tile([128, BPC // 16], dt.int16)
            nc.sync.dma_start(out=iota[:], in_=iota_in[:])
            nc.sync.dma_start(out=cw[:], in_=cw_in[:])
            nc.sync.dma_start(out=ww[:], in_=ww_in[:])
            nc.sync.dma_start(out=w2[:], in_=w2_in[:])
            nc.sync.dma_start(out=lw[:, 0:DIM], in_=lw_in[0:128, :])
            nc.sync.dma_start(out=lw[:, DIM:2 * DIM], in_=lw_in[128:256, :])
            nc.sync.dma_start(out=lb[:], in_=lb_in[:])
            nc.sync.dma_start(out=ident[:], in_=ident_in[:])
            nc.sync.dma_start(out=sidx_sb[:], in_=sidx_in[:])
            primer = cpool.tile([128, DIM], f32)
            nc.gpsimd.dma_gather(
                primer[:].rearrange("p (k d) -> p k d", d=DIM),
                id_in[0:25000, :], sidx_sb[:, 0:8],
                128, 128, DIM, single_packet=False)

            eidx_sb = pp.tile([128, es.S // 16], dt.int16)
            eloc_sb = pp.tile([128, es.NCH], bf)
            widx_sb = pp.tile([128, ws.S // 16], dt.int16)
            wloc_sb = pp.tile([128, ws.NCH], bf)
            wsc_sb = pp.tile([128, ws.NCH], bf)
            vfT_sb = pp.tile([WDIM, BPC], bf)
            nc.sync.dma_start(out=eidx_sb[:], in_=eidx_in[:])
            nc.sync.dma_start(out=eloc_sb[:], in_=eloc_in[:])
            nc.sync.dma_start(out=widx_sb[:], in_=widx_in[:])
            nc.sync.dma_start(out=wloc_sb[:], in_=wloc_in[:])
            nc.sync.dma_start(out=wsc_sb[:], in_=wsc_in[:])
            nc.sync.dma_start(out=vfT_sb[:], in_=vfT_in[:])

            tfT_sb = pp.tile([WDIM, IT * 128], bf)
            fT_sb = pp.tile([DIM, IT * 128], bf)
            x2T_sb = pp.tile([DIM, NT * 128], f32)
            tfsum_sb = pp.tile([WDIM, IT * 128], f32)
            agg_sb = pp.tile([DIM, NT * 128], f32)
            nc.vector.memset(tfsum_sb[:], 0.0)
            nc.vector.memset(agg_sb[:], 0.0)

            # ---- words: t_feat^T accumulation ----
            wps = None
            for wq, (r, ch0, nb) in enumerate(ws.batches):
                r0, r1 = wt_regions[r]
                wpay = wwp.tile([128, WB * WDIM], bf, tag="wpay")
                pay3 = wpay[:].rearrange("p (k d) -> p k d", d=WDIM)
                nc.gpsimd.dma_gather(
                    wpay[:, 0:nb * WDIM].rearrange("p (k d) -> p k d", d=WDIM),
                    wt_in[r0:r1, :],
                    widx_sb[:, ch0 * 8:(ch0 + nb) * 8],
                    nb * 128, nb * 128, WDIM, single_packet=False,
                    queue_num=wq % 4)
                woh = wwp.tile([128, WB * SLOTW], bf, tag="woh")
                oh3 = woh[:].rearrange("p (k d) -> p k d", d=SLOTW)
                nc.vector.tensor_tensor(
                    out=oh3[:, 0:nb, :],
                    in0=wloc_sb[:, ch0:ch0 + nb][:, :, None].to_broadcast(
                        [128, nb, SLOTW]),
                    in1=iota[:][:, None, 0:SLOTW].to_broadcast([128, nb, SLOTW]),
                    op=mybir.AluOpType.is_equal)
                nc.vector.tensor_tensor(
                    out=oh3[:, 0:nb, :], in0=oh3[:, 0:nb, :],
                    in1=wsc_sb[:, ch0:ch0 + nb][:, :, None].to_broadcast(
                        [128, nb, SLOTW]),
                    op=mybir.AluOpType.mult)
                for k in range(nb):
                    ch = ch0 + k
                    t = int(ws.tile_of[ch])
                    if ws.is_first[ch]:
                        wps = psw.tile([WDIM, 512], f32, tag="wp")
                    nc.tensor.matmul(
                        out=wps[:, 0:SLOTW], lhsT=pay3[:, k, :], rhs=oh3[:, k, :],
                        start=ws.is_first[ch], stop=ws.is_last[ch])
                    if ws.is_last[ch]:
                        sl = tfsum_sb[:, t * SLOTW:(t + 1) * SLOTW]
                        nc.vector.tensor_tensor(out=sl, in0=sl,
                                                in1=wps[:, 0:SLOTW],
                                                op=mybir.AluOpType.add)

            for t in range(IT):
                nc.scalar.activation(
                    tfT_sb[:, t * 128:(t + 1) * 128],
                    tfsum_sb[:, t * 128:(t + 1) * 128],
                    mybir.ActivationFunctionType.Copy)

            # ---- f^T = lrelu(lw^T cat^T + lb); fh feeds item-tile x2 ----
            for t in range(IT):
                fp = psm.tile([DIM, 512], f32, tag="mm")
                nc.tensor.matmul(out=fp[:, 0:128], lhsT=lw[:, 0:DIM],
                                 rhs=vfT_sb[:, t * 128:(t + 1) * 128],
                                 start=True, stop=False)
                nc.tensor.matmul(out=fp[:, 0:128], lhsT=lw[:, DIM:2 * DIM],
                                 rhs=tfT_sb[:, t * 128:(t + 1) * 128],
                                 start=False, stop=True)
                nc.scalar.activation(
                    fT_sb[:, t * 128:(t + 1) * 128], fp[:, 0:128],
                    mybir.ActivationFunctionType.Lrelu,
                    bias=lb[:], alpha=SLOPE)

            # ---- edges: agg^T accumulation with on-the-fly normalize ----
            es_has_pair = set()
            for (_r, _c0, _nb) in es.batches:
                _k = 0
                while _k < _nb:
                    _ch = _c0 + _k
                    if (_k + 1 < _nb) and not es.is_first[_ch + 1]:
                        es_has_pair.add((int(es.region_of[_ch]),
                                         int(es.tile_of[_ch])))
                        _k += 2
                    else:
                        _k += 1
            if True:
              eps = None
              for eq, (r, ch0, nb) in enumerate(es.batches):
                r0, r1 = id_regions[r]
                epay = ewp.tile([128, EB * DIM], f32, tag="epay")
                pay3 = epay[:].rearrange("p (k d) -> p k d", d=DIM)
                nc.gpsimd.dma_gather(
                    epay[:, 0:nb * DIM].rearrange("p (k d) -> p k d", d=DIM),
                    id_in[r0:r1, :],
                    eidx_sb[:, ch0 * 8:(ch0 + nb) * 8],
                    nb * 128, nb * 128, DIM, single_packet=False,
                    queue_num=eq % 4)
                esq = midp.tile([128, EB * DIM], f32, tag="esq")
                sq3 = esq[:].rearrange("p (k d) -> p k d", d=DIM)
                nc.vector.tensor_tensor(out=sq3[:, 0:nb, :], in0=pay3[:, 0:nb, :],
                                        in1=pay3[:, 0:nb, :],
                                        op=mybir.AluOpType.mult)
                ss = ewp.tile([128, EB], f32, tag="ess")
                nc.vector.reduce_sum(out=ss[:, 0:nb], in_=sq3[:, 0:nb, :],
                                     axis=mybir.AxisListType.X)
                nc.scalar.sqrt(ss[:, 0:nb], ss[:, 0:nb])
                nc.vector.reciprocal(ss[:, 0:nb], ss[:, 0:nb])
                epayb = ewp.tile([128, EB * DIM], bf, tag="epayb")
                payb3 = epayb[:].rearrange("p (k d) -> p k d", d=DIM)
                nc.vector.tensor_tensor(
                    out=payb3[:, 0:nb, :], in0=pay3[:, 0:nb, :],
                    in1=ss[:, 0:nb][:, :, None].to_broadcast([128, nb, DIM]),
                    op=mybir.AluOpType.mult)
                eoh = ewp.tile([128, EB * SLOTW], bf, tag="eoh")
                oh3 = eoh[:].rearrange("p (k d) -> p k d", d=SLOTW)
                nc.vector.tensor_tensor(
                    out=oh3[:, 0:nb, :],
                    in0=eloc_sb[:, ch0:ch0 + nb][:, :, None].to_broadcast(
                        [128, nb, SLOTW]),
                    in1=iota[:][:, None, 0:SLOTW].to_broadcast([128, nb, SLOTW]),
                    op=mybir.AluOpType.is_equal)
                k = 0
                while k < nb:
                    ch = ch0 + k
                    t = int(es.tile_of[ch])
                    if es.is_first[ch]:
                        eps = pse.tile([128, 512], f32, tag="ep")
                    pair = (k + 1 < nb) and not es.is_first[ch + 1]
                    if pair:
                        stop = es.is_last[ch + 1]
                        nc.tensor.matmul(
                            out=eps[:, 0:128],
                            lhsT=epayb[:, k * DIM:(k + 2) * DIM],
                            rhs=eoh[:, k * SLOTW:(k + 2) * SLOTW],
                            start=es.is_first[ch], stop=stop)
                        k += 2
                    else:
                        stop = es.is_last[ch]
                        nc.tensor.matmul(
                            out=eps[0:DIM, 0:SLOTW],
                            lhsT=epayb[:, k * DIM:(k + 1) * DIM],
                            rhs=eoh[:, k * SLOTW:(k + 1) * SLOTW],
                            start=es.is_first[ch], stop=stop)
                        k += 1
                    if stop:
                        g = (int(es.region_of[ch]), t)
                        sl = agg_sb[:, t * SLOTW:(t + 1) * SLOTW]
                        nc.vector.tensor_tensor(out=sl, in0=sl,
                                                in1=eps[0:DIM, 0:SLOTW],
                                                op=mybir.AluOpType.add)
                        if g in es_has_pair:
                            nc.vector.tensor_tensor(
                                out=sl, in0=sl,
                                in1=eps[DIM:128, SLOTW:128],
                                op=mybir.AluOpType.add)

              # ---- node tail: x2^T = lrelu(ww^T x1^T (+ w2^T f^T)) ----
              for t in range(NT):
                aggT = xp.tile([DIM, 128], bf, tag="aggT")
                nc.scalar.activation(aggT[:], agg_sb[:, t * 128:(t + 1) * 128],
                                     mybir.ActivationFunctionType.Copy)
                x1p = psm.tile([DIM, 512], f32, tag="mm")
                nc.tensor.matmul(out=x1p[:, 0:128], lhsT=cw[:], rhs=aggT[:],
                                 start=True, stop=True)
                x1T = xp.tile([DIM, 128], bf, tag="x1T")
                nc.scalar.activation(x1T[:], x1p[:, 0:128],
                                     mybir.ActivationFunctionType.Lrelu,
                                     alpha=SLOPE)
                x2p = psm.tile([DIM, 512], f32, tag="mm")
                nc.tensor.matmul(out=x2p[:, 0:128], lhsT=ww[:], rhs=x1T[:],
                                 start=True, stop=(t < IT))
                if t >= IT:
                    ti = t - IT
                    nc.tensor.matmul(out=x2p[:, 0:128], lhsT=w2[:],
                                     rhs=fT_sb[:, ti * 128:(ti + 1) * 128],
                                     start=False, stop=True)
                nc.scalar.activation(x2T_sb[:, t * 128:(t + 1) * 128], x2p[:, 0:128],
                                     mybir.ActivationFunctionType.Lrelu,
                                     alpha=SLOPE)

            # ---- scores: transpose x2^T tiles to rows, route item rows ----
            x2r_u = pp.tile([128, IT * DIM], f32)
            x2r_i = pp.tile([128, IT * DIM], f32)
            for t in range(IT):
                ps_t = psm.tile([128, 512], f32, tag="tr")
                nc.tensor.transpose(out=ps_t[:, 0:DIM],
                                    in_=x2T_sb[:, t * 128:(t + 1) * 128],
                                    identity=ident[0:DIM, 0:DIM])
                nc.scalar.activation(x2r_u[:, t * DIM:(t + 1) * DIM], ps_t[:, 0:DIM],
                                     mybir.ActivationFunctionType.Copy)
            for t in range(IT):
                ps_t = psm.tile([128, 512], f32, tag="tr")
                nc.tensor.transpose(out=ps_t[:, 0:DIM],
                                    in_=x2T_sb[:, (IT + t) * 128:(IT + t + 1) * 128],
                                    identity=ident[0:DIM, 0:DIM])
                nc.scalar.activation(x2r_i[:, t * DIM:(t + 1) * DIM], ps_t[:, 0:DIM],
                                     mybir.ActivationFunctionType.Copy)
            nc.sync.dma_start(
                out=x2i_dram[:, :].rearrange("(t p) d -> p t d", p=128),
                in_=x2r_i[:].rearrange("p (t d) -> p t d", d=DIM))
            ipay = pp.tile([128, IT * DIM], f32)
            nc.gpsimd.dma_gather(
                ipay[:].rearrange("p (k d) -> p k d", d=DIM),
                x2i_dram[:, :],
                sidx_sb[:],
                BPC, BPC, DIM, single_packet=False)
            prod = pp.tile([128, IT * DIM], f32)
            nc.vector.tensor_tensor(
                out=prod[:].rearrange("p (k d) -> p k d", d=DIM),
                in0=x2r_u[:].rearrange("p (k d) -> p k d", d=DIM),
                in1=ipay[:].rearrange("p (k d) -> p k d", d=DIM),
                op=mybir.AluOpType.mult)
            sc = pp.tile([128, 8], f32)
            nc.vector.reduce_sum(out=sc[:],
                                 in_=prod[:].rearrange("p (k d) -> p k d", d=DIM),
                                 axis=mybir.AxisListType.X)
            nc.sync.dma_start(out=out[:], in_=sc[:])

    nc.finalize()
    return nc


# ------------------------------------------------------------------- kernel

def kernel(**inputs):
    from concourse.bass_utils import run_bass_kernel_spmd

    pr = _prep(inputs)
    es, ws = pr["es"], pr["ws"]
    key = es.key() + ws.key()
    if key not in _CACHE:
        _CACHE[key] = _build_program(es, ws)
    nc = _CACHE[key]

    iota_bf = np.broadcast_to(np.arange(128, dtype=bf16), (128, 128)).copy()
    ident = np.eye(128, dtype=np.float32)
    wt_bf = np.asarray(inputs["word_table"], np.float32).astype(bf16)
    lb_col = np.asarray(inputs["lin_b"], np.float32).reshape(DIM, 1).copy()
    cw_bf = np.asarray(inputs["conv_weight"], np.float32).astype(bf16)
    ww_bf = np.asarray(inputs["weight_W"], np.float32).astype(bf16)
    w2_bf = np.asarray(inputs["weight_2"], np.float32).astype(bf16)
    lw_bf = np.asarray(inputs["lin_w"], np.float32).astype(bf16)
    id_emb = np.ascontiguousarray(np.asarray(inputs["id_embedding"], np.float32))

    in_maps = []
    for c in range(NC):
        in_maps.append({
            "id_emb": id_emb,
            "wt_bf": wt_bf,
            "eidx": pr["eidx"][c],
            "eloc": pr["eloc"][c],
            "widx": pr["widx"][c],
            "wloc": pr["wloc"][c],
            "wsc": pr["wsc"][c],
            "vfT": pr["vfT"][c],
            "cw_bf": cw_bf,
            "ww_bf": ww_bf,
            "w2_bf": w2_bf,
            "lw_bf": lw_bf,
            "lb_col": lb_col,
            "ident": ident,
            "sidx": pr["sidx"][c],
            "iota_bf": iota_bf,
        })
    res = run_bass_kernel_spmd(nc, in_maps, list(range(NC)))
    scores = np.empty(B, np.float32)
    for c in range(NC):
        w = res.results[c]["scores_w"]           # [128, 8]
        sc = np.asarray(w, np.float32).T.ravel()  # sc[position]
        scores[pr["outperm"][c]] = sc
    return scores


kernel.run_traced = None  # set by test harness if needed
